# revision 1
# baseline (speedup 1.0000x reference)
"""Bass/Trainium2 kernel for BiGraphContrastLayer (GNN message passing).

Computes, for two edge lists (pos/neg) over the same node features:
    h_g = PReLU( D_in^-1/2 A_g D_out^-1/2 feats @ W + b )
returning stack([h_pos, h_neg]) of shape [2, N, Dout].

Strategy (8 NeuronCores, SPMD, no collectives), using the linearity
   (D_in^-1/2 A D_out^-1/2 feats) @ W = D_in^-1/2 A (D_out^-1/2 feats W):

  Phase 1 (y-phase): y_g = bf16( ns_g * (feats @ W) ), computed from a
    host-transposed feats (so featsT tiles are directly the matmul lhsT),
    with the per-node ns scale applied on the PSUM->SBUF read. Stored to
    DRAM per graph.
  Phase 2 (gather/aggregate): edges are bucketed by 128-node dst tile and
    sorted by src. dma_gather (int16, 4 row-banks of <=25088 rows) pulls
    y[src] rows for ~128-edge chunks; a one-hot matmul (lhsT = onehot of
    dst offsets) segment-sums each chunk into the dst tile's PSUM
    accumulator. Final nd-scale + PReLU on DVE, store.

  Host does integer index work only: degree bincounts, sorting, bucketing,
  dealing dst tiles to cores so all 8 cores share one instruction stream
  (signature-matched by per-bank chunk counts), building int16 wrapped
  gather indices, and replicating the small params per the sharding hint.
"""

import math
import tempfile
from dataclasses import dataclass

import numpy as np

P = 128   # partitions
D = 128   # feature dim (Din == Dout == 128)
NBANK = 4


# --------------------------------------------------------------------------
# Config
# --------------------------------------------------------------------------
@dataclass
class Config:
    n_nodes: int = 100000
    n_cores: int = 8
    xbatch: int = 8    # feats tiles per y-phase batch
    sg: int = 10       # dst-tile positions per gather supergroup
    y_act_split: bool = True  # pos-graph y scale on ScalarE, neg on DVE
    act_prelu: bool = True    # final nd-scale+PReLU on ScalarE (not in sim)
    oh_gpsimd_mod: int = 0    # every Nth one-hot build goes to GpSimd (0=off)
    gbufs: int = 2            # gather buffer count

    @property
    def t_global(self) -> int:
        return math.ceil(self.n_nodes / P)

    @property
    def n_pad(self) -> int:
        return self.t_global * P

    @property
    def t_core(self) -> int:
        return math.ceil(self.t_global / self.n_cores)

    @property
    def bank_tiles(self) -> int:
        return math.ceil(self.t_global / NBANK)

    @property
    def bank_rows(self) -> int:
        return self.bank_tiles * P


# --------------------------------------------------------------------------
# Host-side preprocessing (integer index manipulation only)
# --------------------------------------------------------------------------
def _row_of(n, cfg: Config):
    """y DRAM row of node n: within each xbatch of tiles, rows are laid
    p-major (node (t0+i)*128+p -> row t0*128 + p*nb + i) so the y-phase
    store writes nb*256B contiguous per partition."""
    xb, tg = cfg.xbatch, cfg.t_global
    t = n // P
    p = n % P
    t0 = (t // xb) * xb
    nb = np.minimum(xb, tg - t0)
    return t0 * P + p * nb + (t - t0)


def _plan_graph(src, dst, cfg: Config):
    """Bucket edges by dst tile, sort by src within tile, bank-split, and
    deal tiles to cores with per-bank-signature matching.

    Returns dict with:
      core_tiles  [n_cores, t_core]  global tile id per position (-1 null)
      cstar       [t_core, NBANK]    shared chunk counts per position/bank
      tile_edges  list per global tile: (src_sorted, off_sorted, bank_cnt)
    """
    tg, ncores, tcore = cfg.t_global, cfg.n_cores, cfg.t_core
    order = np.argsort(dst, kind="stable")
    src_s = src[order]
    dst_s = dst[order]
    tile_cnt = np.bincount(dst_s // P, minlength=tg)
    starts = np.zeros(tg + 1, np.int64)
    np.cumsum(tile_cnt, out=starts[1:])

    row_s = _row_of(src_s, cfg)
    bank_of = row_s // cfg.bank_rows
    sig = np.zeros((tg, NBANK), np.int64)
    tile_edges = []
    for t in range(tg):
        e0, e1 = int(starts[t]), int(starts[t + 1])
        so = np.argsort(row_s[e0:e1], kind="stable")
        ts_src = row_s[e0:e1][so]
        ts_off = (dst_s[e0:e1][so] % P).astype(np.int64)
        bc = np.bincount(bank_of[e0:e1], minlength=NBANK)
        sig[t] = -(-bc // P)  # ceil chunks per bank
        sig[t, 0] = max(sig[t, 0], 1)
        tile_edges.append((ts_src, ts_off, bc))

    # Deal: sort tiles by signature so consecutive groups of n_cores tiles
    # have matching/near-matching per-bank chunk counts.
    sigkey = sig @ (np.int64(32) ** np.arange(NBANK - 1, -1, -1))
    keys = np.argsort(sigkey, kind="stable")
    # pad with null tiles to n_cores * t_core
    n_slots = ncores * tcore
    dealt = np.full(n_slots, -1, np.int64)
    dealt[: len(keys)] = keys[::-1]  # descending signature order
    core_tiles = np.zeros((ncores, tcore), np.int64)
    cstar = np.zeros((tcore, NBANK), np.int64)
    for k in range(tcore):
        grp = dealt[k * ncores : (k + 1) * ncores]
        for c in range(ncores):
            core_tiles[c, k] = grp[c]
        s = np.zeros(NBANK, np.int64)
        for t in grp:
            if t >= 0:
                s = np.maximum(s, sig[t])
        s[0] = max(s[0], 1)
        cstar[k] = s
    return dict(core_tiles=core_tiles, cstar=cstar, tile_edges=tile_edges)


def _slot_layout(cstar, cfg: Config):
    """Shared (all-cores) slot layout for one graph.

    Slot space = sequence of supergroups; within a supergroup, bank-major:
      for b in banks: for k in sg positions: cstar[k, b] chunks.
    Returns:
      sg_list: list of (k0, kn)
      calls:   list of (sg_idx, bank, chunk0, nchunks)  [nchunks > 0]
      chunk_of: dict (k, b, c) -> global chunk index
      n_chunks total
    """
    tcore, sg = cfg.t_core, cfg.sg
    sg_list = []
    k0 = 0
    while k0 < tcore:
        kn = min(sg, tcore - k0)
        sg_list.append((k0, kn))
        k0 += kn
    calls = []
    chunk_of = {}
    cpos = 0
    for si, (k0, kn) in enumerate(sg_list):
        for b in range(NBANK):
            c0 = cpos
            for ki in range(kn):
                k = k0 + ki
                for c in range(int(cstar[k, b])):
                    chunk_of[(k, b, c)] = cpos
                    cpos += 1
            if cpos > c0:
                calls.append((si, b, c0, cpos - c0))
    return sg_list, calls, chunk_of, cpos


def _fill_core_graph(plan, layout, core, cfg: Config):
    """Build IDX16 (wrapped) and OFF arrays for one core, one graph."""
    sg_list, calls, chunk_of, n_chunks = layout
    cstar = plan["cstar"]
    idx = np.zeros((n_chunks, P), np.int16)
    off = np.full((n_chunks, P), 512.0, np.float32)
    for k in range(cfg.t_core):
        t = plan["core_tiles"][core, k]
        if t < 0:
            continue
        ts_src, ts_off, bc = plan["tile_edges"][t]
        bstart = np.zeros(NBANK + 1, np.int64)
        np.cumsum(bc, out=bstart[1:])
        for b in range(NBANK):
            nb = int(bc[b])
            cnum = int(cstar[k, b])
            if cnum == 0:
                continue
            nslot = cnum * P
            es = np.zeros(nslot, np.int64)
            eo = np.full(nslot, 512.0, np.float32)
            es[:nb] = ts_src[bstart[b] : bstart[b + 1]] - b * cfg.bank_rows
            eo[:nb] = ts_off[bstart[b] : bstart[b + 1]]
            for c in range(cnum):
                ci = chunk_of[(k, b, c)]
                idx[ci] = es[c * P : (c + 1) * P]
                off[ci] = eo[c * P : (c + 1) * P]
    # wrap: flat slot i (within a call's range) -> [i%16, i//16], replicated
    # to 128 partitions. Build per call, concatenated along columns.
    ncols = n_chunks * P // 16
    idx_w = np.zeros((P, ncols), np.int16)
    for (si, b, c0, nch) in calls:
        flat = idx[c0 : c0 + nch].reshape(-1)  # ni slots
        blk = flat.reshape(-1, 16).T  # [16, ni/16]
        idx_w[:, c0 * 8 : (c0 + nch) * 8] = np.tile(blk, (8, 1))
    return idx_w, off.T.copy()  # off -> [P, n_chunks] f32


def preprocess(feats, W, b, prelu_a, src_pos, dst_pos, src_neg, dst_neg,
               cfg: Config):
    n, ncores, tcore, tg = cfg.n_nodes, cfg.n_cores, cfg.t_core, cfg.t_global
    feats = np.asarray(feats, np.float32)
    W = np.asarray(W, np.float32)
    b = np.asarray(b, np.float32)
    prelu_a = np.asarray(prelu_a, np.float32)

    feats_pad = np.zeros((cfg.n_pad, D), np.float32)
    feats_pad[:n] = feats
    featsT = np.ascontiguousarray(feats_pad.T)  # [D, n_pad]

    plans, layouts, deg_outs, deg_ins = [], [], [], []
    for src, dst in ((src_pos, dst_pos), (src_neg, dst_neg)):
        src = np.asarray(src, np.int64)
        dst = np.asarray(dst, np.int64)
        deg_outs.append(np.bincount(src, minlength=n).astype(np.int32))
        deg_ins.append(np.bincount(dst, minlength=n).astype(np.int32))
        plan = _plan_graph(src, dst, cfg)
        plans.append(plan)
        layouts.append(_slot_layout(plan["cstar"], cfg))

    dego_arr = np.zeros((P, 2 * tg), np.int32)
    for g in range(2):
        dpad = np.zeros(cfg.n_pad, np.int32)
        dpad[:n] = deg_outs[g]
        dego_arr[:, g * tg : (g + 1) * tg] = dpad.reshape(tg, P).T

    degi_arr = np.zeros((ncores, P, 2 * tcore), np.int32)
    for g in range(2):
        dpad = np.zeros(cfg.n_pad, np.int32)
        dpad[:n] = deg_ins[g]
        dpad_t = dpad.reshape(tg, P).T
        for core in range(ncores):
            ct = plans[g]["core_tiles"][core]
            valid = ct >= 0
            degi_arr[core, :, g * tcore : (g + 1) * tcore][:, valid] = (
                dpad_t[:, ct[valid]])

    iota = np.tile(np.arange(P, dtype=np.float32), (P, 1)).astype(
        np.dtype("bfloat16"))
    a_rep = np.full((P, 1), float(prelu_a.reshape(-1)[0]), np.float32)
    b_rep = np.tile(b.reshape(1, D), (P, 1)).astype(np.float32)

    in_maps = []
    for core in range(ncores):
        iw_p, off_p = _fill_core_graph(plans[0], layouts[0], core, cfg)
        iw_n, off_n = _fill_core_graph(plans[1], layouts[1], core, cfg)
        in_maps.append({
            "featst": featsT,
            "w_in": W,
            "a_rep": a_rep,
            "b_rep": b_rep,
            "dego": dego_arr,
            "degi": degi_arr[core],
            "idx_in": np.concatenate([iw_p, iw_n], axis=1),
            "off_in": np.concatenate([off_p, off_n], axis=1),
            "iota_in": iota,
        })
    meta = {
        "layouts": layouts,
        "cstar": [plans[0]["cstar"], plans[1]["cstar"]],
        "use_bias": bool(np.any(b != 0.0)),
    }
    return in_maps, plans, meta


# --------------------------------------------------------------------------
# Device kernel builder
# --------------------------------------------------------------------------
def build_kernel(nc, tc, cfg: Config, meta):
    from contextlib import ExitStack

    import concourse.mybir as mybir

    f32 = mybir.dt.float32
    bf16 = mybir.dt.bfloat16
    i32 = mybir.dt.int32
    i16 = mybir.dt.int16
    Alu = mybir.AluOpType
    Act = mybir.ActivationFunctionType

    tg, tcore, npad = cfg.t_global, cfg.t_core, cfg.n_pad
    layouts = meta["layouts"]
    cstar = meta["cstar"]
    use_bias = meta["use_bias"]
    n_chunks = [layouts[g][3] for g in range(2)]
    ncols = [n_chunks[g] * P // 16 for g in range(2)]

    featst = nc.dram_tensor("featst", [P, npad], f32, kind="ExternalInput").ap()
    w_in = nc.dram_tensor("w_in", [P, D], f32, kind="ExternalInput").ap()
    a_rep = nc.dram_tensor("a_rep", [P, 1], f32, kind="ExternalInput").ap()
    b_rep = nc.dram_tensor("b_rep", [P, D], f32, kind="ExternalInput").ap()
    dego = nc.dram_tensor("dego", [P, 2 * tg], i32, kind="ExternalInput").ap()
    degi = nc.dram_tensor("degi", [P, 2 * tcore], i32, kind="ExternalInput").ap()
    idx_in = nc.dram_tensor("idx_in", [P, sum(ncols)], i16,
                            kind="ExternalInput").ap()
    off_in = nc.dram_tensor("off_in", [P, sum(n_chunks)], f32,
                            kind="ExternalInput").ap()
    iota_in = nc.dram_tensor("iota_in", [P, P], bf16, kind="ExternalInput").ap()
    out = nc.dram_tensor("out", [2, tcore, P, D], f32, kind="ExternalOutput").ap()

    y_dram = [nc.dram_tensor(f"y{g}", [npad, D], bf16, kind="Internal").ap()
              for g in range(2)]

    with ExitStack() as ctx:
        const = ctx.enter_context(tc.tile_pool(name="const", bufs=1))
        work = ctx.enter_context(tc.tile_pool(name="work", bufs=2))
        xpool = ctx.enter_context(tc.tile_pool(name="xpool", bufs=3))
        mpool = ctx.enter_context(tc.tile_pool(name="mpool", bufs=3))
        gpool = ctx.enter_context(tc.tile_pool(name="gpool", bufs=cfg.gbufs))
        import os as _os
        ipool = ctx.enter_context(tc.tile_pool(
            name="ipool", bufs=int(_os.environ.get("IPB", "3"))))
        ohpool = ctx.enter_context(tc.tile_pool(name="ohpool", bufs=6))
        tpool = ctx.enter_context(tc.tile_pool(name="tpool", bufs=4))
        spool = ctx.enter_context(tc.tile_pool(name="spool", bufs=3))
        ypool = ctx.enter_context(tc.tile_pool(
            name="ypool", bufs=int(_os.environ.get("YPB", "4")), space="PSUM"))
        ppool = ctx.enter_context(tc.tile_pool(
            name="ppool", bufs=int(_os.environ.get("PPB", "4")), space="PSUM"))

        # ---- constants ----
        w_sb = const.tile([P, D], bf16)
        nc.gpsimd.dma_start(out=w_sb[:], in_=w_in)  # f32 -> bf16 cast DMA
        iota_sb = const.tile([P, P], bf16)
        nc.sync.dma_start(out=iota_sb[:], in_=iota_in)
        a_sb = const.tile([P, 1], f32)
        nc.sync.dma_start(out=a_sb[:], in_=a_rep)
        if use_bias:
            b_sb = const.tile([P, D], f32)
            nc.sync.dma_start(out=b_sb[:], in_=b_rep)

        # ---- norms from degrees:  norm = (deg>0) / sqrt(max(deg,1)) ----
        def make_norm(deg_ap, width, tagn):
            dg = work.tile([P, width], i32, tag=f"dg{tagn}")
            nc.sync.dma_start(out=dg[:], in_=deg_ap)
            f = work.tile([P, width], f32, tag=f"f{tagn}")
            nc.vector.tensor_copy(out=f[:], in_=dg[:])
            m = work.tile([P, width], f32, tag=f"m{tagn}")
            nc.vector.tensor_scalar(out=m[:], in0=f[:], scalar1=1.0,
                                    scalar2=None, op0=Alu.max)
            r = work.tile([P, width], f32, tag=f"r{tagn}")
            nc.vector.reciprocal(out=r[:], in_=m[:])
            s = work.tile([P, width], f32, tag=f"s{tagn}")
            nc.scalar.activation(out=s[:], in_=r[:], func=Act.Sqrt)
            z = work.tile([P, width], f32, tag=f"z{tagn}")
            nc.vector.tensor_scalar(out=z[:], in0=f[:], scalar1=1.0,
                                    scalar2=None, op0=Alu.min)
            ns = const.tile([P, width], f32, tag=f"o{tagn}")
            nc.vector.tensor_tensor(out=ns[:], in0=s[:], in1=z[:], op=Alu.mult)
            return ns

        ns_sb = make_norm(dego, 2 * tg, "o")       # out-deg norms, all nodes
        nd_sb = make_norm(degi, 2 * tcore, "i")    # in-deg norms, owned slots
        and_sb = const.tile([P, 2 * tcore], f32)
        nc.vector.tensor_tensor(out=and_sb[:], in0=nd_sb[:],
                                in1=a_sb[:, :1].to_broadcast([P, 2 * tcore]),
                                op=Alu.mult)

        # ---- y-phase: y_g = bf16(ns_g * (feats @ W)) ----
        t0 = 0
        while t0 < tg:
            nb = min(cfg.xbatch, tg - t0)
            ld = xpool.tile([P, nb * P], f32, tag="xload")
            nc.sync.dma_start(out=ld[:], in_=featst[:, t0 * P : (t0 + nb) * P])
            ldb = xpool.tile([P, nb * P], bf16, tag="xcast")
            nc.vector.tensor_copy(out=ldb[:], in_=ld[:])
            ybuf0 = xpool.tile([P, nb, D], bf16, tag="ybuf0")
            ybuf1 = xpool.tile([P, nb, D], bf16, tag="ybuf1")
            ybuf = [ybuf0, ybuf1]
            for i in range(nb):
                psy = ypool.tile([P, D], f32)
                nc.tensor.matmul(out=psy[:], lhsT=ldb[:, i * P : (i + 1) * P],
                                 rhs=w_sb[:], start=True, stop=True)
                col = t0 + i
                if cfg.y_act_split:
                    nc.scalar.activation(out=ybuf[0][:, i, :], in_=psy[:],
                                         func=Act.Copy,
                                         scale=ns_sb[:, col : col + 1])
                else:
                    nc.vector.tensor_scalar(out=ybuf[0][:, i, :], in0=psy[:],
                                            scalar1=ns_sb[:, col : col + 1],
                                            scalar2=None, op0=Alu.mult)
                nc.vector.tensor_scalar(out=ybuf[1][:, i, :], in0=psy[:],
                                        scalar1=ns_sb[:, tg + col : tg + col + 1],
                                        scalar2=None, op0=Alu.mult)
            for g in range(2):
                nc.sync.dma_start(
                    out=y_dram[g][t0 * P : (t0 + nb) * P, :].rearrange(
                        "(p i) d -> p i d", i=nb),
                    in_=ybuf[g][:])
            t0 += nb

        # ---- gather + one-hot segment-sum + nd-scale + prelu ----
        col_base = [0, ncols[0]]          # idx column offset per graph
        chk_base = [0, n_chunks[0]]       # off column offset per graph
        cbs_all = []
        for g in range(2):
            calls_by_sg = {}
            for (si, b, c0, nch) in layouts[g][1]:
                calls_by_sg.setdefault(si, []).append((b, c0, nch))
            cbs_all.append(calls_by_sg)
        # interleave the two graphs' supergroups so one graph's gathers fill
        # DMA while the other's PSUM chain drains
        jobs = []
        for si in range(max(len(layouts[0][0]), len(layouts[1][0]))):
            for g in range(2):
                if si < len(layouts[g][0]):
                    jobs.append((g, si))
        for (g, si) in jobs:
            sg_list, calls, chunk_of, _ = layouts[g]
            cs = cstar[g]
            calls_by_sg = cbs_all[g]
            if True:
                (k0, kn) = sg_list[si]
                sg_chunks = sum(int(cs[k0 + ki, b]) for ki in range(kn)
                                for b in range(NBANK))
                c0_sg = chunk_of[(k0, 0, 0)]
                gt = gpool.tile([P, sg_chunks, D], bf16, tag="gather")
                it = ipool.tile([P, sg_chunks * 8], i16, tag="gidx")
                nc.sync.dma_start(
                    out=it[:],
                    in_=idx_in[:, col_base[g] + c0_sg * 8 :
                               col_base[g] + (c0_sg + sg_chunks) * 8])
                ot = ipool.tile([P, sg_chunks], f32, tag="goff")
                nc.sync.dma_start(
                    out=ot[:],
                    in_=off_in[:, chk_base[g] + c0_sg :
                               chk_base[g] + c0_sg + sg_chunks])
                for (b, c0, nch) in calls_by_sg[si]:
                    lo = c0 - c0_sg
                    bank_rows = min(cfg.bank_rows, npad - b * cfg.bank_rows)
                    nc.gpsimd.dma_gather(
                        out_ap=gt[:, lo : lo + nch, :],
                        in_ap=y_dram[g][b * cfg.bank_rows :
                                        b * cfg.bank_rows + bank_rows, :],
                        idxs_ap=it[:, lo * 8 : (lo + nch) * 8],
                        num_idxs=nch * P, num_idxs_reg=nch * P,
                        elem_size=D, single_packet=False)
                stg = spool.tile([P, kn, D], f32, tag="stg")
                for ki in range(kn):
                    k = k0 + ki
                    nonzero = [(b, c) for b in range(NBANK)
                               for c in range(int(cs[k, b]))]
                    ps_a = ppool.tile([P, D], f32)
                    for j, (b, c) in enumerate(nonzero):
                        ci = chunk_of[(k, b, c)]
                        lo = ci - c0_sg
                        oh = ohpool.tile([P, P], bf16)
                        eng = nc.vector
                        if cfg.oh_gpsimd_mod and (ci % cfg.oh_gpsimd_mod == 0):
                            eng = nc.gpsimd
                        eng.tensor_scalar(
                            out=oh[:], in0=iota_sb[:],
                            scalar1=ot[:, lo : lo + 1],
                            scalar2=None, op0=Alu.is_equal)
                        nc.tensor.matmul(
                            out=ps_a[:], lhsT=oh[:], rhs=gt[:, lo, :],
                            start=(j == 0), stop=(j == len(nonzero) - 1))
                    kslot = g * tcore + k
                    if cfg.act_prelu and not use_bias:
                        nc.scalar.activation(
                            out=stg[:, ki, :], in_=ps_a[:], func=Act.Prelu,
                            scale=nd_sb[:, kslot : kslot + 1],
                            alpha=a_sb[:, :1])
                        continue
                    if use_bias:
                        hb = tpool.tile([P, D], f32, tag="hb")
                        nc.vector.tensor_scalar(
                            out=hb[:], in0=ps_a[:],
                            scalar1=nd_sb[:, kslot : kslot + 1],
                            scalar2=None, op0=Alu.mult)
                        hb2 = tpool.tile([P, D], f32, tag="hb2")
                        nc.vector.tensor_tensor(out=hb2[:], in0=hb[:],
                                                in1=b_sb[:], op=Alu.add)
                        neg = tpool.tile([P, D], f32, tag="neg")
                        nc.vector.tensor_scalar(
                            out=neg[:], in0=hb2[:], scalar1=0.0,
                            scalar2=a_sb[:, :1], op0=Alu.min, op1=Alu.mult)
                        pos = tpool.tile([P, D], f32, tag="pos")
                        nc.vector.tensor_scalar(
                            out=pos[:], in0=hb2[:], scalar1=0.0,
                            scalar2=None, op0=Alu.max)
                    else:
                        neg = tpool.tile([P, D], f32, tag="neg")
                        nc.vector.tensor_scalar(
                            out=neg[:], in0=ps_a[:], scalar1=0.0,
                            scalar2=and_sb[:, kslot : kslot + 1],
                            op0=Alu.min, op1=Alu.mult)
                        pos = tpool.tile([P, D], f32, tag="pos")
                        nc.vector.tensor_scalar(
                            out=pos[:], in0=ps_a[:], scalar1=0.0,
                            scalar2=nd_sb[:, kslot : kslot + 1],
                            op0=Alu.max, op1=Alu.mult)
                    nc.vector.tensor_tensor(out=stg[:, ki, :], in0=neg[:],
                                            in1=pos[:], op=Alu.add)
                nc.sync.dma_start(
                    out=out[g, k0 : k0 + kn, :, :].rearrange("k p d -> p k d"),
                    in_=stg[:])
    return out


# --------------------------------------------------------------------------
# Driver
# --------------------------------------------------------------------------
def _build_program(cfg: Config, meta):
    import concourse.bacc as bacc
    import concourse.tile as tile

    nc = bacc.Bacc("TRN2", target_bir_lowering=False, debug=False,
                   enable_asserts=False, num_devices=cfg.n_cores)
    with tile.TileContext(nc) as tc:
        build_kernel(nc, tc, cfg, meta)
    nc.compile()
    return nc


def _unscramble(results, plans, cfg: Config):
    n = cfg.n_nodes
    full = np.zeros((2, n, D), np.float32)
    for g in range(2):
        ct_all = plans[g]["core_tiles"]
        for core in range(cfg.n_cores):
            oc = results[core]["out"]  # [2, t_core, P, D]
            for k in range(cfg.t_core):
                t = int(ct_all[core, k])
                if t < 0:
                    continue
                r0 = t * P
                r1 = min(r0 + P, n)
                full[g, r0:r1] = oc[g, k, : r1 - r0, :]
    return full


_PROGRAM_CACHE = {}


def run(inputs, cfg: Config, trace=False):
    from concourse.bass_utils import run_bass_kernel_spmd

    in_maps, plans, meta = preprocess(
        inputs["feats"], inputs["W"], inputs["b"], inputs["prelu_a"],
        inputs["src_pos"], inputs["dst_pos"],
        inputs["src_neg"], inputs["dst_neg"], cfg)

    key = (cfg.n_nodes, cfg.n_cores, cfg.xbatch, cfg.sg, cfg.y_act_split,
           cfg.act_prelu, cfg.oh_gpsimd_mod, cfg.gbufs,
           meta["cstar"][0].tobytes(), meta["cstar"][1].tobytes(),
           meta["use_bias"])
    nc = _PROGRAM_CACHE.get(key)
    if nc is None:
        nc = _build_program(cfg, meta)
        _PROGRAM_CACHE[key] = nc

    kwargs = {}
    if trace:
        kwargs = dict(trace=True, tmpdir=tempfile.mkdtemp(prefix="bgc_trace_"))
    res = run_bass_kernel_spmd(nc, in_maps, core_ids=list(range(cfg.n_cores)),
                               **kwargs)
    full = _unscramble(res.results, plans, cfg)
    return full, res


def kernel(**inputs) -> np.ndarray:
    cfg = Config()
    full, _ = run(inputs, cfg)
    return full



# revision 17
# speedup vs baseline: 1.8444x; 1.8444x over previous
"""Bass/Trainium2 kernel for BiGraphContrastLayer (GNN message passing).

Computes, for two edge lists (pos/neg) over the same node features:
    h_g = PReLU( D_in^-1/2 A_g D_out^-1/2 feats @ W + b )
returning stack([h_pos, h_neg]) of shape [2, N, Dout].

Strategy (8 NeuronCores, SPMD, no collectives). Uses linearity twice:
    nd_d * sum_e ns_s feats_s  =  sum_e (ns_s nd_d) feats_s
    (agg) @ W                  =  W applied once per dst tile after agg

  Single device pass: edges are bucketed by 128-node dst tile and sorted by
  src within (tile, row-bank); dma_gather (int16 idx over 4 row-bank views
  of one bf16 feats table) pulls feats[src] rows for ~128-edge chunks; a
  weighted one-hot matmul (lhsT = gathered chunk, rhs = onehot of dst offsets
  scaled by the per-edge weight ns_src*nd_dst) accumulates the TRANSPOSED
  aggregate aggT[din, dst] for each dst tile in PSUM; aggT -> bf16 SBUF
  (batched Act copy over 4 tiles), one matmul per tile applies W, and a
  batched PReLU produces the output tile.

  Host does index prep (degree bincounts -> per-edge norm weights, sorting,
  bucketing, dealing dst tiles to cores so all 8 cores share one instruction
  stream) and stages the bf16 feats table; all O(E*D) and O(N*D^2) work runs
  on device.
"""

import math
import tempfile
from dataclasses import dataclass

import numpy as np

P = 128   # partitions
D = 128   # feature dim (Din == Dout == 128)
NBANK = 4  # row-range banks (int16 gather idx addresses <=32768 rows)


# --------------------------------------------------------------------------
# Config
# --------------------------------------------------------------------------
@dataclass
class Config:
    n_nodes: int = 100000
    n_cores: int = 8
    sg: int = 10       # dst-tile positions per gather supergroup
    act_prelu: bool = True    # final PReLU on ScalarE (not in sim)
    act_batch: bool = True    # batch aggT copy / prelu over 4 tiles
    oh_pool_mod: int = 0      # every Nth one-hot build goes to GpSimd (0=off)
    maxc: int = 24            # max chunks per dma_gather call
    gbufs: int = 2            # gather buffer count
    ipool_bufs: int = 3
    ohpool_bufs: int = 6

    @property
    def t_global(self) -> int:
        return math.ceil(self.n_nodes / P)

    @property
    def n_pad(self) -> int:
        return self.t_global * P

    @property
    def t_core(self) -> int:
        return math.ceil(self.t_global / self.n_cores)

    @property
    def bank_rows(self) -> int:
        return math.ceil(self.t_global / NBANK) * P


# --------------------------------------------------------------------------
# Host-side preprocessing
# --------------------------------------------------------------------------
def _norm(deg):
    deg = deg.astype(np.float64)
    return np.where(deg > 0, 1.0 / np.sqrt(np.maximum(deg, 1.0)), 0.0).astype(
        np.float32)


def _plan_graph(src, dst, wgt, cfg: Config):
    """Bucket edges by dst tile, split by src row-bank, sort by src within
    (tile, bank), and deal tiles to cores with signature matching.

    Returns dict with:
      core_tiles  [n_cores, t_core]  global tile id per position (-1 null)
      cstar       [t_core, NBANK]    shared chunk counts per position/bank
      tile_edges  list per global tile: (idx16, off, wgt, bank_cnt)
    """
    tg, ncores, tcore = cfg.t_global, cfg.n_cores, cfg.t_core
    order = np.argsort(dst, kind="stable")
    src_s = src[order]
    dst_s = dst[order]
    wgt_s = wgt[order]
    tile_cnt = np.bincount(dst_s // P, minlength=tg)
    starts = np.zeros(tg + 1, np.int64)
    np.cumsum(tile_cnt, out=starts[1:])

    sig = np.zeros((tg, NBANK), np.int64)
    tile_edges = []
    for t in range(tg):
        e0, e1 = int(starts[t]), int(starts[t + 1])
        ts_src = src_s[e0:e1]
        bank = ts_src // cfg.bank_rows
        so = np.argsort(bank * (2 ** 40) + ts_src, kind="stable")
        ts_src = ts_src[so]
        ts_idx = ts_src - bank[so] * cfg.bank_rows
        ts_off = (dst_s[e0:e1][so] % P).astype(np.float32)
        ts_wgt = wgt_s[e0:e1][so]
        bc = np.bincount(bank[so], minlength=NBANK)
        sig[t] = -(-bc // P)  # ceil chunks per bank
        sig[t, 0] = max(sig[t, 0], 1)
        tile_edges.append((ts_idx, ts_off, ts_wgt, bc))

    # Deal: sort tiles by signature so consecutive groups of n_cores tiles
    # have matching/near-matching per-bank chunk counts.
    sigkey = sig @ (np.int64(32) ** np.arange(NBANK - 1, -1, -1))
    keys = np.argsort(sigkey, kind="stable")
    n_slots = ncores * tcore
    dealt = np.full(n_slots, -1, np.int64)
    dealt[: len(keys)] = keys[::-1]  # descending signature order
    core_tiles = np.zeros((ncores, tcore), np.int64)
    cstar = np.zeros((tcore, NBANK), np.int64)
    for k in range(tcore):
        grp = dealt[k * ncores : (k + 1) * ncores]
        for c in range(ncores):
            core_tiles[c, k] = grp[c]
        s = np.zeros(NBANK, np.int64)
        for t in grp:
            if t >= 0:
                s = np.maximum(s, sig[t])
        s[0] = max(s[0], 1)
        cstar[k] = s
    return dict(core_tiles=core_tiles, cstar=cstar, tile_edges=tile_edges)


def _slot_layout(cstar, cfg: Config):
    """Shared (all-cores) slot layout for one graph.

    Slot space = sequence of supergroups; within a supergroup, bank-major:
      for b in banks: for k in sg positions: cstar[k, b] chunks.
    Returns:
      sg_list: list of (k0, kn)
      calls:   list of (sg_idx, bank, chunk0, nchunks)  [nchunks > 0]
      chunk_of: dict (k, b, c) -> global chunk index
      n_chunks total
    """
    tcore, sg = cfg.t_core, cfg.sg
    sg_list = []
    k0 = 0
    while k0 < tcore:
        kn = min(sg, tcore - k0)
        sg_list.append((k0, kn))
        k0 += kn
    calls = []
    chunk_of = {}
    cpos = 0
    for si, (k0, kn) in enumerate(sg_list):
        for b in range(NBANK):
            c0 = cpos
            for ki in range(kn):
                k = k0 + ki
                for c in range(int(cstar[k, b])):
                    chunk_of[(k, b, c)] = cpos
                    cpos += 1
            while c0 < cpos:
                nch = min(cfg.maxc, cpos - c0)
                calls.append((si, b, c0, nch))
                c0 += nch
    return sg_list, calls, chunk_of, cpos


def _fill_core_graph(plan, layout, core, cfg: Config):
    """Build IDX16 (wrapped), OFF, and WGT arrays for one core, one graph."""
    sg_list, calls, chunk_of, n_chunks = layout
    cstar = plan["cstar"]
    idx = np.zeros((n_chunks, P), np.int16)
    off = np.full((n_chunks, P), 512.0, np.float32)
    wgt = np.zeros((n_chunks, P), np.float32)
    for k in range(cfg.t_core):
        t = plan["core_tiles"][core, k]
        if t < 0:
            continue
        ts_idx, ts_off, ts_wgt, bc = plan["tile_edges"][t]
        bstart = np.zeros(NBANK + 1, np.int64)
        np.cumsum(bc, out=bstart[1:])
        for b in range(NBANK):
            nb = int(bc[b])
            cnum = int(cstar[k, b])
            if cnum == 0:
                continue
            nslot = cnum * P
            es = np.zeros(nslot, np.int64)
            eo = np.full(nslot, 512.0, np.float32)
            ew = np.zeros(nslot, np.float32)
            es[:nb] = ts_idx[bstart[b] : bstart[b + 1]]
            eo[:nb] = ts_off[bstart[b] : bstart[b + 1]]
            ew[:nb] = ts_wgt[bstart[b] : bstart[b + 1]]
            for c in range(cnum):
                ci = chunk_of[(k, b, c)]
                idx[ci] = es[c * P : (c + 1) * P]
                off[ci] = eo[c * P : (c + 1) * P]
                wgt[ci] = ew[c * P : (c + 1) * P]
    # wrap: flat slot i (within a call's range) -> [i%16, i//16], replicated
    # to 128 partitions. Build per call, concatenated along columns.
    ncols = n_chunks * P // 16
    idx_w = np.zeros((P, ncols), np.int16)
    for (si, b, c0, nch) in calls:
        flat = idx[c0 : c0 + nch].reshape(-1)  # ni slots
        blk = flat.reshape(-1, 16).T  # [16, ni/16]
        idx_w[:, c0 * 8 : (c0 + nch) * 8] = np.tile(blk, (8, 1))
    return idx_w, off.T.copy(), wgt.T.copy()  # -> [P, n_chunks] f32


def preprocess(feats, W, b, prelu_a, src_pos, dst_pos, src_neg, dst_neg,
               cfg: Config):
    n = cfg.n_nodes
    feats = np.asarray(feats, np.float32)
    W = np.asarray(W, np.float32)
    b = np.asarray(b, np.float32)
    prelu_a = np.asarray(prelu_a, np.float32)
    bf16 = np.dtype("bfloat16")

    feats_pad = np.zeros((NBANK * cfg.bank_rows, D), np.float32)
    feats_pad[:n] = feats
    fb16 = feats_pad.astype(bf16)

    plans, layouts = [], []
    for src, dst in ((src_pos, dst_pos), (src_neg, dst_neg)):
        src = np.asarray(src, np.int64)
        dst = np.asarray(dst, np.int64)
        ns = _norm(np.bincount(src, minlength=n))
        nd = _norm(np.bincount(dst, minlength=n))
        wgt = (ns[src] * nd[dst]).astype(np.float32)
        plan = _plan_graph(src, dst, wgt, cfg)
        plans.append(plan)
        layouts.append(_slot_layout(plan["cstar"], cfg))

    iota = np.tile(np.arange(P, dtype=np.float32), (P, 1)).astype(bf16)
    a_rep = np.full((P, 1), float(prelu_a.reshape(-1)[0]), np.float32)
    b_rep = np.tile(b.reshape(1, D), (P, 1)).astype(np.float32)
    use_bias = bool(np.any(b != 0.0))

    in_maps = []
    for core in range(cfg.n_cores):
        iw_p, off_p, wgt_p = _fill_core_graph(plans[0], layouts[0], core, cfg)
        iw_n, off_n, wgt_n = _fill_core_graph(plans[1], layouts[1], core, cfg)
        m = {
            "fb16": fb16,
            "w_in": W,
            "a_rep": a_rep,
            "idx_in": np.concatenate([iw_p, iw_n], axis=1),
            "off_in": np.concatenate([off_p, off_n], axis=1),
            "wgt_in": np.concatenate([wgt_p, wgt_n], axis=1),
            "iota_in": iota,
        }
        if use_bias:
            m["b_rep"] = b_rep
        in_maps.append(m)
    meta = {
        "layouts": layouts,
        "cstar": [plans[0]["cstar"], plans[1]["cstar"]],
        "use_bias": use_bias,
    }
    return in_maps, plans, meta


# --------------------------------------------------------------------------
# Device kernel builder
# --------------------------------------------------------------------------
def build_kernel(nc, tc, cfg: Config, meta):
    from contextlib import ExitStack

    import concourse.mybir as mybir

    f32 = mybir.dt.float32
    bf16 = mybir.dt.bfloat16
    i16 = mybir.dt.int16
    Alu = mybir.AluOpType
    Act = mybir.ActivationFunctionType

    tcore, npad = cfg.t_core, cfg.n_pad
    layouts = meta["layouts"]
    cstar = meta["cstar"]
    use_bias = meta["use_bias"]
    n_chunks = [layouts[g][3] for g in range(2)]
    ncols = [n_chunks[g] * P // 16 for g in range(2)]

    fb = nc.dram_tensor("fb16", [NBANK * cfg.bank_rows, D], bf16,
                        kind="ExternalInput").ap()
    w_in = nc.dram_tensor("w_in", [P, D], f32, kind="ExternalInput").ap()
    a_rep = nc.dram_tensor("a_rep", [P, 1], f32, kind="ExternalInput").ap()
    idx_in = nc.dram_tensor("idx_in", [P, sum(ncols)], i16,
                            kind="ExternalInput").ap()
    off_in = nc.dram_tensor("off_in", [P, sum(n_chunks)], f32,
                            kind="ExternalInput").ap()
    wgt_in = nc.dram_tensor("wgt_in", [P, sum(n_chunks)], f32,
                            kind="ExternalInput").ap()
    iota_in = nc.dram_tensor("iota_in", [P, P], bf16, kind="ExternalInput").ap()
    if use_bias:
        b_rep = nc.dram_tensor("b_rep", [P, D], f32, kind="ExternalInput").ap()
    out = nc.dram_tensor("out", [2, P, tcore * D], bf16,
                         kind="ExternalOutput").ap()

    with ExitStack() as ctx:
        const = ctx.enter_context(tc.tile_pool(name="const", bufs=1))
        gpool = ctx.enter_context(tc.tile_pool(name="gpool", bufs=cfg.gbufs))
        ipool = ctx.enter_context(tc.tile_pool(name="ipool", bufs=cfg.ipool_bufs))
        ohpool = ctx.enter_context(tc.tile_pool(name="ohpool",
                                                bufs=cfg.ohpool_bufs))
        atpool = ctx.enter_context(tc.tile_pool(name="atpool", bufs=3))
        tpool = ctx.enter_context(tc.tile_pool(name="tpool", bufs=4))
        spool = ctx.enter_context(tc.tile_pool(name="spool", bufs=3))
        apool = ctx.enter_context(tc.tile_pool(name="apool", bufs=3,
                                               space="PSUM"))
        hpool = ctx.enter_context(tc.tile_pool(name="hpool", bufs=3,
                                               space="PSUM"))

        # ---- constants ----
        w_sb = const.tile([P, D], bf16)
        nc.gpsimd.dma_start(out=w_sb[:], in_=w_in)  # f32 -> bf16 cast DMA
        iota_sb = const.tile([P, P], bf16)
        nc.sync.dma_start(out=iota_sb[:], in_=iota_in)
        a_sb = const.tile([P, 1], f32)
        nc.sync.dma_start(out=a_sb[:], in_=a_rep)
        if use_bias:
            b_sb = const.tile([P, D], f32)
            nc.sync.dma_start(out=b_sb[:], in_=b_rep)

        # ---- gather + weighted one-hot segment-sum + W + prelu ----
        col_base = [0, ncols[0]]          # idx column offset per graph
        chk_base = [0, n_chunks[0]]       # off/wgt column offset per graph
        cbs_all = []
        for g in range(2):
            calls_by_sg = {}
            for (si, b, c0, nch) in layouts[g][1]:
                calls_by_sg.setdefault(si, []).append((b, c0, nch))
            cbs_all.append(calls_by_sg)
        # interleave the two graphs' supergroups so one graph's gathers fill
        # DMA while the other's PSUM chain drains
        jobs = []
        for si in range(max(len(layouts[0][0]), len(layouts[1][0]))):
            for g in range(2):
                if si < len(layouts[g][0]):
                    jobs.append((g, si))
        oh_ct = 0
        for (g, si) in jobs:
            sg_list, calls, chunk_of, _ = layouts[g]
            cs = cstar[g]
            (k0, kn) = sg_list[si]
            sg_chunks = sum(int(cs[k0 + ki, b]) for ki in range(kn)
                            for b in range(NBANK))
            c0_sg = chunk_of[(k0, 0, 0)]
            gt = gpool.tile([P, sg_chunks, D], bf16, tag="gather")
            it = ipool.tile([P, sg_chunks * 8], i16, tag="gidx")
            nc.sync.dma_start(
                out=it[:],
                in_=idx_in[:, col_base[g] + c0_sg * 8 :
                           col_base[g] + (c0_sg + sg_chunks) * 8])
            ot = ipool.tile([P, sg_chunks], f32, tag="goff")
            nc.sync.dma_start(
                out=ot[:],
                in_=off_in[:, chk_base[g] + c0_sg :
                           chk_base[g] + c0_sg + sg_chunks])
            wt = ipool.tile([P, sg_chunks], f32, tag="gwgt")
            nc.sync.dma_start(
                out=wt[:],
                in_=wgt_in[:, chk_base[g] + c0_sg :
                           chk_base[g] + c0_sg + sg_chunks])
            for (b, c0, nch) in cbs_all[g][si]:
                lo = c0 - c0_sg
                nc.gpsimd.dma_gather(
                    out_ap=gt[:, lo : lo + nch, :],
                    in_ap=fb[b * cfg.bank_rows : (b + 1) * cfg.bank_rows, :],
                    idxs_ap=it[:, lo * 8 : (lo + nch) * 8],
                    num_idxs=nch * P, num_idxs_reg=nch * P,
                    elem_size=D, single_packet=False)
            stg = spool.tile([P, kn, D], bf16, tag="stg")
            nb4 = 4 if cfg.act_batch else 1
            kq = 0
            while kq < kn:
                kb = min(nb4, kn - kq)
                agg4 = apool.tile([P, nb4, D], f32)
                for j in range(kb):
                    k = k0 + kq + j
                    nonzero = [(b, c) for b in range(NBANK)
                               for c in range(int(cs[k, b]))]
                    for ji, (b, c) in enumerate(nonzero):
                        ci = chunk_of[(k, b, c)]
                        lo = ci - c0_sg
                        oh = ohpool.tile([P, P], bf16)
                        eng = nc.vector
                        if cfg.oh_pool_mod:
                            oh_ct += 1
                            if oh_ct % cfg.oh_pool_mod == 0:
                                eng = nc.gpsimd
                        eng.tensor_scalar(
                            out=oh[:], in0=iota_sb[:],
                            scalar1=ot[:, lo : lo + 1],
                            scalar2=wt[:, lo : lo + 1],
                            op0=Alu.is_equal, op1=Alu.mult)
                        nc.tensor.matmul(
                            out=agg4[:, j, :], lhsT=gt[:, lo, :], rhs=oh[:],
                            start=(ji == 0), stop=(ji == len(nonzero) - 1))
                at4 = atpool.tile([P, nb4, D], bf16, tag="at4")
                nc.scalar.activation(out=at4[:, :kb, :], in_=agg4[:, :kb, :],
                                     func=Act.Copy)
                h4 = hpool.tile([P, nb4, D], f32)
                for j in range(kb):
                    nc.tensor.matmul(out=h4[:, j, :], lhsT=at4[:, j, :],
                                     rhs=w_sb[:], start=True, stop=True)
                if cfg.act_prelu and not use_bias:
                    nc.scalar.activation(
                        out=stg[:, kq : kq + kb, :], in_=h4[:, :kb, :],
                        func=Act.Prelu, alpha=a_sb[:, :1])
                elif not use_bias:
                    neg = tpool.tile([P, nb4, D], f32, tag="neg")
                    nc.vector.tensor_scalar(
                        out=neg[:, :kb, :], in0=h4[:, :kb, :], scalar1=0.0,
                        scalar2=a_sb[:, :1], op0=Alu.min, op1=Alu.mult)
                    pos = tpool.tile([P, nb4, D], f32, tag="pos")
                    nc.vector.tensor_scalar(
                        out=pos[:, :kb, :], in0=h4[:, :kb, :], scalar1=0.0,
                        scalar2=None, op0=Alu.max)
                    nc.vector.tensor_tensor(
                        out=stg[:, kq : kq + kb, :], in0=neg[:, :kb, :],
                        in1=pos[:, :kb, :], op=Alu.add)
                else:
                    hb = tpool.tile([P, nb4, D], f32, tag="hb")
                    nc.vector.tensor_tensor(
                        out=hb[:, :kb, :], in0=h4[:, :kb, :],
                        in1=b_sb[:, None, :].to_broadcast([P, kb, D]),
                        op=Alu.add)
                    neg = tpool.tile([P, nb4, D], f32, tag="neg")
                    nc.vector.tensor_scalar(
                        out=neg[:, :kb, :], in0=hb[:, :kb, :], scalar1=0.0,
                        scalar2=a_sb[:, :1], op0=Alu.min, op1=Alu.mult)
                    pos = tpool.tile([P, nb4, D], f32, tag="pos")
                    nc.vector.tensor_scalar(
                        out=pos[:, :kb, :], in0=hb[:, :kb, :], scalar1=0.0,
                        scalar2=None, op0=Alu.max)
                    nc.vector.tensor_tensor(
                        out=stg[:, kq : kq + kb, :], in0=neg[:, :kb, :],
                        in1=pos[:, :kb, :], op=Alu.add)
                kq += kb
            nc.sync.dma_start(
                out=out[g, :, k0 * D : (k0 + kn) * D],
                in_=stg[:].rearrange("p k d -> p (k d)"))
    return out


# --------------------------------------------------------------------------
# Driver
# --------------------------------------------------------------------------
def _build_program(cfg: Config, meta):
    import concourse.bacc as bacc
    import concourse.tile as tile

    nc = bacc.Bacc("TRN2", target_bir_lowering=False, debug=False,
                   enable_asserts=False, num_devices=cfg.n_cores)
    with tile.TileContext(nc) as tc:
        build_kernel(nc, tc, cfg, meta)
    nc.compile()
    return nc


def _unscramble(results, plans, cfg: Config):
    n = cfg.n_nodes
    full = np.zeros((2, n, D), np.float32)
    for g in range(2):
        ct_all = plans[g]["core_tiles"]
        for core in range(cfg.n_cores):
            # [2, P, t_core*D] bf16
            oc = np.asarray(results[core]["out"]).astype(np.float32)
            for k in range(cfg.t_core):
                t = int(ct_all[core, k])
                if t < 0:
                    continue
                r0 = t * P
                r1 = min(r0 + P, n)
                full[g, r0:r1] = oc[g, : r1 - r0, k * D : (k + 1) * D]
    return full


_PROGRAM_CACHE = {}


def run(inputs, cfg: Config, trace=False):
    from concourse.bass_utils import run_bass_kernel_spmd

    in_maps, plans, meta = preprocess(
        inputs["feats"], inputs["W"], inputs["b"], inputs["prelu_a"],
        inputs["src_pos"], inputs["dst_pos"],
        inputs["src_neg"], inputs["dst_neg"], cfg)

    key = (cfg.n_nodes, cfg.n_cores, cfg.sg, cfg.act_prelu, cfg.act_batch,
           cfg.oh_pool_mod, cfg.gbufs, cfg.maxc,
           meta["cstar"][0].tobytes(), meta["cstar"][1].tobytes(),
           meta["use_bias"])
    nc = _PROGRAM_CACHE.get(key)
    if nc is None:
        nc = _build_program(cfg, meta)
        _PROGRAM_CACHE[key] = nc

    kwargs = {}
    if trace:
        kwargs = dict(trace=True, tmpdir=tempfile.mkdtemp(prefix="bgc_trace_"))
    res = run_bass_kernel_spmd(nc, in_maps, core_ids=list(range(cfg.n_cores)),
                               **kwargs)
    full = _unscramble(res.results, plans, cfg)
    return full, res


def kernel(**inputs) -> np.ndarray:
    cfg = Config()
    full, _ = run(inputs, cfg)
    return full


# revision 21
# speedup vs baseline: 1.8542x; 1.0053x over previous
"""Bass/Trainium2 kernel for BiGraphContrastLayer (GNN message passing).

Computes, for two edge lists (pos/neg) over the same node features:
    h_g = PReLU( D_in^-1/2 A_g D_out^-1/2 feats @ W + b )
returning stack([h_pos, h_neg]) of shape [2, N, Dout].

Strategy (8 NeuronCores, SPMD, no collectives). Uses linearity twice:
    nd_d * sum_e ns_s feats_s  =  sum_e (ns_s nd_d) feats_s
    (agg) @ W                  =  W applied once per dst tile after agg

  Single device pass: edges are bucketed by 128-node dst tile and sorted by
  src within (tile, row-bank); dma_gather (int16 idx over 4 row-bank views
  of one bf16 feats table) pulls feats[src] rows for ~128-edge chunks; a
  weighted one-hot matmul (lhsT = gathered chunk, rhs = onehot of dst offsets
  scaled by the per-edge weight ns_src*nd_dst) accumulates the TRANSPOSED
  aggregate aggT[din, dst] for each dst tile in PSUM; aggT -> bf16 SBUF
  (batched Act copy over 4 tiles), one matmul per tile applies W, and a
  batched PReLU produces the output tile.

  Host does index prep (degree bincounts -> per-edge norm weights, sorting,
  bucketing, dealing dst tiles to cores so all 8 cores share one instruction
  stream) and stages the bf16 feats table; all O(E*D) and O(N*D^2) work runs
  on device.
"""

import math
import tempfile
from dataclasses import dataclass

import numpy as np

P = 128   # partitions
D = 128   # feature dim (Din == Dout == 128)
NBANK = 4  # row-range banks (int16 gather idx addresses <=32768 rows)


# --------------------------------------------------------------------------
# Config
# --------------------------------------------------------------------------
@dataclass
class Config:
    n_nodes: int = 100000
    n_cores: int = 8
    sg: int = 10       # dst-tile positions per gather supergroup
    act_prelu: bool = True    # final PReLU on ScalarE (not in sim)
    act_batch: bool = True    # batch aggT copy / prelu over 4 tiles
    oh_pool_mod: int = 0      # every Nth one-hot build goes to GpSimd (0=off)
    maxc: int = 48            # max chunks per dma_gather call
    gbufs: int = 3            # gather buffer count
    ipool_bufs: int = 3
    ohpool_bufs: int = 6

    @property
    def t_global(self) -> int:
        return math.ceil(self.n_nodes / P)

    @property
    def n_pad(self) -> int:
        return self.t_global * P

    @property
    def t_core(self) -> int:
        return math.ceil(self.t_global / self.n_cores)

    @property
    def bank_rows(self) -> int:
        return math.ceil(self.t_global / NBANK) * P


# --------------------------------------------------------------------------
# Host-side preprocessing
# --------------------------------------------------------------------------
def _norm(deg):
    deg = deg.astype(np.float64)
    return np.where(deg > 0, 1.0 / np.sqrt(np.maximum(deg, 1.0)), 0.0).astype(
        np.float32)


def _plan_graph(src, dst, wgt, cfg: Config):
    """Bucket edges by dst tile, split by src row-bank, sort by src within
    (tile, bank), and deal tiles to cores with signature matching.

    Returns dict with:
      core_tiles  [n_cores, t_core]  global tile id per position (-1 null)
      cstar       [t_core, NBANK]    shared chunk counts per position/bank
      tile_edges  list per global tile: (idx16, off, wgt, bank_cnt)
    """
    tg, ncores, tcore = cfg.t_global, cfg.n_cores, cfg.t_core
    order = np.argsort(dst, kind="stable")
    src_s = src[order]
    dst_s = dst[order]
    wgt_s = wgt[order]
    tile_cnt = np.bincount(dst_s // P, minlength=tg)
    starts = np.zeros(tg + 1, np.int64)
    np.cumsum(tile_cnt, out=starts[1:])

    sig = np.zeros((tg, NBANK), np.int64)
    tile_edges = []
    for t in range(tg):
        e0, e1 = int(starts[t]), int(starts[t + 1])
        ts_src = src_s[e0:e1]
        bank = ts_src // cfg.bank_rows
        so = np.argsort(bank * (2 ** 40) + ts_src, kind="stable")
        ts_src = ts_src[so]
        ts_idx = ts_src - bank[so] * cfg.bank_rows
        ts_off = (dst_s[e0:e1][so] % P).astype(np.float32)
        ts_wgt = wgt_s[e0:e1][so]
        bc = np.bincount(bank[so], minlength=NBANK)
        sig[t] = -(-bc // P)  # ceil chunks per bank
        sig[t, 0] = max(sig[t, 0], 1)
        tile_edges.append((ts_idx, ts_off, ts_wgt, bc))

    # Deal: sort tiles by signature so consecutive groups of n_cores tiles
    # have matching/near-matching per-bank chunk counts.
    sigkey = sig @ (np.int64(32) ** np.arange(NBANK - 1, -1, -1))
    keys = np.argsort(sigkey, kind="stable")
    n_slots = ncores * tcore
    dealt = np.full(n_slots, -1, np.int64)
    dealt[: len(keys)] = keys[::-1]  # descending signature order
    core_tiles = np.zeros((ncores, tcore), np.int64)
    cstar = np.zeros((tcore, NBANK), np.int64)
    for k in range(tcore):
        grp = dealt[k * ncores : (k + 1) * ncores]
        for c in range(ncores):
            core_tiles[c, k] = grp[c]
        s = np.zeros(NBANK, np.int64)
        for t in grp:
            if t >= 0:
                s = np.maximum(s, sig[t])
        s[0] = max(s[0], 1)
        cstar[k] = s
    return dict(core_tiles=core_tiles, cstar=cstar, tile_edges=tile_edges)


def _slot_layout(cstar, cfg: Config):
    """Shared (all-cores) slot layout for one graph.

    Slot space = sequence of supergroups; within a supergroup, bank-major:
      for b in banks: for k in sg positions: cstar[k, b] chunks.
    Returns:
      sg_list: list of (k0, kn)
      calls:   list of (sg_idx, bank, chunk0, nchunks)  [nchunks > 0]
      chunk_of: dict (k, b, c) -> global chunk index
      n_chunks total
    """
    tcore, sg = cfg.t_core, cfg.sg
    sg_list = []
    k0 = 0
    while k0 < tcore:
        kn = min(sg, tcore - k0)
        sg_list.append((k0, kn))
        k0 += kn
    calls = []
    chunk_of = {}
    cpos = 0
    for si, (k0, kn) in enumerate(sg_list):
        for b in range(NBANK):
            c0 = cpos
            for ki in range(kn):
                k = k0 + ki
                for c in range(int(cstar[k, b])):
                    chunk_of[(k, b, c)] = cpos
                    cpos += 1
            while c0 < cpos:
                nch = min(cfg.maxc, cpos - c0)
                calls.append((si, b, c0, nch))
                c0 += nch
    return sg_list, calls, chunk_of, cpos


def _fill_core_graph(plan, layout, core, cfg: Config):
    """Build IDX16 (wrapped), OFF, and WGT arrays for one core, one graph."""
    sg_list, calls, chunk_of, n_chunks = layout
    cstar = plan["cstar"]
    idx = np.zeros((n_chunks, P), np.int16)
    off = np.full((n_chunks, P), 512.0, np.float32)
    wgt = np.zeros((n_chunks, P), np.float32)
    for k in range(cfg.t_core):
        t = plan["core_tiles"][core, k]
        if t < 0:
            continue
        ts_idx, ts_off, ts_wgt, bc = plan["tile_edges"][t]
        bstart = np.zeros(NBANK + 1, np.int64)
        np.cumsum(bc, out=bstart[1:])
        for b in range(NBANK):
            nb = int(bc[b])
            cnum = int(cstar[k, b])
            if cnum == 0:
                continue
            nslot = cnum * P
            es = np.zeros(nslot, np.int64)
            eo = np.full(nslot, 512.0, np.float32)
            ew = np.zeros(nslot, np.float32)
            es[:nb] = ts_idx[bstart[b] : bstart[b + 1]]
            eo[:nb] = ts_off[bstart[b] : bstart[b + 1]]
            ew[:nb] = ts_wgt[bstart[b] : bstart[b + 1]]
            for c in range(cnum):
                ci = chunk_of[(k, b, c)]
                idx[ci] = es[c * P : (c + 1) * P]
                off[ci] = eo[c * P : (c + 1) * P]
                wgt[ci] = ew[c * P : (c + 1) * P]
    # wrap: flat slot i (within a call's range) -> [i%16, i//16], replicated
    # to 128 partitions. Build per call, concatenated along columns.
    ncols = n_chunks * P // 16
    idx_w = np.zeros((P, ncols), np.int16)
    for (si, b, c0, nch) in calls:
        flat = idx[c0 : c0 + nch].reshape(-1)  # ni slots
        blk = flat.reshape(-1, 16).T  # [16, ni/16]
        idx_w[:, c0 * 8 : (c0 + nch) * 8] = np.tile(blk, (8, 1))
    return idx_w, off.T.copy(), wgt.T.copy()  # -> [P, n_chunks] f32


def preprocess(feats, W, b, prelu_a, src_pos, dst_pos, src_neg, dst_neg,
               cfg: Config):
    n = cfg.n_nodes
    feats = np.asarray(feats, np.float32)
    W = np.asarray(W, np.float32)
    b = np.asarray(b, np.float32)
    prelu_a = np.asarray(prelu_a, np.float32)
    bf16 = np.dtype("bfloat16")

    feats_pad = np.zeros((NBANK * cfg.bank_rows, D), np.float32)
    feats_pad[:n] = feats
    fb16 = feats_pad.astype(bf16)

    plans, layouts = [], []
    for src, dst in ((src_pos, dst_pos), (src_neg, dst_neg)):
        src = np.asarray(src, np.int64)
        dst = np.asarray(dst, np.int64)
        ns = _norm(np.bincount(src, minlength=n))
        nd = _norm(np.bincount(dst, minlength=n))
        # merge duplicate (src, dst) edges: by linearity their contributions
        # sum, so one gathered row with a summed weight is exact.
        pair = src * np.int64(1 << 32) + dst
        upair, ucnt = np.unique(pair, return_counts=True)
        usrc = (upair >> 32).astype(np.int64)
        udst = (upair & ((1 << 32) - 1)).astype(np.int64)
        wgt = (ucnt * ns[usrc] * nd[udst]).astype(np.float32)
        plan = _plan_graph(usrc, udst, wgt, cfg)
        plans.append(plan)
        layouts.append(_slot_layout(plan["cstar"], cfg))

    iota = np.tile(np.arange(P, dtype=np.float32), (P, 1)).astype(bf16)
    a_rep = np.full((P, 1), float(prelu_a.reshape(-1)[0]), np.float32)
    b_rep = np.tile(b.reshape(1, D), (P, 1)).astype(np.float32)
    use_bias = bool(np.any(b != 0.0))

    in_maps = []
    for core in range(cfg.n_cores):
        iw_p, off_p, wgt_p = _fill_core_graph(plans[0], layouts[0], core, cfg)
        iw_n, off_n, wgt_n = _fill_core_graph(plans[1], layouts[1], core, cfg)
        m = {
            "fb16": fb16,
            "w_in": W,
            "a_rep": a_rep,
            "idx_in": np.concatenate([iw_p, iw_n], axis=1),
            "off_in": np.concatenate([off_p, off_n], axis=1),
            "wgt_in": np.concatenate([wgt_p, wgt_n], axis=1),
            "iota_in": iota,
        }
        if use_bias:
            m["b_rep"] = b_rep
        in_maps.append(m)
    meta = {
        "layouts": layouts,
        "cstar": [plans[0]["cstar"], plans[1]["cstar"]],
        "use_bias": use_bias,
    }
    return in_maps, plans, meta


# --------------------------------------------------------------------------
# Device kernel builder
# --------------------------------------------------------------------------
def build_kernel(nc, tc, cfg: Config, meta):
    from contextlib import ExitStack

    import concourse.mybir as mybir

    f32 = mybir.dt.float32
    bf16 = mybir.dt.bfloat16
    i16 = mybir.dt.int16
    Alu = mybir.AluOpType
    Act = mybir.ActivationFunctionType

    tcore, npad = cfg.t_core, cfg.n_pad
    layouts = meta["layouts"]
    cstar = meta["cstar"]
    use_bias = meta["use_bias"]
    n_chunks = [layouts[g][3] for g in range(2)]
    ncols = [n_chunks[g] * P // 16 for g in range(2)]

    fb = nc.dram_tensor("fb16", [NBANK * cfg.bank_rows, D], bf16,
                        kind="ExternalInput").ap()
    w_in = nc.dram_tensor("w_in", [P, D], f32, kind="ExternalInput").ap()
    a_rep = nc.dram_tensor("a_rep", [P, 1], f32, kind="ExternalInput").ap()
    idx_in = nc.dram_tensor("idx_in", [P, sum(ncols)], i16,
                            kind="ExternalInput").ap()
    off_in = nc.dram_tensor("off_in", [P, sum(n_chunks)], f32,
                            kind="ExternalInput").ap()
    wgt_in = nc.dram_tensor("wgt_in", [P, sum(n_chunks)], f32,
                            kind="ExternalInput").ap()
    iota_in = nc.dram_tensor("iota_in", [P, P], bf16, kind="ExternalInput").ap()
    if use_bias:
        b_rep = nc.dram_tensor("b_rep", [P, D], f32, kind="ExternalInput").ap()
    out = nc.dram_tensor("out", [2, P, tcore * D], bf16,
                         kind="ExternalOutput").ap()

    with ExitStack() as ctx:
        const = ctx.enter_context(tc.tile_pool(name="const", bufs=1))
        gpool = ctx.enter_context(tc.tile_pool(name="gpool", bufs=cfg.gbufs))
        ipool = ctx.enter_context(tc.tile_pool(name="ipool", bufs=cfg.ipool_bufs))
        ohpool = ctx.enter_context(tc.tile_pool(name="ohpool",
                                                bufs=cfg.ohpool_bufs))
        atpool = ctx.enter_context(tc.tile_pool(name="atpool", bufs=3))
        tpool = ctx.enter_context(tc.tile_pool(name="tpool", bufs=4))
        spool = ctx.enter_context(tc.tile_pool(name="spool", bufs=3))
        apool = ctx.enter_context(tc.tile_pool(name="apool", bufs=3,
                                               space="PSUM"))
        hpool = ctx.enter_context(tc.tile_pool(name="hpool", bufs=3,
                                               space="PSUM"))

        # ---- constants ----
        w_sb = const.tile([P, D], bf16)
        nc.gpsimd.dma_start(out=w_sb[:], in_=w_in)  # f32 -> bf16 cast DMA
        iota_sb = const.tile([P, P], bf16)
        nc.sync.dma_start(out=iota_sb[:], in_=iota_in)
        a_sb = const.tile([P, 1], f32)
        nc.sync.dma_start(out=a_sb[:], in_=a_rep)
        if use_bias:
            b_sb = const.tile([P, D], f32)
            nc.sync.dma_start(out=b_sb[:], in_=b_rep)

        # ---- gather + weighted one-hot segment-sum + W + prelu ----
        col_base = [0, ncols[0]]          # idx column offset per graph
        chk_base = [0, n_chunks[0]]       # off/wgt column offset per graph
        cbs_all = []
        for g in range(2):
            calls_by_sg = {}
            for (si, b, c0, nch) in layouts[g][1]:
                calls_by_sg.setdefault(si, []).append((b, c0, nch))
            cbs_all.append(calls_by_sg)
        # interleave the two graphs' supergroups so one graph's gathers fill
        # DMA while the other's PSUM chain drains
        jobs = []
        for si in range(max(len(layouts[0][0]), len(layouts[1][0]))):
            for g in range(2):
                if si < len(layouts[g][0]):
                    jobs.append((g, si))
        oh_ct = 0
        for (g, si) in jobs:
            sg_list, calls, chunk_of, _ = layouts[g]
            cs = cstar[g]
            (k0, kn) = sg_list[si]
            sg_chunks = sum(int(cs[k0 + ki, b]) for ki in range(kn)
                            for b in range(NBANK))
            c0_sg = chunk_of[(k0, 0, 0)]
            gt = gpool.tile([P, sg_chunks, D], bf16, tag="gather")
            it = ipool.tile([P, sg_chunks * 8], i16, tag="gidx")
            nc.sync.dma_start(
                out=it[:],
                in_=idx_in[:, col_base[g] + c0_sg * 8 :
                           col_base[g] + (c0_sg + sg_chunks) * 8])
            ot = ipool.tile([P, sg_chunks], f32, tag="goff")
            nc.sync.dma_start(
                out=ot[:],
                in_=off_in[:, chk_base[g] + c0_sg :
                           chk_base[g] + c0_sg + sg_chunks])
            wt = ipool.tile([P, sg_chunks], f32, tag="gwgt")
            nc.sync.dma_start(
                out=wt[:],
                in_=wgt_in[:, chk_base[g] + c0_sg :
                           chk_base[g] + c0_sg + sg_chunks])
            for (b, c0, nch) in cbs_all[g][si]:
                lo = c0 - c0_sg
                nc.gpsimd.dma_gather(
                    out_ap=gt[:, lo : lo + nch, :],
                    in_ap=fb[b * cfg.bank_rows : (b + 1) * cfg.bank_rows, :],
                    idxs_ap=it[:, lo * 8 : (lo + nch) * 8],
                    num_idxs=nch * P, num_idxs_reg=nch * P,
                    elem_size=D, single_packet=False)
            stg = spool.tile([P, kn, D], bf16, tag="stg")
            nb4 = 4 if cfg.act_batch else 1
            kq = 0
            while kq < kn:
                kb = min(nb4, kn - kq)
                agg4 = apool.tile([P, nb4, D], f32)
                for j in range(kb):
                    k = k0 + kq + j
                    nonzero = [(b, c) for b in range(NBANK)
                               for c in range(int(cs[k, b]))]
                    for ji, (b, c) in enumerate(nonzero):
                        ci = chunk_of[(k, b, c)]
                        lo = ci - c0_sg
                        oh = ohpool.tile([P, P], bf16)
                        eng = nc.vector
                        if cfg.oh_pool_mod:
                            oh_ct += 1
                            if oh_ct % cfg.oh_pool_mod == 0:
                                eng = nc.gpsimd
                        eng.tensor_scalar(
                            out=oh[:], in0=iota_sb[:],
                            scalar1=ot[:, lo : lo + 1],
                            scalar2=wt[:, lo : lo + 1],
                            op0=Alu.is_equal, op1=Alu.mult)
                        nc.tensor.matmul(
                            out=agg4[:, j, :], lhsT=gt[:, lo, :], rhs=oh[:],
                            start=(ji == 0), stop=(ji == len(nonzero) - 1))
                at4 = atpool.tile([P, nb4, D], bf16, tag="at4")
                nc.scalar.activation(out=at4[:, :kb, :], in_=agg4[:, :kb, :],
                                     func=Act.Copy)
                h4 = hpool.tile([P, nb4, D], f32)
                for j in range(kb):
                    nc.tensor.matmul(out=h4[:, j, :], lhsT=at4[:, j, :],
                                     rhs=w_sb[:], start=True, stop=True)
                if cfg.act_prelu and not use_bias:
                    nc.scalar.activation(
                        out=stg[:, kq : kq + kb, :], in_=h4[:, :kb, :],
                        func=Act.Prelu, alpha=a_sb[:, :1])
                elif not use_bias:
                    neg = tpool.tile([P, nb4, D], f32, tag="neg")
                    nc.vector.tensor_scalar(
                        out=neg[:, :kb, :], in0=h4[:, :kb, :], scalar1=0.0,
                        scalar2=a_sb[:, :1], op0=Alu.min, op1=Alu.mult)
                    pos = tpool.tile([P, nb4, D], f32, tag="pos")
                    nc.vector.tensor_scalar(
                        out=pos[:, :kb, :], in0=h4[:, :kb, :], scalar1=0.0,
                        scalar2=None, op0=Alu.max)
                    nc.vector.tensor_tensor(
                        out=stg[:, kq : kq + kb, :], in0=neg[:, :kb, :],
                        in1=pos[:, :kb, :], op=Alu.add)
                else:
                    hb = tpool.tile([P, nb4, D], f32, tag="hb")
                    nc.vector.tensor_tensor(
                        out=hb[:, :kb, :], in0=h4[:, :kb, :],
                        in1=b_sb[:, None, :].to_broadcast([P, kb, D]),
                        op=Alu.add)
                    neg = tpool.tile([P, nb4, D], f32, tag="neg")
                    nc.vector.tensor_scalar(
                        out=neg[:, :kb, :], in0=hb[:, :kb, :], scalar1=0.0,
                        scalar2=a_sb[:, :1], op0=Alu.min, op1=Alu.mult)
                    pos = tpool.tile([P, nb4, D], f32, tag="pos")
                    nc.vector.tensor_scalar(
                        out=pos[:, :kb, :], in0=hb[:, :kb, :], scalar1=0.0,
                        scalar2=None, op0=Alu.max)
                    nc.vector.tensor_tensor(
                        out=stg[:, kq : kq + kb, :], in0=neg[:, :kb, :],
                        in1=pos[:, :kb, :], op=Alu.add)
                kq += kb
            nc.sync.dma_start(
                out=out[g, :, k0 * D : (k0 + kn) * D],
                in_=stg[:].rearrange("p k d -> p (k d)"))
    return out


# --------------------------------------------------------------------------
# Driver
# --------------------------------------------------------------------------
def _build_program(cfg: Config, meta):
    import concourse.bacc as bacc
    import concourse.tile as tile

    nc = bacc.Bacc("TRN2", target_bir_lowering=False, debug=False,
                   enable_asserts=False, num_devices=cfg.n_cores)
    with tile.TileContext(nc) as tc:
        build_kernel(nc, tc, cfg, meta)
    nc.compile()
    return nc


def _unscramble(results, plans, cfg: Config):
    n = cfg.n_nodes
    full = np.zeros((2, n, D), np.float32)
    for g in range(2):
        ct_all = plans[g]["core_tiles"]
        for core in range(cfg.n_cores):
            # [2, P, t_core*D] bf16
            oc = np.asarray(results[core]["out"]).astype(np.float32)
            for k in range(cfg.t_core):
                t = int(ct_all[core, k])
                if t < 0:
                    continue
                r0 = t * P
                r1 = min(r0 + P, n)
                full[g, r0:r1] = oc[g, : r1 - r0, k * D : (k + 1) * D]
    return full


_PROGRAM_CACHE = {}


def run(inputs, cfg: Config, trace=False):
    from concourse.bass_utils import run_bass_kernel_spmd

    in_maps, plans, meta = preprocess(
        inputs["feats"], inputs["W"], inputs["b"], inputs["prelu_a"],
        inputs["src_pos"], inputs["dst_pos"],
        inputs["src_neg"], inputs["dst_neg"], cfg)

    key = (cfg.n_nodes, cfg.n_cores, cfg.sg, cfg.act_prelu, cfg.act_batch,
           cfg.oh_pool_mod, cfg.gbufs, cfg.maxc,
           meta["cstar"][0].tobytes(), meta["cstar"][1].tobytes(),
           meta["use_bias"])
    nc = _PROGRAM_CACHE.get(key)
    if nc is None:
        nc = _build_program(cfg, meta)
        _PROGRAM_CACHE[key] = nc

    kwargs = {}
    if trace:
        kwargs = dict(trace=True, tmpdir=tempfile.mkdtemp(prefix="bgc_trace_"))
    res = run_bass_kernel_spmd(nc, in_maps, core_ids=list(range(cfg.n_cores)),
                               **kwargs)
    full = _unscramble(res.results, plans, cfg)
    return full, res


def kernel(**inputs) -> np.ndarray:
    cfg = Config()
    full, _ = run(inputs, cfg)
    return full


# revision 27
# speedup vs baseline: 1.8694x; 1.0082x over previous
"""Bass/Trainium2 kernel for BiGraphContrastLayer (GNN message passing).

Computes, for two edge lists (pos/neg) over the same node features:
    h_g = PReLU( D_in^-1/2 A_g D_out^-1/2 feats @ W + b )
returning stack([h_pos, h_neg]) of shape [2, N, Dout].

Strategy (8 NeuronCores, SPMD, no collectives). Uses linearity twice:
    nd_d * sum_e ns_s feats_s  =  sum_e (ns_s nd_d) feats_s
    (agg) @ W                  =  W applied once per dst tile after agg

  Single device pass: edges are bucketed by 128-node dst tile and sorted by
  src within (tile, row-bank); dma_gather (int16 idx over 4 row-bank views
  of one bf16 feats table) pulls feats[src] rows for ~128-edge chunks; a
  weighted one-hot matmul (lhsT = gathered chunk, rhs = onehot of dst offsets
  scaled by the per-edge weight ns_src*nd_dst) accumulates the TRANSPOSED
  aggregate aggT[din, dst] for each dst tile in PSUM; aggT -> bf16 SBUF
  (batched Act copy over 4 tiles), one matmul per tile applies W, and a
  batched PReLU produces the output tile.

  Host does index prep (degree bincounts -> per-edge norm weights, sorting,
  bucketing, dealing dst tiles to cores so all 8 cores share one instruction
  stream) and stages the bf16 feats table; all O(E*D) and O(N*D^2) work runs
  on device.
"""

import math
import tempfile
from dataclasses import dataclass

import numpy as np

P = 128   # partitions
D = 128   # feature dim (Din == Dout == 128)
NBANK = 4  # row-range banks (int16 gather idx addresses <=32768 rows)


# --------------------------------------------------------------------------
# Config
# --------------------------------------------------------------------------
@dataclass
class Config:
    n_nodes: int = 100000
    n_cores: int = 8
    sg: int = 14       # dst-tile positions per gather supergroup
    sg_first: int = 2  # size of the first supergroup (fast pipeline fill)
    sg_tail: int = 2   # max size of the last supergroup (short drain)
    act_prelu: bool = True    # final PReLU on ScalarE (not in sim)
    act_batch: bool = True    # batch aggT copy / prelu over 4 tiles
    oh_pool_mod: int = 0      # every Nth one-hot build goes to GpSimd (0=off)
    maxc: int = 48            # max chunks per dma_gather call
    gbufs: int = 3            # gather buffer count
    ipool_bufs: int = 3
    ohpool_bufs: int = 6

    @property
    def t_global(self) -> int:
        return math.ceil(self.n_nodes / P)

    @property
    def n_pad(self) -> int:
        return self.t_global * P

    @property
    def t_core(self) -> int:
        return math.ceil(self.t_global / self.n_cores)

    @property
    def bank_rows(self) -> int:
        return math.ceil(self.t_global / NBANK) * P


# --------------------------------------------------------------------------
# Host-side preprocessing
# --------------------------------------------------------------------------
def _norm(deg):
    deg = deg.astype(np.float64)
    return np.where(deg > 0, 1.0 / np.sqrt(np.maximum(deg, 1.0)), 0.0).astype(
        np.float32)


def _plan_graph(src, dst, wgt, cfg: Config):
    """Bucket edges by dst tile, split by src row-bank, sort by src within
    (tile, bank), and deal tiles to cores with signature matching.

    Returns dict with:
      core_tiles  [n_cores, t_core]  global tile id per position (-1 null)
      cstar       [t_core, NBANK]    shared chunk counts per position/bank
      tile_edges  list per global tile: (idx16, off, wgt, bank_cnt)
    """
    tg, ncores, tcore = cfg.t_global, cfg.n_cores, cfg.t_core
    order = np.argsort(dst, kind="stable")
    src_s = src[order]
    dst_s = dst[order]
    wgt_s = wgt[order]
    tile_cnt = np.bincount(dst_s // P, minlength=tg)
    starts = np.zeros(tg + 1, np.int64)
    np.cumsum(tile_cnt, out=starts[1:])

    sig = np.zeros((tg, NBANK), np.int64)
    tile_edges = []
    for t in range(tg):
        e0, e1 = int(starts[t]), int(starts[t + 1])
        ts_src = src_s[e0:e1]
        bank = ts_src // cfg.bank_rows
        so = np.argsort(bank * (2 ** 40) + ts_src, kind="stable")
        ts_src = ts_src[so]
        ts_idx = ts_src - bank[so] * cfg.bank_rows
        ts_off = (dst_s[e0:e1][so] % P).astype(np.float32)
        ts_wgt = wgt_s[e0:e1][so]
        bc = np.bincount(bank[so], minlength=NBANK)
        sig[t] = -(-bc // P)  # ceil chunks per bank
        sig[t, 0] = max(sig[t, 0], 1)
        tile_edges.append((ts_idx, ts_off, ts_wgt, bc))

    # Deal: sort tiles by signature so consecutive groups of n_cores tiles
    # have matching/near-matching per-bank chunk counts.
    sigkey = sig @ (np.int64(32) ** np.arange(NBANK - 1, -1, -1))
    keys = np.argsort(sigkey, kind="stable")
    n_slots = ncores * tcore
    dealt = np.full(n_slots, -1, np.int64)
    dealt[: len(keys)] = keys[::-1]  # descending signature order
    core_tiles = np.zeros((ncores, tcore), np.int64)
    cstar = np.zeros((tcore, NBANK), np.int64)
    for k in range(tcore):
        grp = dealt[k * ncores : (k + 1) * ncores]
        for c in range(ncores):
            core_tiles[c, k] = grp[c]
        s = np.zeros(NBANK, np.int64)
        for t in grp:
            if t >= 0:
                s = np.maximum(s, sig[t])
        s[0] = max(s[0], 1)
        cstar[k] = s
    return dict(core_tiles=core_tiles, cstar=cstar, tile_edges=tile_edges)


def _slot_layout(cstar, cfg: Config):
    """Shared (all-cores) slot layout for one graph.

    Slot space = sequence of supergroups; within a supergroup, bank-major:
      for b in banks: for k in sg positions: cstar[k, b] chunks.
    Returns:
      sg_list: list of (k0, kn)
      calls:   list of (sg_idx, bank, chunk0, nchunks)  [nchunks > 0]
      chunk_of: dict (k, b, c) -> global chunk index
      n_chunks total
    """
    tcore, sg = cfg.t_core, cfg.sg
    sizes = []
    rem = tcore
    if cfg.sg_first and rem > cfg.sg_first:
        sizes.append(cfg.sg_first)
        rem -= cfg.sg_first
    while rem > 0:
        kn = min(sg, rem)
        sizes.append(kn)
        rem -= kn
    # split an oversized final group so the drain after the last gather is
    # short
    if cfg.sg_tail and len(sizes) > 1 and sizes[-1] > cfg.sg_tail:
        last = sizes.pop()
        h = last - cfg.sg_tail
        sizes.extend([h, cfg.sg_tail])
    sg_list = []
    k0 = 0
    for kn in sizes:
        sg_list.append((k0, kn))
        k0 += kn
    calls = []
    chunk_of = {}
    cpos = 0
    for si, (k0, kn) in enumerate(sg_list):
        for b in range(NBANK):
            c0 = cpos
            for ki in range(kn):
                k = k0 + ki
                for c in range(int(cstar[k, b])):
                    chunk_of[(k, b, c)] = cpos
                    cpos += 1
            while c0 < cpos:
                nch = min(cfg.maxc, cpos - c0)
                calls.append((si, b, c0, nch))
                c0 += nch
    return sg_list, calls, chunk_of, cpos


def _fill_core_graph(plan, layout, core, cfg: Config):
    """Build IDX16 (wrapped), OFF, and WGT arrays for one core, one graph."""
    sg_list, calls, chunk_of, n_chunks = layout
    cstar = plan["cstar"]
    idx = np.zeros((n_chunks, P), np.int16)
    off = np.full((n_chunks, P), 512.0, np.float32)
    wgt = np.zeros((n_chunks, P), np.float32)
    for k in range(cfg.t_core):
        t = plan["core_tiles"][core, k]
        if t < 0:
            continue
        ts_idx, ts_off, ts_wgt, bc = plan["tile_edges"][t]
        bstart = np.zeros(NBANK + 1, np.int64)
        np.cumsum(bc, out=bstart[1:])
        for b in range(NBANK):
            nb = int(bc[b])
            cnum = int(cstar[k, b])
            if cnum == 0:
                continue
            nslot = cnum * P
            es = np.zeros(nslot, np.int64)
            eo = np.full(nslot, 512.0, np.float32)
            ew = np.zeros(nslot, np.float32)
            es[:nb] = ts_idx[bstart[b] : bstart[b + 1]]
            eo[:nb] = ts_off[bstart[b] : bstart[b + 1]]
            ew[:nb] = ts_wgt[bstart[b] : bstart[b + 1]]
            for c in range(cnum):
                ci = chunk_of[(k, b, c)]
                idx[ci] = es[c * P : (c + 1) * P]
                off[ci] = eo[c * P : (c + 1) * P]
                wgt[ci] = ew[c * P : (c + 1) * P]
    # wrap: flat slot i (within a call's range) -> [i%16, i//16], replicated
    # to 128 partitions. Build per call, concatenated along columns.
    ncols = n_chunks * P // 16
    idx_w = np.zeros((P, ncols), np.int16)
    for (si, b, c0, nch) in calls:
        flat = idx[c0 : c0 + nch].reshape(-1)  # ni slots
        blk = flat.reshape(-1, 16).T  # [16, ni/16]
        idx_w[:, c0 * 8 : (c0 + nch) * 8] = np.tile(blk, (8, 1))
    return idx_w, off.T.copy(), wgt.T.copy()  # -> [P, n_chunks] f32


def preprocess(feats, W, b, prelu_a, src_pos, dst_pos, src_neg, dst_neg,
               cfg: Config):
    n = cfg.n_nodes
    feats = np.asarray(feats, np.float32)
    W = np.asarray(W, np.float32)
    b = np.asarray(b, np.float32)
    prelu_a = np.asarray(prelu_a, np.float32)
    bf16 = np.dtype("bfloat16")

    feats_pad = np.zeros((NBANK * cfg.bank_rows, D), np.float32)
    feats_pad[:n] = feats
    fb16 = feats_pad.astype(bf16)

    plans, layouts = [], []
    for src, dst in ((src_pos, dst_pos), (src_neg, dst_neg)):
        src = np.asarray(src, np.int64)
        dst = np.asarray(dst, np.int64)
        ns = _norm(np.bincount(src, minlength=n))
        nd = _norm(np.bincount(dst, minlength=n))
        # merge duplicate (src, dst) edges: by linearity their contributions
        # sum, so one gathered row with a summed weight is exact.
        pair = src * np.int64(1 << 32) + dst
        upair, ucnt = np.unique(pair, return_counts=True)
        usrc = (upair >> 32).astype(np.int64)
        udst = (upair & ((1 << 32) - 1)).astype(np.int64)
        wgt = (ucnt * ns[usrc] * nd[udst]).astype(np.float32)
        plan = _plan_graph(usrc, udst, wgt, cfg)
        plans.append(plan)
        layouts.append(_slot_layout(plan["cstar"], cfg))

    iota = np.tile(np.arange(P, dtype=np.float32), (P, 1)).astype(bf16)
    a_rep = np.full((P, 1), float(prelu_a.reshape(-1)[0]), np.float32)
    b_rep = np.tile(b.reshape(1, D), (P, 1)).astype(np.float32)
    use_bias = bool(np.any(b != 0.0))

    in_maps = []
    for core in range(cfg.n_cores):
        iw_p, off_p, wgt_p = _fill_core_graph(plans[0], layouts[0], core, cfg)
        iw_n, off_n, wgt_n = _fill_core_graph(plans[1], layouts[1], core, cfg)
        m = {
            "fb16": fb16,
            "w_in": W,
            "a_rep": a_rep,
            "idx_in": np.concatenate([iw_p, iw_n], axis=1),
            "off_in": np.concatenate([off_p, off_n], axis=1),
            "wgt_in": np.concatenate([wgt_p, wgt_n], axis=1),
            "iota_in": iota,
        }
        if use_bias:
            m["b_rep"] = b_rep
        in_maps.append(m)
    meta = {
        "layouts": layouts,
        "cstar": [plans[0]["cstar"], plans[1]["cstar"]],
        "use_bias": use_bias,
    }
    return in_maps, plans, meta


# --------------------------------------------------------------------------
# Device kernel builder
# --------------------------------------------------------------------------
def build_kernel(nc, tc, cfg: Config, meta):
    from contextlib import ExitStack

    import concourse.mybir as mybir

    f32 = mybir.dt.float32
    bf16 = mybir.dt.bfloat16
    i16 = mybir.dt.int16
    Alu = mybir.AluOpType
    Act = mybir.ActivationFunctionType

    tcore, npad = cfg.t_core, cfg.n_pad
    layouts = meta["layouts"]
    cstar = meta["cstar"]
    use_bias = meta["use_bias"]
    n_chunks = [layouts[g][3] for g in range(2)]
    ncols = [n_chunks[g] * P // 16 for g in range(2)]

    fb = nc.dram_tensor("fb16", [NBANK * cfg.bank_rows, D], bf16,
                        kind="ExternalInput").ap()
    w_in = nc.dram_tensor("w_in", [P, D], f32, kind="ExternalInput").ap()
    a_rep = nc.dram_tensor("a_rep", [P, 1], f32, kind="ExternalInput").ap()
    idx_in = nc.dram_tensor("idx_in", [P, sum(ncols)], i16,
                            kind="ExternalInput").ap()
    off_in = nc.dram_tensor("off_in", [P, sum(n_chunks)], f32,
                            kind="ExternalInput").ap()
    wgt_in = nc.dram_tensor("wgt_in", [P, sum(n_chunks)], f32,
                            kind="ExternalInput").ap()
    iota_in = nc.dram_tensor("iota_in", [P, P], bf16, kind="ExternalInput").ap()
    if use_bias:
        b_rep = nc.dram_tensor("b_rep", [P, D], f32, kind="ExternalInput").ap()
    out = nc.dram_tensor("out", [2, P, tcore * D], bf16,
                         kind="ExternalOutput").ap()

    with ExitStack() as ctx:
        const = ctx.enter_context(tc.tile_pool(name="const", bufs=1))
        gpool = ctx.enter_context(tc.tile_pool(name="gpool", bufs=cfg.gbufs))
        ipool = ctx.enter_context(tc.tile_pool(name="ipool", bufs=cfg.ipool_bufs))
        ohpool = ctx.enter_context(tc.tile_pool(name="ohpool",
                                                bufs=cfg.ohpool_bufs))
        atpool = ctx.enter_context(tc.tile_pool(name="atpool", bufs=3))
        tpool = ctx.enter_context(tc.tile_pool(name="tpool", bufs=4))
        spool = ctx.enter_context(tc.tile_pool(name="spool", bufs=3))
        apool = ctx.enter_context(tc.tile_pool(name="apool", bufs=3,
                                               space="PSUM"))
        hpool = ctx.enter_context(tc.tile_pool(name="hpool", bufs=3,
                                               space="PSUM"))

        # ---- constants ----
        w_sb = const.tile([P, D], bf16)
        nc.gpsimd.dma_start(out=w_sb[:], in_=w_in)  # f32 -> bf16 cast DMA
        iota_sb = const.tile([P, P], bf16)
        nc.sync.dma_start(out=iota_sb[:], in_=iota_in)
        a_sb = const.tile([P, 1], f32)
        nc.sync.dma_start(out=a_sb[:], in_=a_rep)
        if use_bias:
            b_sb = const.tile([P, D], f32)
            nc.sync.dma_start(out=b_sb[:], in_=b_rep)

        # ---- gather + weighted one-hot segment-sum + W + prelu ----
        col_base = [0, ncols[0]]          # idx column offset per graph
        chk_base = [0, n_chunks[0]]       # off/wgt column offset per graph
        cbs_all = []
        for g in range(2):
            calls_by_sg = {}
            for (si, b, c0, nch) in layouts[g][1]:
                calls_by_sg.setdefault(si, []).append((b, c0, nch))
            cbs_all.append(calls_by_sg)
        # interleave the two graphs' supergroups so one graph's gathers fill
        # DMA while the other's PSUM chain drains
        jobs = []
        for si in range(max(len(layouts[0][0]), len(layouts[1][0]))):
            for g in range(2):
                if si < len(layouts[g][0]):
                    jobs.append((g, si))
        oh_ct = 0
        for (g, si) in jobs:
            sg_list, calls, chunk_of, _ = layouts[g]
            cs = cstar[g]
            (k0, kn) = sg_list[si]
            sg_chunks = sum(int(cs[k0 + ki, b]) for ki in range(kn)
                            for b in range(NBANK))
            c0_sg = chunk_of[(k0, 0, 0)]
            gt = gpool.tile([P, sg_chunks, D], bf16, tag="gather")
            it = ipool.tile([P, sg_chunks * 8], i16, tag="gidx")
            nc.sync.dma_start(
                out=it[:],
                in_=idx_in[:, col_base[g] + c0_sg * 8 :
                           col_base[g] + (c0_sg + sg_chunks) * 8])
            ot = ipool.tile([P, sg_chunks], f32, tag="goff")
            nc.sync.dma_start(
                out=ot[:],
                in_=off_in[:, chk_base[g] + c0_sg :
                           chk_base[g] + c0_sg + sg_chunks])
            wt = ipool.tile([P, sg_chunks], f32, tag="gwgt")
            nc.sync.dma_start(
                out=wt[:],
                in_=wgt_in[:, chk_base[g] + c0_sg :
                           chk_base[g] + c0_sg + sg_chunks])
            for (b, c0, nch) in cbs_all[g][si]:
                lo = c0 - c0_sg
                nc.gpsimd.dma_gather(
                    out_ap=gt[:, lo : lo + nch, :],
                    in_ap=fb[b * cfg.bank_rows : (b + 1) * cfg.bank_rows, :],
                    idxs_ap=it[:, lo * 8 : (lo + nch) * 8],
                    num_idxs=nch * P, num_idxs_reg=nch * P,
                    elem_size=D, single_packet=False)
            stg = spool.tile([P, kn, D], bf16, tag="stg")
            nb4 = 4 if cfg.act_batch else 1
            kq = 0
            while kq < kn:
                kb = min(nb4, kn - kq)
                agg4 = apool.tile([P, nb4, D], f32)
                for j in range(kb):
                    k = k0 + kq + j
                    nonzero = [(b, c) for b in range(NBANK)
                               for c in range(int(cs[k, b]))]
                    for ji, (b, c) in enumerate(nonzero):
                        ci = chunk_of[(k, b, c)]
                        lo = ci - c0_sg
                        oh = ohpool.tile([P, P], bf16)
                        eng = nc.vector
                        if cfg.oh_pool_mod:
                            oh_ct += 1
                            if oh_ct % cfg.oh_pool_mod == 0:
                                eng = nc.gpsimd
                        eng.tensor_scalar(
                            out=oh[:], in0=iota_sb[:],
                            scalar1=ot[:, lo : lo + 1],
                            scalar2=wt[:, lo : lo + 1],
                            op0=Alu.is_equal, op1=Alu.mult)
                        nc.tensor.matmul(
                            out=agg4[:, j, :], lhsT=gt[:, lo, :], rhs=oh[:],
                            start=(ji == 0), stop=(ji == len(nonzero) - 1))
                at4 = atpool.tile([P, nb4, D], bf16, tag="at4")
                nc.scalar.activation(out=at4[:, :kb, :], in_=agg4[:, :kb, :],
                                     func=Act.Copy)
                h4 = hpool.tile([P, nb4, D], f32)
                for j in range(kb):
                    nc.tensor.matmul(out=h4[:, j, :], lhsT=at4[:, j, :],
                                     rhs=w_sb[:], start=True, stop=True)
                if cfg.act_prelu and not use_bias:
                    nc.scalar.activation(
                        out=stg[:, kq : kq + kb, :], in_=h4[:, :kb, :],
                        func=Act.Prelu, alpha=a_sb[:, :1])
                elif not use_bias:
                    neg = tpool.tile([P, nb4, D], f32, tag="neg")
                    nc.vector.tensor_scalar(
                        out=neg[:, :kb, :], in0=h4[:, :kb, :], scalar1=0.0,
                        scalar2=a_sb[:, :1], op0=Alu.min, op1=Alu.mult)
                    pos = tpool.tile([P, nb4, D], f32, tag="pos")
                    nc.vector.tensor_scalar(
                        out=pos[:, :kb, :], in0=h4[:, :kb, :], scalar1=0.0,
                        scalar2=None, op0=Alu.max)
                    nc.vector.tensor_tensor(
                        out=stg[:, kq : kq + kb, :], in0=neg[:, :kb, :],
                        in1=pos[:, :kb, :], op=Alu.add)
                else:
                    hb = tpool.tile([P, nb4, D], f32, tag="hb")
                    nc.vector.tensor_tensor(
                        out=hb[:, :kb, :], in0=h4[:, :kb, :],
                        in1=b_sb[:, None, :].to_broadcast([P, kb, D]),
                        op=Alu.add)
                    neg = tpool.tile([P, nb4, D], f32, tag="neg")
                    nc.vector.tensor_scalar(
                        out=neg[:, :kb, :], in0=hb[:, :kb, :], scalar1=0.0,
                        scalar2=a_sb[:, :1], op0=Alu.min, op1=Alu.mult)
                    pos = tpool.tile([P, nb4, D], f32, tag="pos")
                    nc.vector.tensor_scalar(
                        out=pos[:, :kb, :], in0=hb[:, :kb, :], scalar1=0.0,
                        scalar2=None, op0=Alu.max)
                    nc.vector.tensor_tensor(
                        out=stg[:, kq : kq + kb, :], in0=neg[:, :kb, :],
                        in1=pos[:, :kb, :], op=Alu.add)
                kq += kb
            nc.sync.dma_start(
                out=out[g, :, k0 * D : (k0 + kn) * D],
                in_=stg[:].rearrange("p k d -> p (k d)"))
    return out


# --------------------------------------------------------------------------
# Driver
# --------------------------------------------------------------------------
def _build_program(cfg: Config, meta):
    import concourse.bacc as bacc
    import concourse.tile as tile

    nc = bacc.Bacc("TRN2", target_bir_lowering=False, debug=False,
                   enable_asserts=False, num_devices=cfg.n_cores)
    with tile.TileContext(nc) as tc:
        build_kernel(nc, tc, cfg, meta)
    nc.compile()
    return nc


def _unscramble(results, plans, cfg: Config):
    n = cfg.n_nodes
    full = np.zeros((2, n, D), np.float32)
    for g in range(2):
        ct_all = plans[g]["core_tiles"]
        for core in range(cfg.n_cores):
            # [2, P, t_core*D] bf16
            oc = np.asarray(results[core]["out"]).astype(np.float32)
            for k in range(cfg.t_core):
                t = int(ct_all[core, k])
                if t < 0:
                    continue
                r0 = t * P
                r1 = min(r0 + P, n)
                full[g, r0:r1] = oc[g, : r1 - r0, k * D : (k + 1) * D]
    return full


_PROGRAM_CACHE = {}


def run(inputs, cfg: Config, trace=False):
    from concourse.bass_utils import run_bass_kernel_spmd

    in_maps, plans, meta = preprocess(
        inputs["feats"], inputs["W"], inputs["b"], inputs["prelu_a"],
        inputs["src_pos"], inputs["dst_pos"],
        inputs["src_neg"], inputs["dst_neg"], cfg)

    key = (cfg.n_nodes, cfg.n_cores, cfg.sg, cfg.sg_first, cfg.sg_tail,
           cfg.act_prelu, cfg.act_batch,
           cfg.oh_pool_mod, cfg.gbufs, cfg.maxc,
           meta["cstar"][0].tobytes(), meta["cstar"][1].tobytes(),
           meta["use_bias"])
    nc = _PROGRAM_CACHE.get(key)
    if nc is None:
        nc = _build_program(cfg, meta)
        _PROGRAM_CACHE[key] = nc

    kwargs = {}
    if trace:
        kwargs = dict(trace=True, tmpdir=tempfile.mkdtemp(prefix="bgc_trace_"))
    res = run_bass_kernel_spmd(nc, in_maps, core_ids=list(range(cfg.n_cores)),
                               **kwargs)
    full = _unscramble(res.results, plans, cfg)
    return full, res


def kernel(**inputs) -> np.ndarray:
    cfg = Config()
    full, _ = run(inputs, cfg)
    return full


# revision 28
# speedup vs baseline: 1.8703x; 1.0004x over previous
"""Bass/Trainium2 kernel for BiGraphContrastLayer (GNN message passing).

Computes, for two edge lists (pos/neg) over the same node features:
    h_g = PReLU( D_in^-1/2 A_g D_out^-1/2 feats @ W + b )
returning stack([h_pos, h_neg]) of shape [2, N, Dout].

Strategy (8 NeuronCores, SPMD, no collectives). Uses linearity twice:
    nd_d * sum_e ns_s feats_s  =  sum_e (ns_s nd_d) feats_s
    (agg) @ W                  =  W applied once per dst tile after agg

  Single device pass: edges are bucketed by 128-node dst tile and sorted by
  src within (tile, row-bank); dma_gather (int16 idx over 4 row-bank views
  of one bf16 feats table) pulls feats[src] rows for ~128-edge chunks; a
  weighted one-hot matmul (lhsT = gathered chunk, rhs = onehot of dst offsets
  scaled by the per-edge weight ns_src*nd_dst) accumulates the TRANSPOSED
  aggregate aggT[din, dst] for each dst tile in PSUM; aggT -> bf16 SBUF
  (batched Act copy over 4 tiles), one matmul per tile applies W, and a
  batched PReLU produces the output tile.

  Host does index prep (degree bincounts -> per-edge norm weights, sorting,
  bucketing, dealing dst tiles to cores so all 8 cores share one instruction
  stream) and stages the bf16 feats table; all O(E*D) and O(N*D^2) work runs
  on device.
"""

import math
import tempfile
from dataclasses import dataclass

import numpy as np

P = 128   # partitions
D = 128   # feature dim (Din == Dout == 128)
NBANK = 4  # row-range banks (int16 gather idx addresses <=32768 rows)


# --------------------------------------------------------------------------
# Config
# --------------------------------------------------------------------------
@dataclass
class Config:
    n_nodes: int = 100000
    n_cores: int = 8
    sg: int = 14       # dst-tile positions per gather supergroup
    sg_first: int = 2  # size of the first supergroup (fast pipeline fill)
    sg_tail: int = 2   # max size of the last supergroup (short drain)
    act_prelu: bool = True    # final PReLU on ScalarE (not in sim)
    act_batch: bool = True    # batch aggT copy / prelu over 4 tiles
    oh_pool_mod: int = 0      # every Nth one-hot build goes to GpSimd (0=off)
    maxc: int = 48            # max chunks per dma_gather call
    gbufs: int = 3            # gather buffer count
    ipool_bufs: int = 3
    ohpool_bufs: int = 8

    @property
    def t_global(self) -> int:
        return math.ceil(self.n_nodes / P)

    @property
    def n_pad(self) -> int:
        return self.t_global * P

    @property
    def t_core(self) -> int:
        return math.ceil(self.t_global / self.n_cores)

    @property
    def bank_rows(self) -> int:
        return math.ceil(self.t_global / NBANK) * P


# --------------------------------------------------------------------------
# Host-side preprocessing
# --------------------------------------------------------------------------
def _norm(deg):
    deg = deg.astype(np.float64)
    return np.where(deg > 0, 1.0 / np.sqrt(np.maximum(deg, 1.0)), 0.0).astype(
        np.float32)


def _plan_graph(src, dst, wgt, cfg: Config):
    """Bucket edges by dst tile, split by src row-bank, sort by src within
    (tile, bank), and deal tiles to cores with signature matching.

    Returns dict with:
      core_tiles  [n_cores, t_core]  global tile id per position (-1 null)
      cstar       [t_core, NBANK]    shared chunk counts per position/bank
      tile_edges  list per global tile: (idx16, off, wgt, bank_cnt)
    """
    tg, ncores, tcore = cfg.t_global, cfg.n_cores, cfg.t_core
    order = np.argsort(dst, kind="stable")
    src_s = src[order]
    dst_s = dst[order]
    wgt_s = wgt[order]
    tile_cnt = np.bincount(dst_s // P, minlength=tg)
    starts = np.zeros(tg + 1, np.int64)
    np.cumsum(tile_cnt, out=starts[1:])

    sig = np.zeros((tg, NBANK), np.int64)
    tile_edges = []
    for t in range(tg):
        e0, e1 = int(starts[t]), int(starts[t + 1])
        ts_src = src_s[e0:e1]
        bank = ts_src // cfg.bank_rows
        so = np.argsort(bank * (2 ** 40) + ts_src, kind="stable")
        ts_src = ts_src[so]
        ts_idx = ts_src - bank[so] * cfg.bank_rows
        ts_off = (dst_s[e0:e1][so] % P).astype(np.float32)
        ts_wgt = wgt_s[e0:e1][so]
        bc = np.bincount(bank[so], minlength=NBANK)
        sig[t] = -(-bc // P)  # ceil chunks per bank
        sig[t, 0] = max(sig[t, 0], 1)
        tile_edges.append((ts_idx, ts_off, ts_wgt, bc))

    # Deal: sort tiles by signature so consecutive groups of n_cores tiles
    # have matching/near-matching per-bank chunk counts.
    sigkey = sig @ (np.int64(32) ** np.arange(NBANK - 1, -1, -1))
    keys = np.argsort(sigkey, kind="stable")
    n_slots = ncores * tcore
    dealt = np.full(n_slots, -1, np.int64)
    dealt[: len(keys)] = keys[::-1]  # descending signature order
    core_tiles = np.zeros((ncores, tcore), np.int64)
    cstar = np.zeros((tcore, NBANK), np.int64)
    for k in range(tcore):
        grp = dealt[k * ncores : (k + 1) * ncores]
        for c in range(ncores):
            core_tiles[c, k] = grp[c]
        s = np.zeros(NBANK, np.int64)
        for t in grp:
            if t >= 0:
                s = np.maximum(s, sig[t])
        s[0] = max(s[0], 1)
        cstar[k] = s
    return dict(core_tiles=core_tiles, cstar=cstar, tile_edges=tile_edges)


def _slot_layout(cstar, cfg: Config):
    """Shared (all-cores) slot layout for one graph.

    Slot space = sequence of supergroups; within a supergroup, bank-major:
      for b in banks: for k in sg positions: cstar[k, b] chunks.
    Returns:
      sg_list: list of (k0, kn)
      calls:   list of (sg_idx, bank, chunk0, nchunks)  [nchunks > 0]
      chunk_of: dict (k, b, c) -> global chunk index
      n_chunks total
    """
    tcore, sg = cfg.t_core, cfg.sg
    sizes = []
    rem = tcore
    if cfg.sg_first and rem > cfg.sg_first:
        sizes.append(cfg.sg_first)
        rem -= cfg.sg_first
    while rem > 0:
        kn = min(sg, rem)
        sizes.append(kn)
        rem -= kn
    # split an oversized final group so the drain after the last gather is
    # short
    if cfg.sg_tail and len(sizes) > 1 and sizes[-1] > cfg.sg_tail:
        last = sizes.pop()
        h = last - cfg.sg_tail
        sizes.extend([h, cfg.sg_tail])
    sg_list = []
    k0 = 0
    for kn in sizes:
        sg_list.append((k0, kn))
        k0 += kn
    calls = []
    chunk_of = {}
    cpos = 0
    for si, (k0, kn) in enumerate(sg_list):
        for b in range(NBANK):
            c0 = cpos
            for ki in range(kn):
                k = k0 + ki
                for c in range(int(cstar[k, b])):
                    chunk_of[(k, b, c)] = cpos
                    cpos += 1
            while c0 < cpos:
                nch = min(cfg.maxc, cpos - c0)
                calls.append((si, b, c0, nch))
                c0 += nch
    return sg_list, calls, chunk_of, cpos


def _fill_core_graph(plan, layout, core, cfg: Config):
    """Build IDX16 (wrapped), OFF, and WGT arrays for one core, one graph."""
    sg_list, calls, chunk_of, n_chunks = layout
    cstar = plan["cstar"]
    idx = np.zeros((n_chunks, P), np.int16)
    off = np.full((n_chunks, P), 512.0, np.float32)
    wgt = np.zeros((n_chunks, P), np.float32)
    for k in range(cfg.t_core):
        t = plan["core_tiles"][core, k]
        if t < 0:
            continue
        ts_idx, ts_off, ts_wgt, bc = plan["tile_edges"][t]
        bstart = np.zeros(NBANK + 1, np.int64)
        np.cumsum(bc, out=bstart[1:])
        for b in range(NBANK):
            nb = int(bc[b])
            cnum = int(cstar[k, b])
            if cnum == 0:
                continue
            nslot = cnum * P
            es = np.zeros(nslot, np.int64)
            eo = np.full(nslot, 512.0, np.float32)
            ew = np.zeros(nslot, np.float32)
            es[:nb] = ts_idx[bstart[b] : bstart[b + 1]]
            eo[:nb] = ts_off[bstart[b] : bstart[b + 1]]
            ew[:nb] = ts_wgt[bstart[b] : bstart[b + 1]]
            for c in range(cnum):
                ci = chunk_of[(k, b, c)]
                idx[ci] = es[c * P : (c + 1) * P]
                off[ci] = eo[c * P : (c + 1) * P]
                wgt[ci] = ew[c * P : (c + 1) * P]
    # wrap: flat slot i (within a call's range) -> [i%16, i//16], replicated
    # to 128 partitions. Build per call, concatenated along columns.
    ncols = n_chunks * P // 16
    idx_w = np.zeros((P, ncols), np.int16)
    for (si, b, c0, nch) in calls:
        flat = idx[c0 : c0 + nch].reshape(-1)  # ni slots
        blk = flat.reshape(-1, 16).T  # [16, ni/16]
        idx_w[:, c0 * 8 : (c0 + nch) * 8] = np.tile(blk, (8, 1))
    return idx_w, off.T.copy(), wgt.T.copy()  # -> [P, n_chunks] f32


def preprocess(feats, W, b, prelu_a, src_pos, dst_pos, src_neg, dst_neg,
               cfg: Config):
    n = cfg.n_nodes
    feats = np.asarray(feats, np.float32)
    W = np.asarray(W, np.float32)
    b = np.asarray(b, np.float32)
    prelu_a = np.asarray(prelu_a, np.float32)
    bf16 = np.dtype("bfloat16")

    feats_pad = np.zeros((NBANK * cfg.bank_rows, D), np.float32)
    feats_pad[:n] = feats
    fb16 = feats_pad.astype(bf16)

    plans, layouts = [], []
    for src, dst in ((src_pos, dst_pos), (src_neg, dst_neg)):
        src = np.asarray(src, np.int64)
        dst = np.asarray(dst, np.int64)
        ns = _norm(np.bincount(src, minlength=n))
        nd = _norm(np.bincount(dst, minlength=n))
        # merge duplicate (src, dst) edges: by linearity their contributions
        # sum, so one gathered row with a summed weight is exact.
        pair = src * np.int64(1 << 32) + dst
        upair, ucnt = np.unique(pair, return_counts=True)
        usrc = (upair >> 32).astype(np.int64)
        udst = (upair & ((1 << 32) - 1)).astype(np.int64)
        wgt = (ucnt * ns[usrc] * nd[udst]).astype(np.float32)
        plan = _plan_graph(usrc, udst, wgt, cfg)
        plans.append(plan)
        layouts.append(_slot_layout(plan["cstar"], cfg))

    iota = np.tile(np.arange(P, dtype=np.float32), (P, 1)).astype(bf16)
    a_rep = np.full((P, 1), float(prelu_a.reshape(-1)[0]), np.float32)
    b_rep = np.tile(b.reshape(1, D), (P, 1)).astype(np.float32)
    use_bias = bool(np.any(b != 0.0))

    in_maps = []
    for core in range(cfg.n_cores):
        iw_p, off_p, wgt_p = _fill_core_graph(plans[0], layouts[0], core, cfg)
        iw_n, off_n, wgt_n = _fill_core_graph(plans[1], layouts[1], core, cfg)
        m = {
            "fb16": fb16,
            "w_in": W,
            "a_rep": a_rep,
            "idx_in": np.concatenate([iw_p, iw_n], axis=1),
            "off_in": np.concatenate([off_p, off_n], axis=1),
            "wgt_in": np.concatenate([wgt_p, wgt_n], axis=1),
            "iota_in": iota,
        }
        if use_bias:
            m["b_rep"] = b_rep
        in_maps.append(m)
    meta = {
        "layouts": layouts,
        "cstar": [plans[0]["cstar"], plans[1]["cstar"]],
        "use_bias": use_bias,
    }
    return in_maps, plans, meta


# --------------------------------------------------------------------------
# Device kernel builder
# --------------------------------------------------------------------------
def build_kernel(nc, tc, cfg: Config, meta):
    from contextlib import ExitStack

    import concourse.mybir as mybir

    f32 = mybir.dt.float32
    bf16 = mybir.dt.bfloat16
    i16 = mybir.dt.int16
    Alu = mybir.AluOpType
    Act = mybir.ActivationFunctionType

    tcore, npad = cfg.t_core, cfg.n_pad
    layouts = meta["layouts"]
    cstar = meta["cstar"]
    use_bias = meta["use_bias"]
    n_chunks = [layouts[g][3] for g in range(2)]
    ncols = [n_chunks[g] * P // 16 for g in range(2)]

    fb = nc.dram_tensor("fb16", [NBANK * cfg.bank_rows, D], bf16,
                        kind="ExternalInput").ap()
    w_in = nc.dram_tensor("w_in", [P, D], f32, kind="ExternalInput").ap()
    a_rep = nc.dram_tensor("a_rep", [P, 1], f32, kind="ExternalInput").ap()
    idx_in = nc.dram_tensor("idx_in", [P, sum(ncols)], i16,
                            kind="ExternalInput").ap()
    off_in = nc.dram_tensor("off_in", [P, sum(n_chunks)], f32,
                            kind="ExternalInput").ap()
    wgt_in = nc.dram_tensor("wgt_in", [P, sum(n_chunks)], f32,
                            kind="ExternalInput").ap()
    iota_in = nc.dram_tensor("iota_in", [P, P], bf16, kind="ExternalInput").ap()
    if use_bias:
        b_rep = nc.dram_tensor("b_rep", [P, D], f32, kind="ExternalInput").ap()
    out = nc.dram_tensor("out", [2, P, tcore * D], bf16,
                         kind="ExternalOutput").ap()

    with ExitStack() as ctx:
        const = ctx.enter_context(tc.tile_pool(name="const", bufs=1))
        gpool = ctx.enter_context(tc.tile_pool(name="gpool", bufs=cfg.gbufs))
        ipool = ctx.enter_context(tc.tile_pool(name="ipool", bufs=cfg.ipool_bufs))
        ohpool = ctx.enter_context(tc.tile_pool(name="ohpool",
                                                bufs=cfg.ohpool_bufs))
        atpool = ctx.enter_context(tc.tile_pool(name="atpool", bufs=3))
        tpool = ctx.enter_context(tc.tile_pool(name="tpool", bufs=4))
        spool = ctx.enter_context(tc.tile_pool(name="spool", bufs=3))
        apool = ctx.enter_context(tc.tile_pool(name="apool", bufs=3,
                                               space="PSUM"))
        hpool = ctx.enter_context(tc.tile_pool(name="hpool", bufs=3,
                                               space="PSUM"))

        # ---- constants ----
        w_sb = const.tile([P, D], bf16)
        nc.gpsimd.dma_start(out=w_sb[:], in_=w_in)  # f32 -> bf16 cast DMA
        iota_sb = const.tile([P, P], bf16)
        nc.sync.dma_start(out=iota_sb[:], in_=iota_in)
        a_sb = const.tile([P, 1], f32)
        nc.sync.dma_start(out=a_sb[:], in_=a_rep)
        if use_bias:
            b_sb = const.tile([P, D], f32)
            nc.sync.dma_start(out=b_sb[:], in_=b_rep)

        # ---- gather + weighted one-hot segment-sum + W + prelu ----
        col_base = [0, ncols[0]]          # idx column offset per graph
        chk_base = [0, n_chunks[0]]       # off/wgt column offset per graph
        cbs_all = []
        for g in range(2):
            calls_by_sg = {}
            for (si, b, c0, nch) in layouts[g][1]:
                calls_by_sg.setdefault(si, []).append((b, c0, nch))
            cbs_all.append(calls_by_sg)
        # interleave the two graphs' supergroups so one graph's gathers fill
        # DMA while the other's PSUM chain drains
        jobs = []
        for si in range(max(len(layouts[0][0]), len(layouts[1][0]))):
            for g in range(2):
                if si < len(layouts[g][0]):
                    jobs.append((g, si))
        oh_ct = 0
        for (g, si) in jobs:
            sg_list, calls, chunk_of, _ = layouts[g]
            cs = cstar[g]
            (k0, kn) = sg_list[si]
            sg_chunks = sum(int(cs[k0 + ki, b]) for ki in range(kn)
                            for b in range(NBANK))
            c0_sg = chunk_of[(k0, 0, 0)]
            gt = gpool.tile([P, sg_chunks, D], bf16, tag="gather")
            it = ipool.tile([P, sg_chunks * 8], i16, tag="gidx")
            nc.sync.dma_start(
                out=it[:],
                in_=idx_in[:, col_base[g] + c0_sg * 8 :
                           col_base[g] + (c0_sg + sg_chunks) * 8])
            ot = ipool.tile([P, sg_chunks], f32, tag="goff")
            nc.sync.dma_start(
                out=ot[:],
                in_=off_in[:, chk_base[g] + c0_sg :
                           chk_base[g] + c0_sg + sg_chunks])
            wt = ipool.tile([P, sg_chunks], f32, tag="gwgt")
            nc.sync.dma_start(
                out=wt[:],
                in_=wgt_in[:, chk_base[g] + c0_sg :
                           chk_base[g] + c0_sg + sg_chunks])
            for (b, c0, nch) in cbs_all[g][si]:
                lo = c0 - c0_sg
                nc.gpsimd.dma_gather(
                    out_ap=gt[:, lo : lo + nch, :],
                    in_ap=fb[b * cfg.bank_rows : (b + 1) * cfg.bank_rows, :],
                    idxs_ap=it[:, lo * 8 : (lo + nch) * 8],
                    num_idxs=nch * P, num_idxs_reg=nch * P,
                    elem_size=D, single_packet=False)
            stg = spool.tile([P, kn, D], bf16, tag="stg")
            nb4 = 4 if cfg.act_batch else 1
            kq = 0
            while kq < kn:
                kb = min(nb4, kn - kq)
                agg4 = apool.tile([P, nb4, D], f32)
                for j in range(kb):
                    k = k0 + kq + j
                    nonzero = [(b, c) for b in range(NBANK)
                               for c in range(int(cs[k, b]))]
                    for ji, (b, c) in enumerate(nonzero):
                        ci = chunk_of[(k, b, c)]
                        lo = ci - c0_sg
                        oh = ohpool.tile([P, P], bf16)
                        eng = nc.vector
                        if cfg.oh_pool_mod:
                            oh_ct += 1
                            if oh_ct % cfg.oh_pool_mod == 0:
                                eng = nc.gpsimd
                        eng.tensor_scalar(
                            out=oh[:], in0=iota_sb[:],
                            scalar1=ot[:, lo : lo + 1],
                            scalar2=wt[:, lo : lo + 1],
                            op0=Alu.is_equal, op1=Alu.mult)
                        nc.tensor.matmul(
                            out=agg4[:, j, :], lhsT=gt[:, lo, :], rhs=oh[:],
                            start=(ji == 0), stop=(ji == len(nonzero) - 1))
                at4 = atpool.tile([P, nb4, D], bf16, tag="at4")
                nc.scalar.activation(out=at4[:, :kb, :], in_=agg4[:, :kb, :],
                                     func=Act.Copy)
                h4 = hpool.tile([P, nb4, D], f32)
                for j in range(kb):
                    nc.tensor.matmul(out=h4[:, j, :], lhsT=at4[:, j, :],
                                     rhs=w_sb[:], start=True, stop=True)
                if cfg.act_prelu and not use_bias:
                    nc.scalar.activation(
                        out=stg[:, kq : kq + kb, :], in_=h4[:, :kb, :],
                        func=Act.Prelu, alpha=a_sb[:, :1])
                elif not use_bias:
                    neg = tpool.tile([P, nb4, D], f32, tag="neg")
                    nc.vector.tensor_scalar(
                        out=neg[:, :kb, :], in0=h4[:, :kb, :], scalar1=0.0,
                        scalar2=a_sb[:, :1], op0=Alu.min, op1=Alu.mult)
                    pos = tpool.tile([P, nb4, D], f32, tag="pos")
                    nc.vector.tensor_scalar(
                        out=pos[:, :kb, :], in0=h4[:, :kb, :], scalar1=0.0,
                        scalar2=None, op0=Alu.max)
                    nc.vector.tensor_tensor(
                        out=stg[:, kq : kq + kb, :], in0=neg[:, :kb, :],
                        in1=pos[:, :kb, :], op=Alu.add)
                else:
                    hb = tpool.tile([P, nb4, D], f32, tag="hb")
                    nc.vector.tensor_tensor(
                        out=hb[:, :kb, :], in0=h4[:, :kb, :],
                        in1=b_sb[:, None, :].to_broadcast([P, kb, D]),
                        op=Alu.add)
                    neg = tpool.tile([P, nb4, D], f32, tag="neg")
                    nc.vector.tensor_scalar(
                        out=neg[:, :kb, :], in0=hb[:, :kb, :], scalar1=0.0,
                        scalar2=a_sb[:, :1], op0=Alu.min, op1=Alu.mult)
                    pos = tpool.tile([P, nb4, D], f32, tag="pos")
                    nc.vector.tensor_scalar(
                        out=pos[:, :kb, :], in0=hb[:, :kb, :], scalar1=0.0,
                        scalar2=None, op0=Alu.max)
                    nc.vector.tensor_tensor(
                        out=stg[:, kq : kq + kb, :], in0=neg[:, :kb, :],
                        in1=pos[:, :kb, :], op=Alu.add)
                kq += kb
            nc.sync.dma_start(
                out=out[g, :, k0 * D : (k0 + kn) * D],
                in_=stg[:].rearrange("p k d -> p (k d)"))
    return out


# --------------------------------------------------------------------------
# Driver
# --------------------------------------------------------------------------
def _build_program(cfg: Config, meta):
    import concourse.bacc as bacc
    import concourse.tile as tile

    nc = bacc.Bacc("TRN2", target_bir_lowering=False, debug=False,
                   enable_asserts=False, num_devices=cfg.n_cores)
    with tile.TileContext(nc) as tc:
        build_kernel(nc, tc, cfg, meta)
    nc.compile()
    return nc


def _unscramble(results, plans, cfg: Config):
    n = cfg.n_nodes
    full = np.zeros((2, n, D), np.float32)
    for g in range(2):
        ct_all = plans[g]["core_tiles"]
        for core in range(cfg.n_cores):
            # [2, P, t_core*D] bf16
            oc = np.asarray(results[core]["out"]).astype(np.float32)
            for k in range(cfg.t_core):
                t = int(ct_all[core, k])
                if t < 0:
                    continue
                r0 = t * P
                r1 = min(r0 + P, n)
                full[g, r0:r1] = oc[g, : r1 - r0, k * D : (k + 1) * D]
    return full


_PROGRAM_CACHE = {}


def run(inputs, cfg: Config, trace=False):
    from concourse.bass_utils import run_bass_kernel_spmd

    in_maps, plans, meta = preprocess(
        inputs["feats"], inputs["W"], inputs["b"], inputs["prelu_a"],
        inputs["src_pos"], inputs["dst_pos"],
        inputs["src_neg"], inputs["dst_neg"], cfg)

    key = (cfg.n_nodes, cfg.n_cores, cfg.sg, cfg.sg_first, cfg.sg_tail,
           cfg.act_prelu, cfg.act_batch,
           cfg.oh_pool_mod, cfg.gbufs, cfg.maxc,
           meta["cstar"][0].tobytes(), meta["cstar"][1].tobytes(),
           meta["use_bias"])
    nc = _PROGRAM_CACHE.get(key)
    if nc is None:
        nc = _build_program(cfg, meta)
        _PROGRAM_CACHE[key] = nc

    kwargs = {}
    if trace:
        kwargs = dict(trace=True, tmpdir=tempfile.mkdtemp(prefix="bgc_trace_"))
    res = run_bass_kernel_spmd(nc, in_maps, core_ids=list(range(cfg.n_cores)),
                               **kwargs)
    full = _unscramble(res.results, plans, cfg)
    return full, res


def kernel(**inputs) -> np.ndarray:
    cfg = Config()
    full, _ = run(inputs, cfg)
    return full


# revision 36
# speedup vs baseline: 2.0500x; 1.0961x over previous
"""Bass/Trainium2 kernel for BiGraphContrastLayer (GNN message passing).

Computes, for two edge lists (pos/neg) over the same node features:
    h_g = PReLU( D_in^-1/2 A_g D_out^-1/2 feats @ W + b )
returning stack([h_pos, h_neg]) of shape [2, N, Dout].

Strategy (8 NeuronCores, SPMD, no collectives). Uses linearity twice:
    nd_d * sum_e ns_s feats_s  =  sum_e (ns_s nd_d) feats_s
    (agg) @ W                  =  W applied once per dst tile after agg

  Single device pass: edges are bucketed by 128-node dst tile and sorted by
  src within (tile, row-bank); dma_gather (int16 idx over 4 row-bank views
  of one bf16 feats table) pulls feats[src] rows for ~128-edge chunks; a
  weighted one-hot matmul (lhsT = gathered chunk, rhs = onehot of dst offsets
  scaled by the per-edge weight ns_src*nd_dst) accumulates the TRANSPOSED
  aggregate aggT[din, dst] for each dst tile in PSUM; aggT -> bf16 SBUF
  (batched Act copy over 4 tiles), one matmul per tile applies W, and a
  batched PReLU produces the output tile.

  Host does index prep (degree bincounts -> per-edge norm weights, sorting,
  bucketing, dealing dst tiles to cores so all 8 cores share one instruction
  stream) and stages the bf16 feats table; all O(E*D) and O(N*D^2) work runs
  on device.
"""

import math
import tempfile
from dataclasses import dataclass

import numpy as np

P = 128   # partitions
D = 128   # feature dim (Din == Dout == 128)
NBANK = 4  # row-range banks (int16 gather idx addresses <=32768 rows)


# --------------------------------------------------------------------------
# Config
# --------------------------------------------------------------------------
@dataclass
class Config:
    n_nodes: int = 100000
    n_cores: int = 8
    sg: int = 14       # dst-tile positions per gather supergroup
    sg_first: int = 2  # size of the first supergroup (fast pipeline fill)
    sg_tail: int = 2   # max size of the last supergroup (short drain)
    act_prelu: bool = True    # final PReLU on ScalarE (not in sim)
    act_batch: bool = True    # batch aggT copy / prelu over 4 tiles
    oh_pool_mod: int = 0      # every Nth one-hot build goes to GpSimd (0=off)
    maxc: int = 48            # max chunks per dma_gather call
    gbufs: int = 3            # gather buffer count
    ipool_bufs: int = 3
    ohpool_bufs: int = 8

    @property
    def t_global(self) -> int:
        return math.ceil(self.n_nodes / P)

    @property
    def n_pad(self) -> int:
        return self.t_global * P

    @property
    def t_core(self) -> int:
        return math.ceil(self.t_global / self.n_cores)

    @property
    def bank_rows(self) -> int:
        return math.ceil(self.t_global / NBANK) * P


# --------------------------------------------------------------------------
# Host-side preprocessing
# --------------------------------------------------------------------------
def _norm(deg):
    deg = deg.astype(np.float64)
    return np.where(deg > 0, 1.0 / np.sqrt(np.maximum(deg, 1.0)), 0.0).astype(
        np.float32)


def _assign_banks(edge_lists, cfg: Config):
    """Assign nodes to gather banks (<=32768 rows each, int16 idx reach) to
    minimize per-(dst tile, bank) ceil-to-128 chunk fragmentation.

    Starts from contiguous row-range banks, then repairs buckets whose count
    sits just above a multiple of 128 by moving that remainder's nodes to
    other banks (each move's waste change across the node's other buckets is
    ~zero-sum, so erasing the remainder deletes a whole chunk).
    Deterministic. Returns (bank_of[N], table_row[N], bases, sizes).
    """
    n, tg = cfg.n_nodes, cfg.t_global
    srcs, tils = [], []
    for g, (src, dst) in enumerate(edge_lists):
        pair = np.unique(src * np.int64(1 << 32) + dst)
        srcs.append(pair >> 32)
        tils.append(g * tg + (pair & ((1 << 32) - 1)) // P)
    src_all = np.concatenate(srcs)
    til_all = np.concatenate(tils)
    o = np.argsort(src_all, kind="stable")
    sa, ta = src_all[o], til_all[o]
    nstarts = np.searchsorted(sa, np.arange(n + 1))
    o2 = np.argsort(til_all, kind="stable")
    tn_node = src_all[o2]
    tstarts = np.searchsorted(til_all[o2], np.arange(2 * tg + 1))

    cap = 32768
    span = min(cfg.bank_rows, cap)
    bank_of = np.minimum(np.arange(n) // span, NBANK - 1).astype(np.int8)
    counts = np.zeros((2 * tg, NBANK), np.int32)
    np.add.at(counts, (ta, bank_of[sa]), 1)
    fill = np.bincount(bank_of, minlength=NBANK).astype(np.int64)

    def move(node, b1):
        b0 = bank_of[node]
        t_n = ta[nstarts[node]:nstarts[node + 1]]
        counts[t_n, b0] -= 1
        counts[t_n, b1] += 1
        bank_of[node] = b1
        fill[b0] -= 1
        fill[b1] += 1

    def marginal_all(node):
        b0 = bank_of[node]
        t_n = ta[nstarts[node]:nstarts[node + 1]]
        cb = counts[t_n]
        rem = np.where(cb[:, b0] % 128 == 1, -127, 1).sum()
        d = np.where(cb % 128 == 0, 127, -1).sum(0) + rem
        d[b0] = 1 << 30
        d[fill >= cap] = 1 << 30
        return d

    for _ in range(2):
        r = counts % 128
        cand = np.argwhere((r > 0) & (r <= 40) & (counts > 128))
        if len(cand) == 0:
            break
        for (t, b) in cand[np.argsort(r[cand[:, 0], cand[:, 1]], kind="stable")]:
            rr = int(counts[t, b] % 128)
            if rr == 0 or rr > 40 or counts[t, b] <= 128:
                continue
            nodes = tn_node[tstarts[t]:tstarts[t + 1]]
            nodes = nodes[bank_of[nodes] == b]
            if len(nodes) < rr:
                continue
            ranked = sorted((int(marginal_all(nd).min()), int(nd))
                            for nd in nodes)[:rr]
            applied, tot = [], 0
            for (_, nd) in ranked:
                d = marginal_all(nd)
                b1 = int(np.argmin(d))
                tot += int(d[b1])
                move(nd, b1)
                applied.append(nd)
            if tot >= 0:
                for nd in reversed(applied):
                    move(nd, b)

    order = np.argsort(bank_of, kind="stable")
    table_row = np.empty(n, np.int64)
    table_row[order] = np.arange(n)
    sizes = np.bincount(bank_of, minlength=NBANK).astype(np.int64)
    bases = np.zeros(NBANK + 1, np.int64)
    np.cumsum(sizes, out=bases[1:])
    return bank_of, table_row, bases[:NBANK], sizes


def _plan_graph(src, dst, wgt, bank_of, table_row, bases, cfg: Config):
    """Bucket edges by dst tile, split by src bank (host-assigned), sort by
    table row within (tile, bank), and deal tiles to cores with signature
    matching.

    Returns dict with:
      core_tiles  [n_cores, t_core]  global tile id per position (-1 null)
      cstar       [t_core, NBANK]    shared chunk counts per position/bank
      tile_edges  list per global tile: (idx16, off, wgt, bank_cnt)
    """
    tg, ncores, tcore = cfg.t_global, cfg.n_cores, cfg.t_core
    order = np.argsort(dst, kind="stable")
    src_s = src[order]
    dst_s = dst[order]
    wgt_s = wgt[order]
    tile_cnt = np.bincount(dst_s // P, minlength=tg)
    starts = np.zeros(tg + 1, np.int64)
    np.cumsum(tile_cnt, out=starts[1:])

    sig = np.zeros((tg, NBANK), np.int64)
    tile_edges = []
    for t in range(tg):
        e0, e1 = int(starts[t]), int(starts[t + 1])
        ts_src = src_s[e0:e1]
        bank = bank_of[ts_src].astype(np.int64)
        row = table_row[ts_src]
        so = np.argsort(bank * (2 ** 40) + row, kind="stable")
        ts_idx = (row - bases[bank])[so]
        assert len(ts_idx) == 0 or (0 <= ts_idx.min() and ts_idx.max() < 32768)
        ts_off = (dst_s[e0:e1][so] % P).astype(np.float32)
        ts_wgt = wgt_s[e0:e1][so]
        bc = np.bincount(bank[so], minlength=NBANK)
        sig[t] = -(-bc // P)  # ceil chunks per bank
        tile_edges.append((ts_idx, ts_off, ts_wgt, bc))

    # Deal: sort tiles by signature so consecutive groups of n_cores tiles
    # have matching/near-matching per-bank chunk counts.
    sigkey = sig @ (np.int64(32) ** np.arange(NBANK - 1, -1, -1))
    keys = np.argsort(sigkey, kind="stable")
    n_slots = ncores * tcore
    dealt = np.full(n_slots, -1, np.int64)
    dealt[: len(keys)] = keys[::-1]  # descending signature order
    core_tiles = np.zeros((ncores, tcore), np.int64)
    cstar = np.zeros((tcore, NBANK), np.int64)
    for k in range(tcore):
        grp = dealt[k * ncores : (k + 1) * ncores]
        for c in range(ncores):
            core_tiles[c, k] = grp[c]
        s = np.zeros(NBANK, np.int64)
        for t in grp:
            if t >= 0:
                s = np.maximum(s, sig[t])
        if s.sum() == 0:
            s[0] = 1  # every position needs >=1 chunk (PSUM init matmul)
        cstar[k] = s
    return dict(core_tiles=core_tiles, cstar=cstar, tile_edges=tile_edges)


def _slot_layout(cstar, cfg: Config):
    """Shared (all-cores) slot layout for one graph.

    Slot space = sequence of supergroups; within a supergroup, bank-major:
      for b in banks: for k in sg positions: cstar[k, b] chunks.
    Returns:
      sg_list: list of (k0, kn)
      calls:   list of (sg_idx, bank, chunk0, nchunks)  [nchunks > 0]
      chunk_of: dict (k, b, c) -> global chunk index
      n_chunks total
    """
    tcore, sg = cfg.t_core, cfg.sg
    sizes = []
    rem = tcore
    if cfg.sg_first and rem > cfg.sg_first:
        sizes.append(cfg.sg_first)
        rem -= cfg.sg_first
    while rem > 0:
        kn = min(sg, rem)
        sizes.append(kn)
        rem -= kn
    # split an oversized final group so the drain after the last gather is
    # short
    if cfg.sg_tail and len(sizes) > 1 and sizes[-1] > cfg.sg_tail:
        last = sizes.pop()
        h = last - cfg.sg_tail
        sizes.extend([h, cfg.sg_tail])
    sg_list = []
    k0 = 0
    for kn in sizes:
        sg_list.append((k0, kn))
        k0 += kn
    calls = []
    chunk_of = {}
    cpos = 0
    for si, (k0, kn) in enumerate(sg_list):
        for b in range(NBANK):
            c0 = cpos
            for ki in range(kn):
                k = k0 + ki
                for c in range(int(cstar[k, b])):
                    chunk_of[(k, b, c)] = cpos
                    cpos += 1
            while c0 < cpos:
                nch = min(cfg.maxc, cpos - c0)
                calls.append((si, b, c0, nch))
                c0 += nch
    return sg_list, calls, chunk_of, cpos


def _fill_core_graph(plan, layout, core, cfg: Config):
    """Build IDX16 (wrapped), OFF, and WGT arrays for one core, one graph."""
    sg_list, calls, chunk_of, n_chunks = layout
    cstar = plan["cstar"]
    idx = np.zeros((n_chunks, P), np.int16)
    off = np.full((n_chunks, P), 512.0, np.float32)
    wgt = np.zeros((n_chunks, P), np.float32)
    for k in range(cfg.t_core):
        t = plan["core_tiles"][core, k]
        if t < 0:
            continue
        ts_idx, ts_off, ts_wgt, bc = plan["tile_edges"][t]
        bstart = np.zeros(NBANK + 1, np.int64)
        np.cumsum(bc, out=bstart[1:])
        for b in range(NBANK):
            nb = int(bc[b])
            cnum = int(cstar[k, b])
            if cnum == 0:
                continue
            nslot = cnum * P
            es = np.zeros(nslot, np.int64)
            eo = np.full(nslot, 512.0, np.float32)
            ew = np.zeros(nslot, np.float32)
            es[:nb] = ts_idx[bstart[b] : bstart[b + 1]]
            eo[:nb] = ts_off[bstart[b] : bstart[b + 1]]
            ew[:nb] = ts_wgt[bstart[b] : bstart[b + 1]]
            for c in range(cnum):
                ci = chunk_of[(k, b, c)]
                idx[ci] = es[c * P : (c + 1) * P]
                off[ci] = eo[c * P : (c + 1) * P]
                wgt[ci] = ew[c * P : (c + 1) * P]
    # wrap: flat slot i (within a call's range) -> [i%16, i//16], replicated
    # to 128 partitions. Build per call, concatenated along columns.
    ncols = n_chunks * P // 16
    idx_w = np.zeros((P, ncols), np.int16)
    for (si, b, c0, nch) in calls:
        flat = idx[c0 : c0 + nch].reshape(-1)  # ni slots
        blk = flat.reshape(-1, 16).T  # [16, ni/16]
        idx_w[:, c0 * 8 : (c0 + nch) * 8] = np.tile(blk, (8, 1))
    return idx_w, off.T.copy(), wgt.T.copy()  # -> [P, n_chunks] f32


def preprocess(feats, W, b, prelu_a, src_pos, dst_pos, src_neg, dst_neg,
               cfg: Config):
    n = cfg.n_nodes
    feats = np.asarray(feats, np.float32)
    W = np.asarray(W, np.float32)
    b = np.asarray(b, np.float32)
    prelu_a = np.asarray(prelu_a, np.float32)
    bf16 = np.dtype("bfloat16")

    edge_lists = []
    wgts = []
    for src, dst in ((src_pos, dst_pos), (src_neg, dst_neg)):
        src = np.asarray(src, np.int64)
        dst = np.asarray(dst, np.int64)
        ns = _norm(np.bincount(src, minlength=n))
        nd = _norm(np.bincount(dst, minlength=n))
        # merge duplicate (src, dst) edges: by linearity their contributions
        # sum, so one gathered row with a summed weight is exact.
        pair = src * np.int64(1 << 32) + dst
        upair, ucnt = np.unique(pair, return_counts=True)
        usrc = (upair >> 32).astype(np.int64)
        udst = (upair & ((1 << 32) - 1)).astype(np.int64)
        edge_lists.append((usrc, udst))
        wgts.append((ucnt * ns[usrc] * nd[udst]).astype(np.float32))

    bank_of, table_row, bases, sizes = _assign_banks(edge_lists, cfg)

    feats_pad = np.zeros((cfg.n_pad, D), np.float32)
    feats_pad[table_row] = feats[:n]
    fb16 = feats_pad.astype(bf16)

    plans, layouts = [], []
    for (usrc, udst), wgt in zip(edge_lists, wgts):
        plan = _plan_graph(usrc, udst, wgt, bank_of, table_row, bases, cfg)
        plans.append(plan)
        layouts.append(_slot_layout(plan["cstar"], cfg))

    iota = np.tile(np.arange(P, dtype=np.float32), (P, 1)).astype(bf16)
    a_rep = np.full((P, 1), float(prelu_a.reshape(-1)[0]), np.float32)
    b_rep = np.tile(b.reshape(1, D), (P, 1)).astype(np.float32)
    use_bias = bool(np.any(b != 0.0))

    in_maps = []
    for core in range(cfg.n_cores):
        iw_p, off_p, wgt_p = _fill_core_graph(plans[0], layouts[0], core, cfg)
        iw_n, off_n, wgt_n = _fill_core_graph(plans[1], layouts[1], core, cfg)
        m = {
            "fb16": fb16,
            "w_in": W,
            "a_rep": a_rep,
            "idx_in": np.concatenate([iw_p, iw_n], axis=1),
            "off_in": np.concatenate([off_p, off_n], axis=1),
            "wgt_in": np.concatenate([wgt_p, wgt_n], axis=1),
            "iota_in": iota,
        }
        if use_bias:
            m["b_rep"] = b_rep
        in_maps.append(m)
    meta = {
        "layouts": layouts,
        "cstar": [plans[0]["cstar"], plans[1]["cstar"]],
        "use_bias": use_bias,
        "bank_bases": bases,
        "bank_sizes": sizes,
    }
    return in_maps, plans, meta


# --------------------------------------------------------------------------
# Device kernel builder
# --------------------------------------------------------------------------
def build_kernel(nc, tc, cfg: Config, meta):
    from contextlib import ExitStack

    import concourse.mybir as mybir

    f32 = mybir.dt.float32
    bf16 = mybir.dt.bfloat16
    i16 = mybir.dt.int16
    Alu = mybir.AluOpType
    Act = mybir.ActivationFunctionType

    tcore, npad = cfg.t_core, cfg.n_pad
    layouts = meta["layouts"]
    cstar = meta["cstar"]
    use_bias = meta["use_bias"]
    n_chunks = [layouts[g][3] for g in range(2)]
    ncols = [n_chunks[g] * P // 16 for g in range(2)]

    fb = nc.dram_tensor("fb16", [cfg.n_pad, D], bf16,
                        kind="ExternalInput").ap()
    w_in = nc.dram_tensor("w_in", [P, D], f32, kind="ExternalInput").ap()
    a_rep = nc.dram_tensor("a_rep", [P, 1], f32, kind="ExternalInput").ap()
    idx_in = nc.dram_tensor("idx_in", [P, sum(ncols)], i16,
                            kind="ExternalInput").ap()
    off_in = nc.dram_tensor("off_in", [P, sum(n_chunks)], f32,
                            kind="ExternalInput").ap()
    wgt_in = nc.dram_tensor("wgt_in", [P, sum(n_chunks)], f32,
                            kind="ExternalInput").ap()
    iota_in = nc.dram_tensor("iota_in", [P, P], bf16, kind="ExternalInput").ap()
    if use_bias:
        b_rep = nc.dram_tensor("b_rep", [P, D], f32, kind="ExternalInput").ap()
    out = nc.dram_tensor("out", [2, P, tcore * D], bf16,
                         kind="ExternalOutput").ap()

    with ExitStack() as ctx:
        const = ctx.enter_context(tc.tile_pool(name="const", bufs=1))
        gpool = ctx.enter_context(tc.tile_pool(name="gpool", bufs=cfg.gbufs))
        ipool = ctx.enter_context(tc.tile_pool(name="ipool", bufs=cfg.ipool_bufs))
        ohpool = ctx.enter_context(tc.tile_pool(name="ohpool",
                                                bufs=cfg.ohpool_bufs))
        atpool = ctx.enter_context(tc.tile_pool(name="atpool", bufs=3))
        tpool = ctx.enter_context(tc.tile_pool(name="tpool", bufs=4))
        spool = ctx.enter_context(tc.tile_pool(name="spool", bufs=3))
        apool = ctx.enter_context(tc.tile_pool(name="apool", bufs=3,
                                               space="PSUM"))
        hpool = ctx.enter_context(tc.tile_pool(name="hpool", bufs=3,
                                               space="PSUM"))

        # ---- constants ----
        w_sb = const.tile([P, D], bf16)
        nc.gpsimd.dma_start(out=w_sb[:], in_=w_in)  # f32 -> bf16 cast DMA
        iota_sb = const.tile([P, P], bf16)
        nc.sync.dma_start(out=iota_sb[:], in_=iota_in)
        a_sb = const.tile([P, 1], f32)
        nc.sync.dma_start(out=a_sb[:], in_=a_rep)
        if use_bias:
            b_sb = const.tile([P, D], f32)
            nc.sync.dma_start(out=b_sb[:], in_=b_rep)

        # ---- gather + weighted one-hot segment-sum + W + prelu ----
        col_base = [0, ncols[0]]          # idx column offset per graph
        chk_base = [0, n_chunks[0]]       # off/wgt column offset per graph
        cbs_all = []
        for g in range(2):
            calls_by_sg = {}
            for (si, b, c0, nch) in layouts[g][1]:
                calls_by_sg.setdefault(si, []).append((b, c0, nch))
            cbs_all.append(calls_by_sg)
        # interleave the two graphs' supergroups so one graph's gathers fill
        # DMA while the other's PSUM chain drains
        jobs = []
        for si in range(max(len(layouts[0][0]), len(layouts[1][0]))):
            for g in range(2):
                if si < len(layouts[g][0]):
                    jobs.append((g, si))
        oh_ct = 0
        for (g, si) in jobs:
            sg_list, calls, chunk_of, _ = layouts[g]
            cs = cstar[g]
            (k0, kn) = sg_list[si]
            sg_chunks = sum(int(cs[k0 + ki, b]) for ki in range(kn)
                            for b in range(NBANK))
            c0_sg = chunk_of[(k0, 0, 0)]
            gt = gpool.tile([P, sg_chunks, D], bf16, tag="gather")
            it = ipool.tile([P, sg_chunks * 8], i16, tag="gidx")
            nc.sync.dma_start(
                out=it[:],
                in_=idx_in[:, col_base[g] + c0_sg * 8 :
                           col_base[g] + (c0_sg + sg_chunks) * 8])
            ot = ipool.tile([P, sg_chunks], f32, tag="goff")
            nc.sync.dma_start(
                out=ot[:],
                in_=off_in[:, chk_base[g] + c0_sg :
                           chk_base[g] + c0_sg + sg_chunks])
            wt = ipool.tile([P, sg_chunks], f32, tag="gwgt")
            nc.sync.dma_start(
                out=wt[:],
                in_=wgt_in[:, chk_base[g] + c0_sg :
                           chk_base[g] + c0_sg + sg_chunks])
            bases = meta["bank_bases"]
            sizes = meta["bank_sizes"]
            for (b, c0, nch) in cbs_all[g][si]:
                lo = c0 - c0_sg
                nc.gpsimd.dma_gather(
                    out_ap=gt[:, lo : lo + nch, :],
                    in_ap=fb[int(bases[b]) : int(bases[b] + sizes[b]), :],
                    idxs_ap=it[:, lo * 8 : (lo + nch) * 8],
                    num_idxs=nch * P, num_idxs_reg=nch * P,
                    elem_size=D, single_packet=False)
            stg = spool.tile([P, kn, D], bf16, tag="stg")
            nb4 = 4 if cfg.act_batch else 1
            kq = 0
            while kq < kn:
                kb = min(nb4, kn - kq)
                agg4 = apool.tile([P, nb4, D], f32)
                for j in range(kb):
                    k = k0 + kq + j
                    nonzero = [(b, c) for b in range(NBANK)
                               for c in range(int(cs[k, b]))]
                    for ji, (b, c) in enumerate(nonzero):
                        ci = chunk_of[(k, b, c)]
                        lo = ci - c0_sg
                        oh = ohpool.tile([P, P], bf16)
                        eng = nc.vector
                        if cfg.oh_pool_mod:
                            oh_ct += 1
                            if oh_ct % cfg.oh_pool_mod == 0:
                                eng = nc.gpsimd
                        eng.tensor_scalar(
                            out=oh[:], in0=iota_sb[:],
                            scalar1=ot[:, lo : lo + 1],
                            scalar2=wt[:, lo : lo + 1],
                            op0=Alu.is_equal, op1=Alu.mult)
                        nc.tensor.matmul(
                            out=agg4[:, j, :], lhsT=gt[:, lo, :], rhs=oh[:],
                            start=(ji == 0), stop=(ji == len(nonzero) - 1))
                at4 = atpool.tile([P, nb4, D], bf16, tag="at4")
                nc.scalar.activation(out=at4[:, :kb, :], in_=agg4[:, :kb, :],
                                     func=Act.Copy)
                h4 = hpool.tile([P, nb4, D], f32)
                for j in range(kb):
                    nc.tensor.matmul(out=h4[:, j, :], lhsT=at4[:, j, :],
                                     rhs=w_sb[:], start=True, stop=True)
                if cfg.act_prelu and not use_bias:
                    nc.scalar.activation(
                        out=stg[:, kq : kq + kb, :], in_=h4[:, :kb, :],
                        func=Act.Prelu, alpha=a_sb[:, :1])
                elif not use_bias:
                    neg = tpool.tile([P, nb4, D], f32, tag="neg")
                    nc.vector.tensor_scalar(
                        out=neg[:, :kb, :], in0=h4[:, :kb, :], scalar1=0.0,
                        scalar2=a_sb[:, :1], op0=Alu.min, op1=Alu.mult)
                    pos = tpool.tile([P, nb4, D], f32, tag="pos")
                    nc.vector.tensor_scalar(
                        out=pos[:, :kb, :], in0=h4[:, :kb, :], scalar1=0.0,
                        scalar2=None, op0=Alu.max)
                    nc.vector.tensor_tensor(
                        out=stg[:, kq : kq + kb, :], in0=neg[:, :kb, :],
                        in1=pos[:, :kb, :], op=Alu.add)
                else:
                    hb = tpool.tile([P, nb4, D], f32, tag="hb")
                    nc.vector.tensor_tensor(
                        out=hb[:, :kb, :], in0=h4[:, :kb, :],
                        in1=b_sb[:, None, :].to_broadcast([P, kb, D]),
                        op=Alu.add)
                    neg = tpool.tile([P, nb4, D], f32, tag="neg")
                    nc.vector.tensor_scalar(
                        out=neg[:, :kb, :], in0=hb[:, :kb, :], scalar1=0.0,
                        scalar2=a_sb[:, :1], op0=Alu.min, op1=Alu.mult)
                    pos = tpool.tile([P, nb4, D], f32, tag="pos")
                    nc.vector.tensor_scalar(
                        out=pos[:, :kb, :], in0=hb[:, :kb, :], scalar1=0.0,
                        scalar2=None, op0=Alu.max)
                    nc.vector.tensor_tensor(
                        out=stg[:, kq : kq + kb, :], in0=neg[:, :kb, :],
                        in1=pos[:, :kb, :], op=Alu.add)
                kq += kb
            nc.sync.dma_start(
                out=out[g, :, k0 * D : (k0 + kn) * D],
                in_=stg[:].rearrange("p k d -> p (k d)"))
    return out


# --------------------------------------------------------------------------
# Driver
# --------------------------------------------------------------------------
def _build_program(cfg: Config, meta):
    import concourse.bacc as bacc
    import concourse.tile as tile

    nc = bacc.Bacc("TRN2", target_bir_lowering=False, debug=False,
                   enable_asserts=False, num_devices=cfg.n_cores)
    with tile.TileContext(nc) as tc:
        build_kernel(nc, tc, cfg, meta)
    nc.compile()
    return nc


def _unscramble(results, plans, cfg: Config):
    n = cfg.n_nodes
    full = np.zeros((2, n, D), np.float32)
    for g in range(2):
        ct_all = plans[g]["core_tiles"]
        for core in range(cfg.n_cores):
            # [2, P, t_core*D] bf16
            oc = np.asarray(results[core]["out"]).astype(np.float32)
            for k in range(cfg.t_core):
                t = int(ct_all[core, k])
                if t < 0:
                    continue
                r0 = t * P
                r1 = min(r0 + P, n)
                full[g, r0:r1] = oc[g, : r1 - r0, k * D : (k + 1) * D]
    return full


_PROGRAM_CACHE = {}


def run(inputs, cfg: Config, trace=False):
    from concourse.bass_utils import run_bass_kernel_spmd

    in_maps, plans, meta = preprocess(
        inputs["feats"], inputs["W"], inputs["b"], inputs["prelu_a"],
        inputs["src_pos"], inputs["dst_pos"],
        inputs["src_neg"], inputs["dst_neg"], cfg)

    key = (cfg.n_nodes, cfg.n_cores, cfg.sg, cfg.sg_first, cfg.sg_tail,
           cfg.act_prelu, cfg.act_batch,
           cfg.oh_pool_mod, cfg.gbufs, cfg.maxc,
           meta["cstar"][0].tobytes(), meta["cstar"][1].tobytes(),
           meta["bank_sizes"].tobytes(), meta["use_bias"])
    nc = _PROGRAM_CACHE.get(key)
    if nc is None:
        nc = _build_program(cfg, meta)
        _PROGRAM_CACHE[key] = nc

    kwargs = {}
    if trace:
        kwargs = dict(trace=True, tmpdir=tempfile.mkdtemp(prefix="bgc_trace_"))
    res = run_bass_kernel_spmd(nc, in_maps, core_ids=list(range(cfg.n_cores)),
                               **kwargs)
    full = _unscramble(res.results, plans, cfg)
    return full, res


def kernel(**inputs) -> np.ndarray:
    cfg = Config()
    full, _ = run(inputs, cfg)
    return full


# revision 38
# speedup vs baseline: 2.0547x; 1.0023x over previous
"""Bass/Trainium2 kernel for BiGraphContrastLayer (GNN message passing).

Computes, for two edge lists (pos/neg) over the same node features:
    h_g = PReLU( D_in^-1/2 A_g D_out^-1/2 feats @ W + b )
returning stack([h_pos, h_neg]) of shape [2, N, Dout].

Strategy (8 NeuronCores, SPMD, no collectives). Uses linearity twice:
    nd_d * sum_e ns_s feats_s  =  sum_e (ns_s nd_d) feats_s
    (agg) @ W                  =  W applied once per dst tile after agg

  Single device pass: edges are bucketed by 128-node dst tile and sorted by
  src within (tile, row-bank); dma_gather (int16 idx over 4 row-bank views
  of one bf16 feats table) pulls feats[src] rows for ~128-edge chunks; a
  weighted one-hot matmul (lhsT = gathered chunk, rhs = onehot of dst offsets
  scaled by the per-edge weight ns_src*nd_dst) accumulates the TRANSPOSED
  aggregate aggT[din, dst] for each dst tile in PSUM; aggT -> bf16 SBUF
  (batched Act copy over 4 tiles), one matmul per tile applies W, and a
  batched PReLU produces the output tile.

  Host does index prep (degree bincounts -> per-edge norm weights, sorting,
  bucketing, dealing dst tiles to cores so all 8 cores share one instruction
  stream) and stages the bf16 feats table; all O(E*D) and O(N*D^2) work runs
  on device.
"""

import math
import tempfile
from dataclasses import dataclass

import numpy as np

P = 128   # partitions
D = 128   # feature dim (Din == Dout == 128)
NBANK = 4  # row-range banks (int16 gather idx addresses <=32768 rows)


# --------------------------------------------------------------------------
# Config
# --------------------------------------------------------------------------
@dataclass
class Config:
    n_nodes: int = 100000
    n_cores: int = 8
    sg: int = 14       # dst-tile positions per gather supergroup
    sg_first: int = 2  # size of the first supergroup (fast pipeline fill)
    sg_tail: int = 2   # max size of the last supergroup (short drain)
    act_prelu: bool = True    # final PReLU on ScalarE (not in sim)
    act_batch: bool = True    # batch aggT copy / prelu over 4 tiles
    oh_pool_mod: int = 0      # every Nth one-hot build goes to GpSimd (0=off)
    maxc: int = 48            # max chunks per dma_gather call
    repair_rounds: int = 3    # bank-repair passes
    repair_rmax: int = 56     # max remainder (mod 128) worth repairing
    gbufs: int = 3            # gather buffer count
    ipool_bufs: int = 3
    ohpool_bufs: int = 8

    @property
    def t_global(self) -> int:
        return math.ceil(self.n_nodes / P)

    @property
    def n_pad(self) -> int:
        return self.t_global * P

    @property
    def t_core(self) -> int:
        return math.ceil(self.t_global / self.n_cores)

    @property
    def bank_rows(self) -> int:
        return math.ceil(self.t_global / NBANK) * P


# --------------------------------------------------------------------------
# Host-side preprocessing
# --------------------------------------------------------------------------
def _norm(deg):
    deg = deg.astype(np.float64)
    return np.where(deg > 0, 1.0 / np.sqrt(np.maximum(deg, 1.0)), 0.0).astype(
        np.float32)


def _assign_banks(edge_lists, cfg: Config):
    """Assign nodes to gather banks (<=32768 rows each, int16 idx reach) to
    minimize per-(dst tile, bank) ceil-to-128 chunk fragmentation.

    Starts from contiguous row-range banks, then repairs buckets whose count
    sits just above a multiple of 128 by moving that remainder's nodes to
    other banks (each move's waste change across the node's other buckets is
    ~zero-sum, so erasing the remainder deletes a whole chunk).
    Deterministic. Returns (bank_of[N], table_row[N], bases, sizes).
    """
    n, tg = cfg.n_nodes, cfg.t_global
    srcs, tils = [], []
    for g, (src, dst) in enumerate(edge_lists):
        pair = np.unique(src * np.int64(1 << 32) + dst)
        srcs.append(pair >> 32)
        tils.append(g * tg + (pair & ((1 << 32) - 1)) // P)
    src_all = np.concatenate(srcs)
    til_all = np.concatenate(tils)
    o = np.argsort(src_all, kind="stable")
    sa, ta = src_all[o], til_all[o]
    nstarts = np.searchsorted(sa, np.arange(n + 1))
    o2 = np.argsort(til_all, kind="stable")
    tn_node = src_all[o2]
    tstarts = np.searchsorted(til_all[o2], np.arange(2 * tg + 1))

    cap = 32768
    span = min(cfg.bank_rows, cap)
    bank_of = np.minimum(np.arange(n) // span, NBANK - 1).astype(np.int8)
    counts = np.zeros((2 * tg, NBANK), np.int32)
    np.add.at(counts, (ta, bank_of[sa]), 1)
    fill = np.bincount(bank_of, minlength=NBANK).astype(np.int64)

    def move(node, b1):
        b0 = bank_of[node]
        t_n = ta[nstarts[node]:nstarts[node + 1]]
        counts[t_n, b0] -= 1
        counts[t_n, b1] += 1
        bank_of[node] = b1
        fill[b0] -= 1
        fill[b1] += 1

    def marginal_all(node):
        b0 = bank_of[node]
        t_n = ta[nstarts[node]:nstarts[node + 1]]
        cb = counts[t_n]
        rem = np.where(cb[:, b0] % 128 == 1, -127, 1).sum()
        d = np.where(cb % 128 == 0, 127, -1).sum(0) + rem
        d[b0] = 1 << 30
        d[fill >= cap] = 1 << 30
        return d

    for _ in range(cfg.repair_rounds):
        r = counts % 128
        cand = np.argwhere((r > 0) & (r <= cfg.repair_rmax) & (counts > 128))
        if len(cand) == 0:
            break
        for (t, b) in cand[np.argsort(r[cand[:, 0], cand[:, 1]], kind="stable")]:
            rr = int(counts[t, b] % 128)
            if rr == 0 or rr > cfg.repair_rmax or counts[t, b] <= 128:
                continue
            nodes = tn_node[tstarts[t]:tstarts[t + 1]]
            nodes = nodes[bank_of[nodes] == b]
            if len(nodes) < rr:
                continue
            ranked = sorted((int(marginal_all(nd).min()), int(nd))
                            for nd in nodes)[:rr]
            applied, tot = [], 0
            for (_, nd) in ranked:
                d = marginal_all(nd)
                b1 = int(np.argmin(d))
                tot += int(d[b1])
                move(nd, b1)
                applied.append(nd)
            if tot >= 0:
                for nd in reversed(applied):
                    move(nd, b)

    order = np.argsort(bank_of, kind="stable")
    table_row = np.empty(n, np.int64)
    table_row[order] = np.arange(n)
    sizes = np.bincount(bank_of, minlength=NBANK).astype(np.int64)
    bases = np.zeros(NBANK + 1, np.int64)
    np.cumsum(sizes, out=bases[1:])
    return bank_of, table_row, bases[:NBANK], sizes


def _plan_graph(src, dst, wgt, bank_of, table_row, bases, cfg: Config):
    """Bucket edges by dst tile, split by src bank (host-assigned), sort by
    table row within (tile, bank), and deal tiles to cores with signature
    matching.

    Returns dict with:
      core_tiles  [n_cores, t_core]  global tile id per position (-1 null)
      cstar       [t_core, NBANK]    shared chunk counts per position/bank
      tile_edges  list per global tile: (idx16, off, wgt, bank_cnt)
    """
    tg, ncores, tcore = cfg.t_global, cfg.n_cores, cfg.t_core
    order = np.argsort(dst, kind="stable")
    src_s = src[order]
    dst_s = dst[order]
    wgt_s = wgt[order]
    tile_cnt = np.bincount(dst_s // P, minlength=tg)
    starts = np.zeros(tg + 1, np.int64)
    np.cumsum(tile_cnt, out=starts[1:])

    sig = np.zeros((tg, NBANK), np.int64)
    tile_edges = []
    for t in range(tg):
        e0, e1 = int(starts[t]), int(starts[t + 1])
        ts_src = src_s[e0:e1]
        bank = bank_of[ts_src].astype(np.int64)
        row = table_row[ts_src]
        so = np.argsort(bank * (2 ** 40) + row, kind="stable")
        ts_idx = (row - bases[bank])[so]
        assert len(ts_idx) == 0 or (0 <= ts_idx.min() and ts_idx.max() < 32768)
        ts_off = (dst_s[e0:e1][so] % P).astype(np.float32)
        ts_wgt = wgt_s[e0:e1][so]
        bc = np.bincount(bank[so], minlength=NBANK)
        sig[t] = -(-bc // P)  # ceil chunks per bank
        tile_edges.append((ts_idx, ts_off, ts_wgt, bc))

    # Deal: sort tiles by signature so consecutive groups of n_cores tiles
    # have matching/near-matching per-bank chunk counts.
    sigkey = sig @ (np.int64(32) ** np.arange(NBANK - 1, -1, -1))
    keys = np.argsort(sigkey, kind="stable")
    n_slots = ncores * tcore
    dealt = np.full(n_slots, -1, np.int64)
    dealt[: len(keys)] = keys[::-1]  # descending signature order
    core_tiles = np.zeros((ncores, tcore), np.int64)
    cstar = np.zeros((tcore, NBANK), np.int64)
    for k in range(tcore):
        grp = dealt[k * ncores : (k + 1) * ncores]
        for c in range(ncores):
            core_tiles[c, k] = grp[c]
        s = np.zeros(NBANK, np.int64)
        for t in grp:
            if t >= 0:
                s = np.maximum(s, sig[t])
        if s.sum() == 0:
            s[0] = 1  # every position needs >=1 chunk (PSUM init matmul)
        cstar[k] = s
    return dict(core_tiles=core_tiles, cstar=cstar, tile_edges=tile_edges)


def _slot_layout(cstar, cfg: Config):
    """Shared (all-cores) slot layout for one graph.

    Slot space = sequence of supergroups; within a supergroup, bank-major:
      for b in banks: for k in sg positions: cstar[k, b] chunks.
    Returns:
      sg_list: list of (k0, kn)
      calls:   list of (sg_idx, bank, chunk0, nchunks)  [nchunks > 0]
      chunk_of: dict (k, b, c) -> global chunk index
      n_chunks total
    """
    tcore, sg = cfg.t_core, cfg.sg
    sizes = []
    rem = tcore
    if cfg.sg_first and rem > cfg.sg_first:
        sizes.append(cfg.sg_first)
        rem -= cfg.sg_first
    while rem > 0:
        kn = min(sg, rem)
        sizes.append(kn)
        rem -= kn
    # split an oversized final group so the drain after the last gather is
    # short
    if cfg.sg_tail and len(sizes) > 1 and sizes[-1] > cfg.sg_tail:
        last = sizes.pop()
        h = last - cfg.sg_tail
        sizes.extend([h, cfg.sg_tail])
    sg_list = []
    k0 = 0
    for kn in sizes:
        sg_list.append((k0, kn))
        k0 += kn
    calls = []
    chunk_of = {}
    cpos = 0
    for si, (k0, kn) in enumerate(sg_list):
        for b in range(NBANK):
            c0 = cpos
            for ki in range(kn):
                k = k0 + ki
                for c in range(int(cstar[k, b])):
                    chunk_of[(k, b, c)] = cpos
                    cpos += 1
            while c0 < cpos:
                nch = min(cfg.maxc, cpos - c0)
                calls.append((si, b, c0, nch))
                c0 += nch
    return sg_list, calls, chunk_of, cpos


def _fill_core_graph(plan, layout, core, cfg: Config):
    """Build IDX16 (wrapped), OFF, and WGT arrays for one core, one graph."""
    sg_list, calls, chunk_of, n_chunks = layout
    cstar = plan["cstar"]
    idx = np.zeros((n_chunks, P), np.int16)
    off = np.full((n_chunks, P), 512.0, np.float32)
    wgt = np.zeros((n_chunks, P), np.float32)
    for k in range(cfg.t_core):
        t = plan["core_tiles"][core, k]
        if t < 0:
            continue
        ts_idx, ts_off, ts_wgt, bc = plan["tile_edges"][t]
        bstart = np.zeros(NBANK + 1, np.int64)
        np.cumsum(bc, out=bstart[1:])
        for b in range(NBANK):
            nb = int(bc[b])
            cnum = int(cstar[k, b])
            if cnum == 0:
                continue
            nslot = cnum * P
            es = np.zeros(nslot, np.int64)
            eo = np.full(nslot, 512.0, np.float32)
            ew = np.zeros(nslot, np.float32)
            es[:nb] = ts_idx[bstart[b] : bstart[b + 1]]
            eo[:nb] = ts_off[bstart[b] : bstart[b + 1]]
            ew[:nb] = ts_wgt[bstart[b] : bstart[b + 1]]
            for c in range(cnum):
                ci = chunk_of[(k, b, c)]
                idx[ci] = es[c * P : (c + 1) * P]
                off[ci] = eo[c * P : (c + 1) * P]
                wgt[ci] = ew[c * P : (c + 1) * P]
    # wrap: flat slot i (within a call's range) -> [i%16, i//16], replicated
    # to 128 partitions. Build per call, concatenated along columns.
    ncols = n_chunks * P // 16
    idx_w = np.zeros((P, ncols), np.int16)
    for (si, b, c0, nch) in calls:
        flat = idx[c0 : c0 + nch].reshape(-1)  # ni slots
        blk = flat.reshape(-1, 16).T  # [16, ni/16]
        idx_w[:, c0 * 8 : (c0 + nch) * 8] = np.tile(blk, (8, 1))
    return idx_w, off.T.copy(), wgt.T.copy()  # -> [P, n_chunks] f32


def preprocess(feats, W, b, prelu_a, src_pos, dst_pos, src_neg, dst_neg,
               cfg: Config):
    n = cfg.n_nodes
    feats = np.asarray(feats, np.float32)
    W = np.asarray(W, np.float32)
    b = np.asarray(b, np.float32)
    prelu_a = np.asarray(prelu_a, np.float32)
    bf16 = np.dtype("bfloat16")

    edge_lists = []
    wgts = []
    for src, dst in ((src_pos, dst_pos), (src_neg, dst_neg)):
        src = np.asarray(src, np.int64)
        dst = np.asarray(dst, np.int64)
        ns = _norm(np.bincount(src, minlength=n))
        nd = _norm(np.bincount(dst, minlength=n))
        # merge duplicate (src, dst) edges: by linearity their contributions
        # sum, so one gathered row with a summed weight is exact.
        pair = src * np.int64(1 << 32) + dst
        upair, ucnt = np.unique(pair, return_counts=True)
        usrc = (upair >> 32).astype(np.int64)
        udst = (upair & ((1 << 32) - 1)).astype(np.int64)
        edge_lists.append((usrc, udst))
        wgts.append((ucnt * ns[usrc] * nd[udst]).astype(np.float32))

    bank_of, table_row, bases, sizes = _assign_banks(edge_lists, cfg)

    feats_pad = np.zeros((cfg.n_pad, D), np.float32)
    feats_pad[table_row] = feats[:n]
    fb16 = feats_pad.astype(bf16)

    plans, layouts = [], []
    for (usrc, udst), wgt in zip(edge_lists, wgts):
        plan = _plan_graph(usrc, udst, wgt, bank_of, table_row, bases, cfg)
        plans.append(plan)
        layouts.append(_slot_layout(plan["cstar"], cfg))

    iota = np.tile(np.arange(P, dtype=np.float32), (P, 1)).astype(bf16)
    a_rep = np.full((P, 1), float(prelu_a.reshape(-1)[0]), np.float32)
    b_rep = np.tile(b.reshape(1, D), (P, 1)).astype(np.float32)
    use_bias = bool(np.any(b != 0.0))

    in_maps = []
    for core in range(cfg.n_cores):
        iw_p, off_p, wgt_p = _fill_core_graph(plans[0], layouts[0], core, cfg)
        iw_n, off_n, wgt_n = _fill_core_graph(plans[1], layouts[1], core, cfg)
        m = {
            "fb16": fb16,
            "w_in": W,
            "a_rep": a_rep,
            "idx_in": np.concatenate([iw_p, iw_n], axis=1),
            "off_in": np.concatenate([off_p, off_n], axis=1),
            "wgt_in": np.concatenate([wgt_p, wgt_n], axis=1),
            "iota_in": iota,
        }
        if use_bias:
            m["b_rep"] = b_rep
        in_maps.append(m)
    meta = {
        "layouts": layouts,
        "cstar": [plans[0]["cstar"], plans[1]["cstar"]],
        "use_bias": use_bias,
        "bank_bases": bases,
        "bank_sizes": sizes,
    }
    return in_maps, plans, meta


# --------------------------------------------------------------------------
# Device kernel builder
# --------------------------------------------------------------------------
def build_kernel(nc, tc, cfg: Config, meta):
    from contextlib import ExitStack

    import concourse.mybir as mybir

    f32 = mybir.dt.float32
    bf16 = mybir.dt.bfloat16
    i16 = mybir.dt.int16
    Alu = mybir.AluOpType
    Act = mybir.ActivationFunctionType

    tcore, npad = cfg.t_core, cfg.n_pad
    layouts = meta["layouts"]
    cstar = meta["cstar"]
    use_bias = meta["use_bias"]
    n_chunks = [layouts[g][3] for g in range(2)]
    ncols = [n_chunks[g] * P // 16 for g in range(2)]

    fb = nc.dram_tensor("fb16", [cfg.n_pad, D], bf16,
                        kind="ExternalInput").ap()
    w_in = nc.dram_tensor("w_in", [P, D], f32, kind="ExternalInput").ap()
    a_rep = nc.dram_tensor("a_rep", [P, 1], f32, kind="ExternalInput").ap()
    idx_in = nc.dram_tensor("idx_in", [P, sum(ncols)], i16,
                            kind="ExternalInput").ap()
    off_in = nc.dram_tensor("off_in", [P, sum(n_chunks)], f32,
                            kind="ExternalInput").ap()
    wgt_in = nc.dram_tensor("wgt_in", [P, sum(n_chunks)], f32,
                            kind="ExternalInput").ap()
    iota_in = nc.dram_tensor("iota_in", [P, P], bf16, kind="ExternalInput").ap()
    if use_bias:
        b_rep = nc.dram_tensor("b_rep", [P, D], f32, kind="ExternalInput").ap()
    out = nc.dram_tensor("out", [2, P, tcore * D], bf16,
                         kind="ExternalOutput").ap()

    with ExitStack() as ctx:
        const = ctx.enter_context(tc.tile_pool(name="const", bufs=1))
        gpool = ctx.enter_context(tc.tile_pool(name="gpool", bufs=cfg.gbufs))
        ipool = ctx.enter_context(tc.tile_pool(name="ipool", bufs=cfg.ipool_bufs))
        ohpool = ctx.enter_context(tc.tile_pool(name="ohpool",
                                                bufs=cfg.ohpool_bufs))
        atpool = ctx.enter_context(tc.tile_pool(name="atpool", bufs=3))
        tpool = ctx.enter_context(tc.tile_pool(name="tpool", bufs=4))
        spool = ctx.enter_context(tc.tile_pool(name="spool", bufs=3))
        apool = ctx.enter_context(tc.tile_pool(name="apool", bufs=3,
                                               space="PSUM"))
        hpool = ctx.enter_context(tc.tile_pool(name="hpool", bufs=3,
                                               space="PSUM"))

        # ---- constants ----
        w_sb = const.tile([P, D], bf16)
        nc.gpsimd.dma_start(out=w_sb[:], in_=w_in)  # f32 -> bf16 cast DMA
        iota_sb = const.tile([P, P], bf16)
        nc.sync.dma_start(out=iota_sb[:], in_=iota_in)
        a_sb = const.tile([P, 1], f32)
        nc.sync.dma_start(out=a_sb[:], in_=a_rep)
        if use_bias:
            b_sb = const.tile([P, D], f32)
            nc.sync.dma_start(out=b_sb[:], in_=b_rep)

        # ---- gather + weighted one-hot segment-sum + W + prelu ----
        col_base = [0, ncols[0]]          # idx column offset per graph
        chk_base = [0, n_chunks[0]]       # off/wgt column offset per graph
        cbs_all = []
        for g in range(2):
            calls_by_sg = {}
            for (si, b, c0, nch) in layouts[g][1]:
                calls_by_sg.setdefault(si, []).append((b, c0, nch))
            cbs_all.append(calls_by_sg)
        # interleave the two graphs' supergroups so one graph's gathers fill
        # DMA while the other's PSUM chain drains
        jobs = []
        for si in range(max(len(layouts[0][0]), len(layouts[1][0]))):
            for g in range(2):
                if si < len(layouts[g][0]):
                    jobs.append((g, si))
        oh_ct = 0
        for (g, si) in jobs:
            sg_list, calls, chunk_of, _ = layouts[g]
            cs = cstar[g]
            (k0, kn) = sg_list[si]
            sg_chunks = sum(int(cs[k0 + ki, b]) for ki in range(kn)
                            for b in range(NBANK))
            c0_sg = chunk_of[(k0, 0, 0)]
            gt = gpool.tile([P, sg_chunks, D], bf16, tag="gather")
            it = ipool.tile([P, sg_chunks * 8], i16, tag="gidx")
            nc.sync.dma_start(
                out=it[:],
                in_=idx_in[:, col_base[g] + c0_sg * 8 :
                           col_base[g] + (c0_sg + sg_chunks) * 8])
            ot = ipool.tile([P, sg_chunks], f32, tag="goff")
            nc.sync.dma_start(
                out=ot[:],
                in_=off_in[:, chk_base[g] + c0_sg :
                           chk_base[g] + c0_sg + sg_chunks])
            wt = ipool.tile([P, sg_chunks], f32, tag="gwgt")
            nc.sync.dma_start(
                out=wt[:],
                in_=wgt_in[:, chk_base[g] + c0_sg :
                           chk_base[g] + c0_sg + sg_chunks])
            bases = meta["bank_bases"]
            sizes = meta["bank_sizes"]
            for (b, c0, nch) in cbs_all[g][si]:
                lo = c0 - c0_sg
                nc.gpsimd.dma_gather(
                    out_ap=gt[:, lo : lo + nch, :],
                    in_ap=fb[int(bases[b]) : int(bases[b] + sizes[b]), :],
                    idxs_ap=it[:, lo * 8 : (lo + nch) * 8],
                    num_idxs=nch * P, num_idxs_reg=nch * P,
                    elem_size=D, single_packet=False)
            stg = spool.tile([P, kn, D], bf16, tag="stg")
            nb4 = 4 if cfg.act_batch else 1
            kq = 0
            while kq < kn:
                kb = min(nb4, kn - kq)
                agg4 = apool.tile([P, nb4, D], f32)
                for j in range(kb):
                    k = k0 + kq + j
                    nonzero = [(b, c) for b in range(NBANK)
                               for c in range(int(cs[k, b]))]
                    for ji, (b, c) in enumerate(nonzero):
                        ci = chunk_of[(k, b, c)]
                        lo = ci - c0_sg
                        oh = ohpool.tile([P, P], bf16)
                        eng = nc.vector
                        if cfg.oh_pool_mod:
                            oh_ct += 1
                            if oh_ct % cfg.oh_pool_mod == 0:
                                eng = nc.gpsimd
                        eng.tensor_scalar(
                            out=oh[:], in0=iota_sb[:],
                            scalar1=ot[:, lo : lo + 1],
                            scalar2=wt[:, lo : lo + 1],
                            op0=Alu.is_equal, op1=Alu.mult)
                        nc.tensor.matmul(
                            out=agg4[:, j, :], lhsT=gt[:, lo, :], rhs=oh[:],
                            start=(ji == 0), stop=(ji == len(nonzero) - 1))
                at4 = atpool.tile([P, nb4, D], bf16, tag="at4")
                nc.scalar.activation(out=at4[:, :kb, :], in_=agg4[:, :kb, :],
                                     func=Act.Copy)
                h4 = hpool.tile([P, nb4, D], f32)
                for j in range(kb):
                    nc.tensor.matmul(out=h4[:, j, :], lhsT=at4[:, j, :],
                                     rhs=w_sb[:], start=True, stop=True)
                if cfg.act_prelu and not use_bias:
                    nc.scalar.activation(
                        out=stg[:, kq : kq + kb, :], in_=h4[:, :kb, :],
                        func=Act.Prelu, alpha=a_sb[:, :1])
                elif not use_bias:
                    neg = tpool.tile([P, nb4, D], f32, tag="neg")
                    nc.vector.tensor_scalar(
                        out=neg[:, :kb, :], in0=h4[:, :kb, :], scalar1=0.0,
                        scalar2=a_sb[:, :1], op0=Alu.min, op1=Alu.mult)
                    pos = tpool.tile([P, nb4, D], f32, tag="pos")
                    nc.vector.tensor_scalar(
                        out=pos[:, :kb, :], in0=h4[:, :kb, :], scalar1=0.0,
                        scalar2=None, op0=Alu.max)
                    nc.vector.tensor_tensor(
                        out=stg[:, kq : kq + kb, :], in0=neg[:, :kb, :],
                        in1=pos[:, :kb, :], op=Alu.add)
                else:
                    hb = tpool.tile([P, nb4, D], f32, tag="hb")
                    nc.vector.tensor_tensor(
                        out=hb[:, :kb, :], in0=h4[:, :kb, :],
                        in1=b_sb[:, None, :].to_broadcast([P, kb, D]),
                        op=Alu.add)
                    neg = tpool.tile([P, nb4, D], f32, tag="neg")
                    nc.vector.tensor_scalar(
                        out=neg[:, :kb, :], in0=hb[:, :kb, :], scalar1=0.0,
                        scalar2=a_sb[:, :1], op0=Alu.min, op1=Alu.mult)
                    pos = tpool.tile([P, nb4, D], f32, tag="pos")
                    nc.vector.tensor_scalar(
                        out=pos[:, :kb, :], in0=hb[:, :kb, :], scalar1=0.0,
                        scalar2=None, op0=Alu.max)
                    nc.vector.tensor_tensor(
                        out=stg[:, kq : kq + kb, :], in0=neg[:, :kb, :],
                        in1=pos[:, :kb, :], op=Alu.add)
                kq += kb
            nc.sync.dma_start(
                out=out[g, :, k0 * D : (k0 + kn) * D],
                in_=stg[:].rearrange("p k d -> p (k d)"))
    return out


# --------------------------------------------------------------------------
# Driver
# --------------------------------------------------------------------------
def _build_program(cfg: Config, meta):
    import concourse.bacc as bacc
    import concourse.tile as tile

    nc = bacc.Bacc("TRN2", target_bir_lowering=False, debug=False,
                   enable_asserts=False, num_devices=cfg.n_cores)
    with tile.TileContext(nc) as tc:
        build_kernel(nc, tc, cfg, meta)
    nc.compile()
    return nc


def _unscramble(results, plans, cfg: Config):
    n = cfg.n_nodes
    full = np.zeros((2, n, D), np.float32)
    for g in range(2):
        ct_all = plans[g]["core_tiles"]
        for core in range(cfg.n_cores):
            # [2, P, t_core*D] bf16
            oc = np.asarray(results[core]["out"]).astype(np.float32)
            for k in range(cfg.t_core):
                t = int(ct_all[core, k])
                if t < 0:
                    continue
                r0 = t * P
                r1 = min(r0 + P, n)
                full[g, r0:r1] = oc[g, : r1 - r0, k * D : (k + 1) * D]
    return full


_PROGRAM_CACHE = {}


def run(inputs, cfg: Config, trace=False):
    from concourse.bass_utils import run_bass_kernel_spmd

    in_maps, plans, meta = preprocess(
        inputs["feats"], inputs["W"], inputs["b"], inputs["prelu_a"],
        inputs["src_pos"], inputs["dst_pos"],
        inputs["src_neg"], inputs["dst_neg"], cfg)

    key = (cfg.n_nodes, cfg.n_cores, cfg.sg, cfg.sg_first, cfg.sg_tail,
           cfg.act_prelu, cfg.act_batch,
           cfg.oh_pool_mod, cfg.gbufs, cfg.maxc,
           meta["cstar"][0].tobytes(), meta["cstar"][1].tobytes(),
           meta["bank_sizes"].tobytes(), meta["use_bias"])
    nc = _PROGRAM_CACHE.get(key)
    if nc is None:
        nc = _build_program(cfg, meta)
        _PROGRAM_CACHE[key] = nc

    kwargs = {}
    if trace:
        kwargs = dict(trace=True, tmpdir=tempfile.mkdtemp(prefix="bgc_trace_"))
    res = run_bass_kernel_spmd(nc, in_maps, core_ids=list(range(cfg.n_cores)),
                               **kwargs)
    full = _unscramble(res.results, plans, cfg)
    return full, res


def kernel(**inputs) -> np.ndarray:
    cfg = Config()
    full, _ = run(inputs, cfg)
    return full


# revision 39
# speedup vs baseline: 2.0793x; 1.0120x over previous
"""Bass/Trainium2 kernel for BiGraphContrastLayer (GNN message passing).

Computes, for two edge lists (pos/neg) over the same node features:
    h_g = PReLU( D_in^-1/2 A_g D_out^-1/2 feats @ W + b )
returning stack([h_pos, h_neg]) of shape [2, N, Dout].

Strategy (8 NeuronCores, SPMD, no collectives). Uses linearity twice:
    nd_d * sum_e ns_s feats_s  =  sum_e (ns_s nd_d) feats_s
    (agg) @ W                  =  W applied once per dst tile after agg

  Single device pass: edges are bucketed by 128-node dst tile and sorted by
  src within (tile, row-bank); dma_gather (int16 idx over 4 row-bank views
  of one bf16 feats table) pulls feats[src] rows for ~128-edge chunks; a
  weighted one-hot matmul (lhsT = gathered chunk, rhs = onehot of dst offsets
  scaled by the per-edge weight ns_src*nd_dst) accumulates the TRANSPOSED
  aggregate aggT[din, dst] for each dst tile in PSUM; aggT -> bf16 SBUF
  (batched Act copy over 4 tiles), one matmul per tile applies W, and a
  batched PReLU produces the output tile.

  Host does index prep (degree bincounts -> per-edge norm weights, sorting,
  bucketing, dealing dst tiles to cores so all 8 cores share one instruction
  stream) and stages the bf16 feats table; all O(E*D) and O(N*D^2) work runs
  on device.
"""

import math
import tempfile
from dataclasses import dataclass

import numpy as np

P = 128   # partitions
D = 128   # feature dim (Din == Dout == 128)
NBANK = 4  # row-range banks (int16 gather idx addresses <=32768 rows)


# --------------------------------------------------------------------------
# Config
# --------------------------------------------------------------------------
@dataclass
class Config:
    n_nodes: int = 100000
    n_cores: int = 8
    sg: int = 14       # dst-tile positions per gather supergroup
    sg_first: int = 4  # size of the first supergroup (fast pipeline fill)
    sg_tail: int = 4   # max size of the last supergroup (short drain)
    act_prelu: bool = True    # final PReLU on ScalarE (not in sim)
    act_batch: bool = True    # batch aggT copy / prelu over 4 tiles
    oh_pool_mod: int = 0      # every Nth one-hot build goes to GpSimd (0=off)
    maxc: int = 48            # max chunks per dma_gather call
    repair_rounds: int = 3    # bank-repair passes
    repair_rmax: int = 56     # max remainder (mod 128) worth repairing
    gbufs: int = 3            # gather buffer count
    ipool_bufs: int = 3
    ohpool_bufs: int = 8

    @property
    def t_global(self) -> int:
        return math.ceil(self.n_nodes / P)

    @property
    def n_pad(self) -> int:
        return self.t_global * P

    @property
    def t_core(self) -> int:
        return math.ceil(self.t_global / self.n_cores)

    @property
    def bank_rows(self) -> int:
        return math.ceil(self.t_global / NBANK) * P


# --------------------------------------------------------------------------
# Host-side preprocessing
# --------------------------------------------------------------------------
def _norm(deg):
    deg = deg.astype(np.float64)
    return np.where(deg > 0, 1.0 / np.sqrt(np.maximum(deg, 1.0)), 0.0).astype(
        np.float32)


def _assign_banks(edge_lists, cfg: Config):
    """Assign nodes to gather banks (<=32768 rows each, int16 idx reach) to
    minimize per-(dst tile, bank) ceil-to-128 chunk fragmentation.

    Starts from contiguous row-range banks, then repairs buckets whose count
    sits just above a multiple of 128 by moving that remainder's nodes to
    other banks (each move's waste change across the node's other buckets is
    ~zero-sum, so erasing the remainder deletes a whole chunk).
    Deterministic. Returns (bank_of[N], table_row[N], bases, sizes).
    """
    n, tg = cfg.n_nodes, cfg.t_global
    srcs, tils = [], []
    for g, (src, dst) in enumerate(edge_lists):
        pair = np.unique(src * np.int64(1 << 32) + dst)
        srcs.append(pair >> 32)
        tils.append(g * tg + (pair & ((1 << 32) - 1)) // P)
    src_all = np.concatenate(srcs)
    til_all = np.concatenate(tils)
    o = np.argsort(src_all, kind="stable")
    sa, ta = src_all[o], til_all[o]
    nstarts = np.searchsorted(sa, np.arange(n + 1))
    o2 = np.argsort(til_all, kind="stable")
    tn_node = src_all[o2]
    tstarts = np.searchsorted(til_all[o2], np.arange(2 * tg + 1))

    cap = 32768
    span = min(cfg.bank_rows, cap)
    bank_of = np.minimum(np.arange(n) // span, NBANK - 1).astype(np.int8)
    counts = np.zeros((2 * tg, NBANK), np.int32)
    np.add.at(counts, (ta, bank_of[sa]), 1)
    fill = np.bincount(bank_of, minlength=NBANK).astype(np.int64)

    def move(node, b1):
        b0 = bank_of[node]
        t_n = ta[nstarts[node]:nstarts[node + 1]]
        counts[t_n, b0] -= 1
        counts[t_n, b1] += 1
        bank_of[node] = b1
        fill[b0] -= 1
        fill[b1] += 1

    def marginal_all(node):
        b0 = bank_of[node]
        t_n = ta[nstarts[node]:nstarts[node + 1]]
        cb = counts[t_n]
        rem = np.where(cb[:, b0] % 128 == 1, -127, 1).sum()
        d = np.where(cb % 128 == 0, 127, -1).sum(0) + rem
        d[b0] = 1 << 30
        d[fill >= cap] = 1 << 30
        return d

    for _ in range(cfg.repair_rounds):
        r = counts % 128
        cand = np.argwhere((r > 0) & (r <= cfg.repair_rmax) & (counts > 128))
        if len(cand) == 0:
            break
        for (t, b) in cand[np.argsort(r[cand[:, 0], cand[:, 1]], kind="stable")]:
            rr = int(counts[t, b] % 128)
            if rr == 0 or rr > cfg.repair_rmax or counts[t, b] <= 128:
                continue
            nodes = tn_node[tstarts[t]:tstarts[t + 1]]
            nodes = nodes[bank_of[nodes] == b]
            if len(nodes) < rr:
                continue
            ranked = sorted((int(marginal_all(nd).min()), int(nd))
                            for nd in nodes)[:rr]
            applied, tot = [], 0
            for (_, nd) in ranked:
                d = marginal_all(nd)
                b1 = int(np.argmin(d))
                tot += int(d[b1])
                move(nd, b1)
                applied.append(nd)
            if tot >= 0:
                for nd in reversed(applied):
                    move(nd, b)

    order = np.argsort(bank_of, kind="stable")
    table_row = np.empty(n, np.int64)
    table_row[order] = np.arange(n)
    sizes = np.bincount(bank_of, minlength=NBANK).astype(np.int64)
    bases = np.zeros(NBANK + 1, np.int64)
    np.cumsum(sizes, out=bases[1:])
    return bank_of, table_row, bases[:NBANK], sizes


def _plan_graph(src, dst, wgt, bank_of, table_row, bases, cfg: Config):
    """Bucket edges by dst tile, split by src bank (host-assigned), sort by
    table row within (tile, bank), and deal tiles to cores with signature
    matching.

    Returns dict with:
      core_tiles  [n_cores, t_core]  global tile id per position (-1 null)
      cstar       [t_core, NBANK]    shared chunk counts per position/bank
      tile_edges  list per global tile: (idx16, off, wgt, bank_cnt)
    """
    tg, ncores, tcore = cfg.t_global, cfg.n_cores, cfg.t_core
    order = np.argsort(dst, kind="stable")
    src_s = src[order]
    dst_s = dst[order]
    wgt_s = wgt[order]
    tile_cnt = np.bincount(dst_s // P, minlength=tg)
    starts = np.zeros(tg + 1, np.int64)
    np.cumsum(tile_cnt, out=starts[1:])

    sig = np.zeros((tg, NBANK), np.int64)
    tile_edges = []
    for t in range(tg):
        e0, e1 = int(starts[t]), int(starts[t + 1])
        ts_src = src_s[e0:e1]
        bank = bank_of[ts_src].astype(np.int64)
        row = table_row[ts_src]
        so = np.argsort(bank * (2 ** 40) + row, kind="stable")
        ts_idx = (row - bases[bank])[so]
        assert len(ts_idx) == 0 or (0 <= ts_idx.min() and ts_idx.max() < 32768)
        ts_off = (dst_s[e0:e1][so] % P).astype(np.float32)
        ts_wgt = wgt_s[e0:e1][so]
        bc = np.bincount(bank[so], minlength=NBANK)
        sig[t] = -(-bc // P)  # ceil chunks per bank
        tile_edges.append((ts_idx, ts_off, ts_wgt, bc))

    # Deal: sort tiles by signature so consecutive groups of n_cores tiles
    # have matching/near-matching per-bank chunk counts.
    sigkey = sig @ (np.int64(32) ** np.arange(NBANK - 1, -1, -1))
    keys = np.argsort(sigkey, kind="stable")
    n_slots = ncores * tcore
    dealt = np.full(n_slots, -1, np.int64)
    dealt[: len(keys)] = keys[::-1]  # descending signature order
    core_tiles = np.zeros((ncores, tcore), np.int64)
    cstar = np.zeros((tcore, NBANK), np.int64)
    for k in range(tcore):
        grp = dealt[k * ncores : (k + 1) * ncores]
        for c in range(ncores):
            core_tiles[c, k] = grp[c]
        s = np.zeros(NBANK, np.int64)
        for t in grp:
            if t >= 0:
                s = np.maximum(s, sig[t])
        if s.sum() == 0:
            s[0] = 1  # every position needs >=1 chunk (PSUM init matmul)
        cstar[k] = s
    return dict(core_tiles=core_tiles, cstar=cstar, tile_edges=tile_edges)


def _slot_layout(cstar, cfg: Config):
    """Shared (all-cores) slot layout for one graph.

    Slot space = sequence of supergroups; within a supergroup, bank-major:
      for b in banks: for k in sg positions: cstar[k, b] chunks.
    Returns:
      sg_list: list of (k0, kn)
      calls:   list of (sg_idx, bank, chunk0, nchunks)  [nchunks > 0]
      chunk_of: dict (k, b, c) -> global chunk index
      n_chunks total
    """
    tcore, sg = cfg.t_core, cfg.sg
    sizes = []
    rem = tcore
    if cfg.sg_first and rem > cfg.sg_first:
        sizes.append(cfg.sg_first)
        rem -= cfg.sg_first
    while rem > 0:
        kn = min(sg, rem)
        sizes.append(kn)
        rem -= kn
    # split an oversized final group so the drain after the last gather is
    # short
    if cfg.sg_tail and len(sizes) > 1 and sizes[-1] > cfg.sg_tail:
        last = sizes.pop()
        h = last - cfg.sg_tail
        sizes.extend([h, cfg.sg_tail])
    sg_list = []
    k0 = 0
    for kn in sizes:
        sg_list.append((k0, kn))
        k0 += kn
    calls = []
    chunk_of = {}
    cpos = 0
    for si, (k0, kn) in enumerate(sg_list):
        for b in range(NBANK):
            c0 = cpos
            for ki in range(kn):
                k = k0 + ki
                for c in range(int(cstar[k, b])):
                    chunk_of[(k, b, c)] = cpos
                    cpos += 1
            while c0 < cpos:
                nch = min(cfg.maxc, cpos - c0)
                calls.append((si, b, c0, nch))
                c0 += nch
    return sg_list, calls, chunk_of, cpos


def _fill_core_graph(plan, layout, core, cfg: Config):
    """Build IDX16 (wrapped), OFF, and WGT arrays for one core, one graph."""
    sg_list, calls, chunk_of, n_chunks = layout
    cstar = plan["cstar"]
    idx = np.zeros((n_chunks, P), np.int16)
    off = np.full((n_chunks, P), 512.0, np.float32)
    wgt = np.zeros((n_chunks, P), np.float32)
    for k in range(cfg.t_core):
        t = plan["core_tiles"][core, k]
        if t < 0:
            continue
        ts_idx, ts_off, ts_wgt, bc = plan["tile_edges"][t]
        bstart = np.zeros(NBANK + 1, np.int64)
        np.cumsum(bc, out=bstart[1:])
        for b in range(NBANK):
            nb = int(bc[b])
            cnum = int(cstar[k, b])
            if cnum == 0:
                continue
            nslot = cnum * P
            es = np.zeros(nslot, np.int64)
            eo = np.full(nslot, 512.0, np.float32)
            ew = np.zeros(nslot, np.float32)
            es[:nb] = ts_idx[bstart[b] : bstart[b + 1]]
            eo[:nb] = ts_off[bstart[b] : bstart[b + 1]]
            ew[:nb] = ts_wgt[bstart[b] : bstart[b + 1]]
            for c in range(cnum):
                ci = chunk_of[(k, b, c)]
                idx[ci] = es[c * P : (c + 1) * P]
                off[ci] = eo[c * P : (c + 1) * P]
                wgt[ci] = ew[c * P : (c + 1) * P]
    # wrap: flat slot i (within a call's range) -> [i%16, i//16], replicated
    # to 128 partitions. Build per call, concatenated along columns.
    ncols = n_chunks * P // 16
    idx_w = np.zeros((P, ncols), np.int16)
    for (si, b, c0, nch) in calls:
        flat = idx[c0 : c0 + nch].reshape(-1)  # ni slots
        blk = flat.reshape(-1, 16).T  # [16, ni/16]
        idx_w[:, c0 * 8 : (c0 + nch) * 8] = np.tile(blk, (8, 1))
    return idx_w, off.T.copy(), wgt.T.copy()  # -> [P, n_chunks] f32


def preprocess(feats, W, b, prelu_a, src_pos, dst_pos, src_neg, dst_neg,
               cfg: Config):
    n = cfg.n_nodes
    feats = np.asarray(feats, np.float32)
    W = np.asarray(W, np.float32)
    b = np.asarray(b, np.float32)
    prelu_a = np.asarray(prelu_a, np.float32)
    bf16 = np.dtype("bfloat16")

    edge_lists = []
    wgts = []
    for src, dst in ((src_pos, dst_pos), (src_neg, dst_neg)):
        src = np.asarray(src, np.int64)
        dst = np.asarray(dst, np.int64)
        ns = _norm(np.bincount(src, minlength=n))
        nd = _norm(np.bincount(dst, minlength=n))
        # merge duplicate (src, dst) edges: by linearity their contributions
        # sum, so one gathered row with a summed weight is exact.
        pair = src * np.int64(1 << 32) + dst
        upair, ucnt = np.unique(pair, return_counts=True)
        usrc = (upair >> 32).astype(np.int64)
        udst = (upair & ((1 << 32) - 1)).astype(np.int64)
        edge_lists.append((usrc, udst))
        wgts.append((ucnt * ns[usrc] * nd[udst]).astype(np.float32))

    bank_of, table_row, bases, sizes = _assign_banks(edge_lists, cfg)

    feats_pad = np.zeros((cfg.n_pad, D), np.float32)
    feats_pad[table_row] = feats[:n]
    fb16 = feats_pad.astype(bf16)

    plans, layouts = [], []
    for (usrc, udst), wgt in zip(edge_lists, wgts):
        plan = _plan_graph(usrc, udst, wgt, bank_of, table_row, bases, cfg)
        plans.append(plan)
        layouts.append(_slot_layout(plan["cstar"], cfg))

    iota = np.tile(np.arange(P, dtype=np.float32), (P, 1)).astype(bf16)
    a_rep = np.full((P, 1), float(prelu_a.reshape(-1)[0]), np.float32)
    b_rep = np.tile(b.reshape(1, D), (P, 1)).astype(np.float32)
    use_bias = bool(np.any(b != 0.0))

    in_maps = []
    for core in range(cfg.n_cores):
        iw_p, off_p, wgt_p = _fill_core_graph(plans[0], layouts[0], core, cfg)
        iw_n, off_n, wgt_n = _fill_core_graph(plans[1], layouts[1], core, cfg)
        m = {
            "fb16": fb16,
            "w_in": W,
            "a_rep": a_rep,
            "idx_in": np.concatenate([iw_p, iw_n], axis=1),
            "off_in": np.concatenate([off_p, off_n], axis=1),
            "wgt_in": np.concatenate([wgt_p, wgt_n], axis=1),
            "iota_in": iota,
        }
        if use_bias:
            m["b_rep"] = b_rep
        in_maps.append(m)
    meta = {
        "layouts": layouts,
        "cstar": [plans[0]["cstar"], plans[1]["cstar"]],
        "use_bias": use_bias,
        "bank_bases": bases,
        "bank_sizes": sizes,
    }
    return in_maps, plans, meta


# --------------------------------------------------------------------------
# Device kernel builder
# --------------------------------------------------------------------------
def build_kernel(nc, tc, cfg: Config, meta):
    from contextlib import ExitStack

    import concourse.mybir as mybir

    f32 = mybir.dt.float32
    bf16 = mybir.dt.bfloat16
    i16 = mybir.dt.int16
    Alu = mybir.AluOpType
    Act = mybir.ActivationFunctionType

    tcore, npad = cfg.t_core, cfg.n_pad
    layouts = meta["layouts"]
    cstar = meta["cstar"]
    use_bias = meta["use_bias"]
    n_chunks = [layouts[g][3] for g in range(2)]
    ncols = [n_chunks[g] * P // 16 for g in range(2)]

    fb = nc.dram_tensor("fb16", [cfg.n_pad, D], bf16,
                        kind="ExternalInput").ap()
    w_in = nc.dram_tensor("w_in", [P, D], f32, kind="ExternalInput").ap()
    a_rep = nc.dram_tensor("a_rep", [P, 1], f32, kind="ExternalInput").ap()
    idx_in = nc.dram_tensor("idx_in", [P, sum(ncols)], i16,
                            kind="ExternalInput").ap()
    off_in = nc.dram_tensor("off_in", [P, sum(n_chunks)], f32,
                            kind="ExternalInput").ap()
    wgt_in = nc.dram_tensor("wgt_in", [P, sum(n_chunks)], f32,
                            kind="ExternalInput").ap()
    iota_in = nc.dram_tensor("iota_in", [P, P], bf16, kind="ExternalInput").ap()
    if use_bias:
        b_rep = nc.dram_tensor("b_rep", [P, D], f32, kind="ExternalInput").ap()
    out = nc.dram_tensor("out", [2, P, tcore * D], bf16,
                         kind="ExternalOutput").ap()

    with ExitStack() as ctx:
        const = ctx.enter_context(tc.tile_pool(name="const", bufs=1))
        gpool = ctx.enter_context(tc.tile_pool(name="gpool", bufs=cfg.gbufs))
        ipool = ctx.enter_context(tc.tile_pool(name="ipool", bufs=cfg.ipool_bufs))
        ohpool = ctx.enter_context(tc.tile_pool(name="ohpool",
                                                bufs=cfg.ohpool_bufs))
        atpool = ctx.enter_context(tc.tile_pool(name="atpool", bufs=3))
        tpool = ctx.enter_context(tc.tile_pool(name="tpool", bufs=4))
        spool = ctx.enter_context(tc.tile_pool(name="spool", bufs=3))
        apool = ctx.enter_context(tc.tile_pool(name="apool", bufs=3,
                                               space="PSUM"))
        hpool = ctx.enter_context(tc.tile_pool(name="hpool", bufs=3,
                                               space="PSUM"))

        # ---- constants ----
        w_sb = const.tile([P, D], bf16)
        nc.gpsimd.dma_start(out=w_sb[:], in_=w_in)  # f32 -> bf16 cast DMA
        iota_sb = const.tile([P, P], bf16)
        nc.sync.dma_start(out=iota_sb[:], in_=iota_in)
        a_sb = const.tile([P, 1], f32)
        nc.sync.dma_start(out=a_sb[:], in_=a_rep)
        if use_bias:
            b_sb = const.tile([P, D], f32)
            nc.sync.dma_start(out=b_sb[:], in_=b_rep)

        # ---- gather + weighted one-hot segment-sum + W + prelu ----
        col_base = [0, ncols[0]]          # idx column offset per graph
        chk_base = [0, n_chunks[0]]       # off/wgt column offset per graph
        cbs_all = []
        for g in range(2):
            calls_by_sg = {}
            for (si, b, c0, nch) in layouts[g][1]:
                calls_by_sg.setdefault(si, []).append((b, c0, nch))
            cbs_all.append(calls_by_sg)
        # interleave the two graphs' supergroups so one graph's gathers fill
        # DMA while the other's PSUM chain drains
        jobs = []
        for si in range(max(len(layouts[0][0]), len(layouts[1][0]))):
            for g in range(2):
                if si < len(layouts[g][0]):
                    jobs.append((g, si))
        oh_ct = 0
        for (g, si) in jobs:
            sg_list, calls, chunk_of, _ = layouts[g]
            cs = cstar[g]
            (k0, kn) = sg_list[si]
            sg_chunks = sum(int(cs[k0 + ki, b]) for ki in range(kn)
                            for b in range(NBANK))
            c0_sg = chunk_of[(k0, 0, 0)]
            gt = gpool.tile([P, sg_chunks, D], bf16, tag="gather")
            it = ipool.tile([P, sg_chunks * 8], i16, tag="gidx")
            nc.sync.dma_start(
                out=it[:],
                in_=idx_in[:, col_base[g] + c0_sg * 8 :
                           col_base[g] + (c0_sg + sg_chunks) * 8])
            ot = ipool.tile([P, sg_chunks], f32, tag="goff")
            nc.sync.dma_start(
                out=ot[:],
                in_=off_in[:, chk_base[g] + c0_sg :
                           chk_base[g] + c0_sg + sg_chunks])
            wt = ipool.tile([P, sg_chunks], f32, tag="gwgt")
            nc.sync.dma_start(
                out=wt[:],
                in_=wgt_in[:, chk_base[g] + c0_sg :
                           chk_base[g] + c0_sg + sg_chunks])
            bases = meta["bank_bases"]
            sizes = meta["bank_sizes"]
            for (b, c0, nch) in cbs_all[g][si]:
                lo = c0 - c0_sg
                nc.gpsimd.dma_gather(
                    out_ap=gt[:, lo : lo + nch, :],
                    in_ap=fb[int(bases[b]) : int(bases[b] + sizes[b]), :],
                    idxs_ap=it[:, lo * 8 : (lo + nch) * 8],
                    num_idxs=nch * P, num_idxs_reg=nch * P,
                    elem_size=D, single_packet=False)
            stg = spool.tile([P, kn, D], bf16, tag="stg")
            nb4 = 4 if cfg.act_batch else 1
            kq = 0
            while kq < kn:
                kb = min(nb4, kn - kq)
                agg4 = apool.tile([P, nb4, D], f32)
                for j in range(kb):
                    k = k0 + kq + j
                    nonzero = [(b, c) for b in range(NBANK)
                               for c in range(int(cs[k, b]))]
                    for ji, (b, c) in enumerate(nonzero):
                        ci = chunk_of[(k, b, c)]
                        lo = ci - c0_sg
                        oh = ohpool.tile([P, P], bf16)
                        eng = nc.vector
                        if cfg.oh_pool_mod:
                            oh_ct += 1
                            if oh_ct % cfg.oh_pool_mod == 0:
                                eng = nc.gpsimd
                        eng.tensor_scalar(
                            out=oh[:], in0=iota_sb[:],
                            scalar1=ot[:, lo : lo + 1],
                            scalar2=wt[:, lo : lo + 1],
                            op0=Alu.is_equal, op1=Alu.mult)
                        nc.tensor.matmul(
                            out=agg4[:, j, :], lhsT=gt[:, lo, :], rhs=oh[:],
                            start=(ji == 0), stop=(ji == len(nonzero) - 1))
                at4 = atpool.tile([P, nb4, D], bf16, tag="at4")
                nc.scalar.activation(out=at4[:, :kb, :], in_=agg4[:, :kb, :],
                                     func=Act.Copy)
                h4 = hpool.tile([P, nb4, D], f32)
                for j in range(kb):
                    nc.tensor.matmul(out=h4[:, j, :], lhsT=at4[:, j, :],
                                     rhs=w_sb[:], start=True, stop=True)
                if cfg.act_prelu and not use_bias:
                    nc.scalar.activation(
                        out=stg[:, kq : kq + kb, :], in_=h4[:, :kb, :],
                        func=Act.Prelu, alpha=a_sb[:, :1])
                elif not use_bias:
                    neg = tpool.tile([P, nb4, D], f32, tag="neg")
                    nc.vector.tensor_scalar(
                        out=neg[:, :kb, :], in0=h4[:, :kb, :], scalar1=0.0,
                        scalar2=a_sb[:, :1], op0=Alu.min, op1=Alu.mult)
                    pos = tpool.tile([P, nb4, D], f32, tag="pos")
                    nc.vector.tensor_scalar(
                        out=pos[:, :kb, :], in0=h4[:, :kb, :], scalar1=0.0,
                        scalar2=None, op0=Alu.max)
                    nc.vector.tensor_tensor(
                        out=stg[:, kq : kq + kb, :], in0=neg[:, :kb, :],
                        in1=pos[:, :kb, :], op=Alu.add)
                else:
                    hb = tpool.tile([P, nb4, D], f32, tag="hb")
                    nc.vector.tensor_tensor(
                        out=hb[:, :kb, :], in0=h4[:, :kb, :],
                        in1=b_sb[:, None, :].to_broadcast([P, kb, D]),
                        op=Alu.add)
                    neg = tpool.tile([P, nb4, D], f32, tag="neg")
                    nc.vector.tensor_scalar(
                        out=neg[:, :kb, :], in0=hb[:, :kb, :], scalar1=0.0,
                        scalar2=a_sb[:, :1], op0=Alu.min, op1=Alu.mult)
                    pos = tpool.tile([P, nb4, D], f32, tag="pos")
                    nc.vector.tensor_scalar(
                        out=pos[:, :kb, :], in0=hb[:, :kb, :], scalar1=0.0,
                        scalar2=None, op0=Alu.max)
                    nc.vector.tensor_tensor(
                        out=stg[:, kq : kq + kb, :], in0=neg[:, :kb, :],
                        in1=pos[:, :kb, :], op=Alu.add)
                kq += kb
            nc.sync.dma_start(
                out=out[g, :, k0 * D : (k0 + kn) * D],
                in_=stg[:].rearrange("p k d -> p (k d)"))
    return out


# --------------------------------------------------------------------------
# Driver
# --------------------------------------------------------------------------
def _build_program(cfg: Config, meta):
    import concourse.bacc as bacc
    import concourse.tile as tile

    nc = bacc.Bacc("TRN2", target_bir_lowering=False, debug=False,
                   enable_asserts=False, num_devices=cfg.n_cores)
    with tile.TileContext(nc) as tc:
        build_kernel(nc, tc, cfg, meta)
    nc.compile()
    return nc


def _unscramble(results, plans, cfg: Config):
    n = cfg.n_nodes
    full = np.zeros((2, n, D), np.float32)
    for g in range(2):
        ct_all = plans[g]["core_tiles"]
        for core in range(cfg.n_cores):
            # [2, P, t_core*D] bf16
            oc = np.asarray(results[core]["out"]).astype(np.float32)
            for k in range(cfg.t_core):
                t = int(ct_all[core, k])
                if t < 0:
                    continue
                r0 = t * P
                r1 = min(r0 + P, n)
                full[g, r0:r1] = oc[g, : r1 - r0, k * D : (k + 1) * D]
    return full


_PROGRAM_CACHE = {}


def run(inputs, cfg: Config, trace=False):
    from concourse.bass_utils import run_bass_kernel_spmd

    in_maps, plans, meta = preprocess(
        inputs["feats"], inputs["W"], inputs["b"], inputs["prelu_a"],
        inputs["src_pos"], inputs["dst_pos"],
        inputs["src_neg"], inputs["dst_neg"], cfg)

    key = (cfg.n_nodes, cfg.n_cores, cfg.sg, cfg.sg_first, cfg.sg_tail,
           cfg.act_prelu, cfg.act_batch,
           cfg.oh_pool_mod, cfg.gbufs, cfg.maxc,
           meta["cstar"][0].tobytes(), meta["cstar"][1].tobytes(),
           meta["bank_sizes"].tobytes(), meta["use_bias"])
    nc = _PROGRAM_CACHE.get(key)
    if nc is None:
        nc = _build_program(cfg, meta)
        _PROGRAM_CACHE[key] = nc

    kwargs = {}
    if trace:
        kwargs = dict(trace=True, tmpdir=tempfile.mkdtemp(prefix="bgc_trace_"))
    res = run_bass_kernel_spmd(nc, in_maps, core_ids=list(range(cfg.n_cores)),
                               **kwargs)
    full = _unscramble(res.results, plans, cfg)
    return full, res


def kernel(**inputs) -> np.ndarray:
    cfg = Config()
    full, _ = run(inputs, cfg)
    return full
